# revision 1
# baseline (speedup 1.0000x reference)
"""Trainium2 Bass kernel for nn_LlamaAttention_cam (sparse_attention).

Sharding: 16 heads across 8 cores (2 heads/core), both batches per core.
Q/K/V projections column-parallel over heads; o_proj row-parallel (per-core
partial outputs summed on host). The CaM merge is a rank-1 correction
(s_tail outer v_e) applied on host from tiny device-side statistics.

Self-contained: hardcodes all shapes; takes full inputs, returns full output.
"""

import math
import os

import numpy as np
import ml_dtypes

B, T, HID, H = 2, 2048, 2048, 16
D = 128
NCORES = 8
HL = H // NCORES  # heads per core = 2
BT = B * T  # 4096
NF = HID // 128  # 16 f-tiles
SCALE = 1.0 / math.sqrt(D)
RB = int(0.25 * T)  # 512 recent budget
WS = T - RB  # 1536
EVICT = WS - 1  # 1535

# jax.random.uniform(jax.random.key(42), (2,16), float32); bernoulli(key,p) == u < p
U_CONST = np.array(
    [[0.59400654, 0.43801308, 0.6285691, 0.00791204, 0.27834702,
      0.7976179, 0.8521497, 0.9625306, 0.67656493, 0.11104441,
      0.4959929, 0.7311437, 0.18970704, 0.1544199, 0.03802836,
      0.33559263],
     [0.92825687, 0.6123972, 0.49262476, 0.733806, 0.18920851,
      0.15386605, 0.037136197, 0.32930005, 0.9372028, 0.5957513,
      0.4615929, 0.6695677, 0.07019377, 0.39408123, 0.55786455,
      0.35412872]], dtype=np.float32)

BF16 = ml_dtypes.bfloat16

_NC_CACHE = {}


def build_nc():
    import concourse.bacc as bacc
    import concourse.mybir as mybir
    import concourse.tile as tile

    f32 = mybir.dt.float32
    bf16 = mybir.dt.bfloat16
    EXP = mybir.ActivationFunctionType.Exp

    nc = bacc.Bacc("TRN2", target_bir_lowering=False, debug=False)
    env = os.environ
    B_QK = int(env.get("BK_QK", "5"))
    B_HSP = int(env.get("BK_HSP", "2"))
    B_ROPE = int(env.get("BK_ROPE", "3"))
    B_SPS = int(env.get("BK_SPS", "3"))
    B_OPS = int(env.get("BK_OPS", "1"))
    B_MSC = int(env.get("BK_MSC", "3"))
    B_PT = int(env.get("BK_PT", "10"))
    B_PR = int(env.get("BK_PR", "3"))
    B_OB = int(env.get("BK_OB", "8"))
    LOOK = int(env.get("BK_LOOK", "3"))
    ILV = env.get("BK_ILV", "1") == "1"
    PAIR_POOL = env.get("BK_PAIRP", "0") == "1"
    CPY = env.get("BK_CPY", "vs")  # per-fo copy engine cycle: v=DVE s=Act
    OBQ = env.get("BK_OBQ", "0") == "1"  # alternate ob DMA queues
    ILVN = int(env.get("BK_ILVN", "1"))  # oproj units per 2 j-steps

    hsT = nc.dram_tensor("hsT", [HID, BT], bf16, kind="ExternalInput")
    wq = nc.dram_tensor("wq", [HID, 256], bf16, kind="ExternalInput")
    wk = nc.dram_tensor("wk", [HID, 256], bf16, kind="ExternalInput")
    wv = nc.dram_tensor("wv", [HID, 256], bf16, kind="ExternalInput")
    wo = nc.dram_tensor("wo", [256, HID], bf16, kind="ExternalInput")
    cosd = nc.dram_tensor("cosT", [128, T], f32, kind="ExternalInput")
    sind = nc.dram_tensor("sinT", [128, T], f32, kind="ExternalInput")
    maskd = nc.dram_tensor("masks", [128, 2048], bf16, kind="ExternalInput")

    outT = nc.dram_tensor("outT", [HID, BT], bf16, kind="ExternalOutput")
    abard = nc.dram_tensor("abar", [4, 128, 16], f32, kind="ExternalOutput")
    sumsd = nc.dram_tensor("sums", [4, 2, T], f32, kind="ExternalOutput")

    with tile.TileContext(nc) as tc:
        with (
            tc.tile_pool(name="singles", bufs=1) as singles,
            tc.tile_pool(name="res", bufs=1) as res,
            tc.tile_pool(name="stats", bufs=1) as stats,
        ):
            # --- small constants (no DMA deps) for the PE warm-up ---
            ones_a = singles.tile([128, 2], bf16, tag="onesa")  # [1, 0]
            ones_b = singles.tile([128, 2], bf16, tag="onesb")  # [1, 1]
            nc.vector.memset(ones_a[:, 0:1], 1.0)
            nc.vector.memset(ones_a[:, 1:2], 0.0)
            nc.vector.memset(ones_b, 1.0)
            wsrc = singles.tile([128, 16], bf16, tag="wsrc")
            nc.vector.memset(wsrc, 0.0)

            # --- constant tiles ---
            wq_sb = singles.tile([128, NF, 256], bf16, tag="wq")
            wk_sb = singles.tile([128, NF, 256], bf16, tag="wk")
            wv_sb = singles.tile([128, NF, 256], bf16, tag="wv")
            wo_sb = singles.tile([128, 2, HID], bf16, tag="wo")
            cos_sb = singles.tile([128, T], f32, tag="cos")
            sin_sb = singles.tile([128, T], f32, tag="sin")
            mask_sb = singles.tile([128, 4, 512], bf16, tag="mask")

            # --- residents ---
            qt = [res.tile([128, BT], bf16, tag=f"qt{h}", name=f"qt{h}")
                  for h in range(HL)]
            kt = [res.tile([128, BT], bf16, tag=f"kt{h}", name=f"kt{h}")
                  for h in range(HL)]
            vres = res.tile([128, 32, 256], bf16, tag="vres")
            ot = [res.tile([128, T], bf16, tag=f"ot{p}", name=f"ot{p}")
                  for p in range(4)]
            abar_raw = [stats.tile([128, 16], f32, tag=f"ab{p}", name=f"ab{p}")
                        for p in range(4)]

            # ================= Phase 1: QKV projections + RoPE ================
            with (
                tc.tile_pool(name="hsp", bufs=B_HSP) as hsp,
                tc.tile_pool(name="rope", bufs=B_ROPE) as rope,
                tc.tile_pool(name="qkps", bufs=B_QK, space="PSUM") as qkps,
                tc.tile_pool(name="vps", bufs=2, space="PSUM") as vps,
                tc.tile_pool(name="wps", bufs=1, space="PSUM") as wps,
            ):
                # PE warm-up: a tiny matmul at t~0 starts the p-state ramp so
                # the real matmuls (gated on DMA) run at full clock.
                warm = wps.tile([2, 16], f32, tag="warm")
                nc.tensor.matmul(warm, lhsT=ones_a, rhs=wsrc,
                                 start=True, stop=True)

                # DMA issue order is the sync-queue service order; front-load
                # exactly what the first matmuls need (wq + hs chunk 0).
                hs0 = hsp.tile([128, NF, 512], bf16, tag="hs", name="hs0")
                nc.sync.dma_start(
                    out=wq_sb[:, 0:8, :],
                    in_=wq[0:1024, :].rearrange("(nf p) d -> p nf d", p=128))
                nc.sync.dma_start(
                    out=hs0[:, 0:8, :],
                    in_=hsT[0:1024, 0:512].rearrange("(nf p) t -> p nf t",
                                                     p=128))
                nc.sync.dma_start(
                    out=wq_sb[:, 8:16, :],
                    in_=wq[1024:2048, :].rearrange("(nf p) d -> p nf d",
                                                   p=128))
                nc.sync.dma_start(
                    out=hs0[:, 8:16, :],
                    in_=hsT[1024:2048, 0:512].rearrange("(nf p) t -> p nf t",
                                                        p=128))
                nc.sync.dma_start(
                    out=wv_sb, in_=wv.rearrange("(nf p) d -> p nf d", p=128))
                nc.sync.dma_start(
                    out=wk_sb, in_=wk.rearrange("(nf p) d -> p nf d", p=128))
                # hs chunk 1 must beat the chunk-0 compute tail; cos/sin only
                # gate RoPE (vector work), mask/wo only phase 2.
                hs1 = hsp.tile([128, NF, 512], bf16, tag="hs", name="hs1")
                nc.sync.dma_start(
                    out=hs1,
                    in_=hsT[:, 512:1024].rearrange("(nf p) t -> p nf t",
                                                   p=128))
                nc.sync.dma_start(out=cos_sb, in_=cosd[:, :])
                nc.sync.dma_start(out=sin_sb, in_=sind[:, :])
                nc.sync.dma_start(
                    out=mask_sb, in_=maskd.rearrange("p (v t) -> p v t", v=4))
                nc.sync.dma_start(
                    out=wo_sb, in_=wo.rearrange("(kt p) f -> p kt f", p=128))

                def rope_apply(ps, dest, tl, swap_eng=nc.gpsimd,
                               late=False):
                    qf = rope.tile([128, 512], f32, tag="qf")
                    if late:
                        nc.vector.tensor_copy(qf, ps)
                    else:
                        nc.scalar.copy(qf, ps)
                    rot = rope.tile([128, 512], f32, tag="rot")
                    swap_eng.dma_start(out=rot[0:64, :], in_=qf[64:128, :])
                    swap_eng.dma_start(out=rot[64:128, :], in_=qf[0:64, :])
                    t1 = rope.tile([128, 512], f32, tag="t1")
                    nc.vector.tensor_mul(t1, rot, sin_sb[:, tl])
                    t2 = rope.tile([128, 512], f32, tag="t2")
                    nc.vector.tensor_mul(t2, qf, cos_sb[:, tl])
                    nc.vector.tensor_add(dest, t1, t2)

                for c in range(8):
                    cs = slice(c * 512, (c + 1) * 512)
                    tl = slice((c % 4) * 512, (c % 4) * 512 + 512)
                    if c == 0:
                        hs_t = hs0
                    elif c == 1:
                        hs_t = hs1
                    else:
                        hs_t = hsp.tile([128, NF, 512], bf16, tag="hs")
                        nc.sync.dma_start(
                            out=hs_t,
                            in_=hsT[:, cs].rearrange("(nf p) t -> p nf t",
                                                     p=128))
                    # Q (h0, h1) -> V -> K (h0, h1): matches DMA arrivals.
                    # Chunk 0: interleave the two Q chains' f-halves so h1's
                    # f0-7 (first-half DMAs only) runs while the second wq/hs
                    # halves are still in flight.
                    if c == 0:
                        qps = [qkps.tile([128, 512], f32, tag="qk",
                                         name=f"qps{h}") for h in range(HL)]
                        for fh in range(2):
                            for h in range(HL):
                                for f in range(fh * 8, fh * 8 + 8):
                                    nc.tensor.matmul(
                                        qps[h],
                                        lhsT=wq_sb[:, f,
                                                   h * 128:(h + 1) * 128],
                                        rhs=hs_t[:, f, :],
                                        start=(f == 0), stop=(f == NF - 1))
                        for h in range(HL):
                            rope_apply(qps[h], qt[h][:, cs], tl, nc.gpsimd)
                    else:
                        for h in range(HL):
                            ps = qkps.tile([128, 512], f32, tag="qk")
                            for f in range(NF):
                                nc.tensor.matmul(
                                    ps,
                                    lhsT=wq_sb[:, f, h * 128:(h + 1) * 128],
                                    rhs=hs_t[:, f, :],
                                    start=(f == 0), stop=(f == NF - 1))
                            rope_apply(ps, qt[h][:, cs], tl,
                                       nc.gpsimd if c < 5 else nc.sync,
                                       late=(c >= 6))
                    for s in range(4):
                        vp = vps.tile([128, 256], f32, tag="v")
                        for f in range(NF):
                            nc.tensor.matmul(
                                vp,
                                lhsT=hs_t[:, f, s * 128:(s + 1) * 128],
                                rhs=wv_sb[:, f, :],
                                start=(f == 0), stop=(f == NF - 1))
                        # keep Act free near the phase boundary: route the
                        # last chunks' V copies through DVE
                        if c >= 6:
                            nc.vector.tensor_copy(vres[:, c * 4 + s, :], vp)
                        else:
                            nc.scalar.copy(vres[:, c * 4 + s, :], vp)
                    for h in range(HL):
                        ps = qkps.tile([128, 512], f32, tag="qk")
                        for f in range(NF):
                            nc.tensor.matmul(
                                ps,
                                lhsT=wk_sb[:, f, h * 128:(h + 1) * 128],
                                rhs=hs_t[:, f, :],
                                start=(f == 0), stop=(f == NF - 1))
                        rope_apply(ps, kt[h][:, cs], tl,
                                   nc.gpsimd if c < 5 else nc.sync,
                                   late=(c >= 6))

            # ========== Phase 2+3: attention + interleaved o_proj ==========
            with (
                tc.tile_pool(name="sps", bufs=B_SPS, space="PSUM") as sps,
                tc.tile_pool(name="ops", bufs=B_OPS, space="PSUM") as ops,
                tc.tile_pool(name="msc", bufs=B_MSC, space="PSUM") as msc,
                tc.tile_pool(name="smp", bufs=1, space="PSUM") as smp,
                tc.tile_pool(name="pt", bufs=B_PT) as ptp,
                tc.tile_pool(name="pr", bufs=B_PR) as prp,
                tc.tile_pool(name="att_sm", bufs=int(env.get("BK_SM", "4"))) as atsm,
                tc.tile_pool(name="ob", bufs=B_OB) as obp,
            ):
                pending = []

                def emit_unit():
                    if pending:
                        pending.pop(0)()

                def flush_units():
                    while pending:
                        pending.pop(0)()

                def enqueue_oproj(b, c, gsz=4):
                    tl = slice(c * 512, (c + 1) * 512)
                    state = {}
                    # GPSIMD cannot read PSUM; mix DVE/Act (Act carries exps)
                    copy_engines = [(nc.vector.tensor_copy if ch == "v"
                                     else nc.scalar.copy) for ch in CPY]

                    def unit(fo):
                        def f():
                            g = fo // gsz
                            if g not in state:
                                state[g] = obp.tile([128, gsz, 512], bf16,
                                                    tag=f"ob{gsz}",
                                                    name="ob_t")
                            ob_t = state[g]
                            fs = slice(fo * 128, (fo + 1) * 128)
                            pp = msc.tile([128, 512], f32, tag="pp")
                            nc.tensor.matmul(
                                pp, lhsT=wo_sb[:, 0, fs],
                                rhs=ot[b * 2 + 0][:, tl],
                                start=True, stop=False)
                            nc.tensor.matmul(
                                pp, lhsT=wo_sb[:, 1, fs],
                                rhs=ot[b * 2 + 1][:, tl],
                                start=False, stop=True)
                            copy_engines[fo % len(CPY)](
                                ob_t[:, fo % gsz, :], pp)
                            if fo % gsz == gsz - 1:
                                rows = slice(g * gsz * 128,
                                             (g + 1) * gsz * 128)
                                cg = slice((b * 4 + c) * 512,
                                           (b * 4 + c + 1) * 512)
                                eng = (nc.gpsimd if (OBQ and g % 2 == 1)
                                       else nc.sync)
                                eng.dma_start(
                                    out=outT[rows, cg].rearrange(
                                        "(nf p) t -> p nf t", p=128),
                                    in_=ob_t)
                        return f

                    for fo in range(16):
                        pending.append(unit(fo))

                # Global step stream: score-matmul lookahead crosses chunk
                # boundaries so the next chunk's exps run during the previous
                # chunk's tail (norm chain / oproj flush) with no PE bubble.
                class Chunk:
                    def __init__(self, p, c):
                        self.p, self.c = p, c
                        self.b, self.h = p // 2, p % 2
                        self.jmax = 4 * (c + 1)
                        self.o_ps = None
                        self.sm_ps = None
                        self.sm_started = False
                        self.pts = {}
                        self.prs = []
                        self.sq = []

                def tile_off(ck, j):
                    # Diagonal k-tile v=1..3: first 128v query cols are fully
                    # causal-masked -> compute only cols [128v:512]. Exact.
                    v = j - 4 * ck.c
                    return 128 * v if 1 <= v <= 3 else 0

                def emit_s(ck, j):
                    b, c = ck.b, ck.c
                    off = tile_off(ck, j)
                    sp = sps.tile([128, 512], f32, tag="s", name="sp")
                    nc.tensor.matmul(
                        sp[:, off:],
                        lhsT=kt[ck.h][:, b * T + j * 128:
                                      b * T + (j + 1) * 128],
                        rhs=qt[ck.h][:, b * T + c * 512 + off:
                                     b * T + (c + 1) * 512],
                        start=True, stop=True)
                    ck.sq.append(sp)

                def emit_epv(ck, j):
                    p, c, b, h = ck.p, ck.c, ck.b, ck.h
                    off = tile_off(ck, j)
                    sp = ck.sq[j]
                    pt_t = ptp.tile([128, 512], bf16, tag="p", name="pt_t")
                    nc.scalar.activation(pt_t[:, off:], sp[:, off:],
                                         EXP, scale=SCALE)
                    if j >= 4 * c:
                        nc.vector.tensor_mul(pt_t[:, off:], pt_t[:, off:],
                                             mask_sb[:, j - 4 * c, off:])
                    if c == 3:
                        nc.vector.tensor_copy(
                            abar_raw[p][:, j:j + 1], sp[:, 511:512])
                    if ck.o_ps is None:
                        ck.o_ps = ops.tile([128, 512], f32, tag="o",
                                           name="o_ps")
                    nc.tensor.matmul(
                        ck.o_ps[:, off:],
                        lhsT=vres[:, b * 16 + j, h * 128:(h + 1) * 128],
                        rhs=pt_t[:, off:],
                        start=(j == 0), stop=(j == ck.jmax - 1))
                    if ck.sm_ps is None:
                        ck.sm_ps = smp.tile([2, 512], f32, tag="sm",
                                            name="sm_ps")
                    if j >= 4 * c:
                        # diagonal tile: individual (possibly trimmed) rowsum
                        nc.tensor.matmul(
                            ck.sm_ps[:, off:],
                            lhsT=(ones_b if c == 3 else ones_a),
                            rhs=pt_t[:, off:],
                            start=(j == 4 * c and c == 0),
                            stop=(j == ck.jmax - 1))
                        return
                    ck.pts[j] = pt_t
                    if j % 2 == 1:
                        pr = prp.tile([128, 512], bf16, tag="pr", name="pr")
                        nc.vector.tensor_add(pr, ck.pts[j - 1], ck.pts[j])
                        nc.tensor.matmul(
                            ck.sm_ps,
                            lhsT=ones_a,
                            rhs=pr,
                            start=(j == 1), stop=False)
                        del ck.pts[j - 1], ck.pts[j]

                def epilogue(ck):
                    p, c = ck.p, ck.c
                    cl = slice(c * 512, (c + 1) * 512)
                    rec = atsm.tile([1, 512], f32, tag="rec", name="rec")
                    nc.vector.reciprocal(rec, ck.sm_ps[0:1, :])
                    bc = atsm.tile([128, 512], f32, tag="bc", name="bc")
                    nc.gpsimd.partition_broadcast(bc, rec)
                    nc.vector.tensor_mul(ot[p][:, cl], ck.o_ps, bc)
                    sm_sb = atsm.tile([2, 512], f32, tag="smsb", name="sm_sb")
                    nc.vector.tensor_copy(sm_sb, ck.sm_ps)
                    nc.sync.dma_start(out=sumsd[p, :, cl], in_=sm_sb)
                    if c == 3:
                        ab_exp = atsm.tile([128, 16], f32, tag="abe",
                                           name="ab_exp")
                        nc.scalar.activation(
                            ab_exp, abar_raw[p], EXP, scale=SCALE)
                        nc.sync.dma_start(out=abard[p], in_=ab_exp)

                chunks = [Chunk(b * 2 + hl, c)
                          for b in range(B) for c in range(4)
                          for hl in range(HL)]
                steps = [(ck, j) for ck in chunks for j in range(ck.jmax)]
                for k in range(LOOK):
                    emit_s(*steps[k])
                for i, (ck, j) in enumerate(steps):
                    if i + LOOK < len(steps):
                        emit_s(*steps[i + LOOK])
                    emit_epv(ck, j)
                    if ILV and j % 2 == 1:
                        for _ in range(ILVN):
                            emit_unit()
                    if j == ck.jmax - 1:
                        epilogue(ck)
                        if ck.h == 1:
                            flush_units()
                            enqueue_oproj(
                                ck.b, ck.c,
                                gsz=(2 if (ck.b, ck.c) == (1, 3) else 4))
                flush_units()

    nc.compile()
    return nc


def _get_nc():
    if "nc" not in _NC_CACHE:
        _NC_CACHE["nc"] = build_nc()
    return _NC_CACHE["nc"]


def _host_inputs(hidden_states, q_w, k_w, v_w, o_w):
    """Per-core input dicts."""
    hsT = np.ascontiguousarray(
        hidden_states.reshape(BT, HID).T).astype(BF16)
    inv = 10000.0 ** (-np.arange(64, dtype=np.float64) / 64.0)
    t = np.arange(T, dtype=np.float64)
    fr = t[None, :] * inv[:, None]  # [64, T]
    cosT = np.cos(np.concatenate([fr, fr], 0)).astype(np.float32)
    sinT = np.sin(np.concatenate([fr, fr], 0)).astype(np.float32)
    sinT[:64] *= -1.0  # sign-baked for swap-halves rotate
    masks = np.zeros((128, 4, 512), dtype=np.float32)
    kk = np.arange(128)[:, None]
    tt = np.arange(512)[None, :]
    for v in range(4):
        masks[:, v, :] = (tt >= 128 * v + kk).astype(np.float32)
    masks = masks.reshape(128, 2048).astype(BF16)

    in_maps = []
    for core in range(NCORES):
        rs = slice(core * 256, (core + 1) * 256)
        in_maps.append({
            "hsT": hsT,
            "wq": np.ascontiguousarray(q_w[rs, :].T).astype(BF16),
            "wk": np.ascontiguousarray(k_w[rs, :].T).astype(BF16),
            "wv": np.ascontiguousarray(v_w[rs, :].T).astype(BF16),
            "wo": np.ascontiguousarray(o_w[:, rs].T).astype(BF16),
            "cosT": cosT,
            "sinT": sinT,
            "masks": masks,
        })
    return in_maps


def _epilogue(out, results, hidden_states, v_w, o_w):
    """Add the CaM rank-1 correction per (b, h) on host."""
    for core in range(NCORES):
        r = results[core]
        for p in range(4):
            b, hl = p // 2, p % 2
            h = core * HL + hl
            rowsum = r["sums"][p][0]  # [T] unnormalized exp row sums
            tails = r["sums"][p][1]
            a_exp = np.asarray(r["abar"][p], np.float64).T.reshape(2048)
            a_bar = a_exp / max(float(rowsum[T - 1]), 1e-30)
            avg_w = max(float(np.mean(a_bar[WS:])), 1e-6)
            prob = float(np.clip(a_bar[EVICT] / avg_w, 0.0, 1.0))
            prob = float(np.nan_to_num(prob, nan=0.0, posinf=1.0, neginf=0.0))
            m = 1.0 if U_CONST[b, h] < prob else 0.0
            if m == 0.0:
                continue
            # exact v_e from fp32 inputs
            v_row = hidden_states[b, EVICT, :] @ v_w[h * D:(h + 1) * D, :].T
            v_e = v_row * (m / RB)  # [D]
            w_e = o_w[:, h * D:(h + 1) * D] @ v_e  # [HID]
            s_tail = (tails / np.maximum(rowsum, 1e-30)).astype(np.float32)
            out[b] += np.outer(s_tail, w_e).astype(np.float32)
    return out


def kernel(hidden_states, attention_mask, q_w, k_w, v_w, o_w):
    from concourse.bass_utils import run_bass_kernel_spmd

    nc = _get_nc()
    in_maps = _host_inputs(hidden_states, q_w, k_w, v_w, o_w)
    trace = bool(int(os.environ.get("BK_TRACE", "0")))
    res = run_bass_kernel_spmd(
        nc, in_maps, core_ids=list(range(NCORES)), trace=trace,
    )
    if trace and res.exec_time_ns is not None:
        print(f"HW exec time: {res.exec_time_ns} ns")
        _NC_CACHE["last_exec_ns"] = res.exec_time_ns
        _NC_CACHE["last_trace"] = res.instructions_and_trace
    results = res.results

    acc = np.zeros((HID, BT), dtype=np.float32)
    for core in range(NCORES):
        acc += np.asarray(results[core]["outT"], np.float32)
    out = np.ascontiguousarray(acc.T).reshape(B, T, HID)
    out = _epilogue(out, results, hidden_states, v_w, o_w)
    return out.astype(np.float32)



# revision 4
# speedup vs baseline: 1.0448x; 1.0448x over previous
"""Trainium2 Bass kernel for nn_LlamaAttention_cam (sparse_attention).

Sharding: 16 heads across 8 cores (2 heads/core), both batches per core.
Q/K/V projections column-parallel over heads; o_proj row-parallel (per-core
partial outputs summed on host). The CaM merge is a rank-1 correction
(s_tail outer v_e) applied on host from tiny device-side statistics.

The projection GEMMs (QKV + o_proj) run as fp8e4 DoubleRow matmuls with
3-term error compensation: X*W ~ Xh*Wh + Xl*Wh + Xh*Wl where Xh = fp8(X),
Xl = fp8(X - Xh). DoubleRow packs a 256-deep contraction at 0.5 cyc/col,
so 3 terms cost 75% of the bf16 equivalent. hs and all weights are split
on the host (free); attn_out is split on-device (Pool engine). Weights are
pre-scaled by 64 into fp8 range; V inherits x64 which puts attn_out in
fp8 range too; the o_proj epilogue copy descales by 1/4096.

Self-contained: hardcodes all shapes; takes full inputs, returns full output.
"""

import math
import os

import numpy as np
import ml_dtypes

B, T, HID, H = 2, 2048, 2048, 16
D = 128
NCORES = 8
HL = H // NCORES  # heads per core = 2
BT = B * T  # 4096
NF = HID // 128  # 16 f-tiles
NG = NF // 2  # 8 f-tile pairs for DoubleRow
SCALE = 1.0 / math.sqrt(D)
RB = int(0.25 * T)  # 512 recent budget
WS = T - RB  # 1536
EVICT = WS - 1  # 1535
WSCL = 64.0  # fp8 pre-scale on wq/wk/wo
VSCL = 16.0  # fp8 pre-scale on wv: max |attn_out*VSCL| ~ 5sigma*16 = 72 < 240
ODESC = 1.0 / (VSCL * WSCL)  # o_proj descale: V carries x16, wo carries x64

# jax.random.uniform(jax.random.key(42), (2,16), float32); bernoulli(key,p) == u < p
U_CONST = np.array(
    [[0.59400654, 0.43801308, 0.6285691, 0.00791204, 0.27834702,
      0.7976179, 0.8521497, 0.9625306, 0.67656493, 0.11104441,
      0.4959929, 0.7311437, 0.18970704, 0.1544199, 0.03802836,
      0.33559263],
     [0.92825687, 0.6123972, 0.49262476, 0.733806, 0.18920851,
      0.15386605, 0.037136197, 0.32930005, 0.9372028, 0.5957513,
      0.4615929, 0.6695677, 0.07019377, 0.39408123, 0.55786455,
      0.35412872]], dtype=np.float32)

BF16 = ml_dtypes.bfloat16
F8 = ml_dtypes.float8_e4m3

_NC_CACHE = {}


def build_nc():
    import concourse.bacc as bacc
    import concourse.mybir as mybir
    import concourse.tile as tile

    f32 = mybir.dt.float32
    bf16 = mybir.dt.bfloat16
    f8 = mybir.dt.float8e4
    EXP = mybir.ActivationFunctionType.Exp
    DR = mybir.MatmulPerfMode.DoubleRow

    nc = bacc.Bacc("TRN2", target_bir_lowering=False, debug=False)
    env = os.environ
    B_QK = int(env.get("BK_QK", "5"))
    B_HSP = int(env.get("BK_HSP", "2"))
    B_ROPE = int(env.get("BK_ROPE", "3"))
    B_SPS = int(env.get("BK_SPS", "3"))
    B_OPS = int(env.get("BK_OPS", "1"))
    B_MSC = int(env.get("BK_MSC", "3"))
    B_PT = int(env.get("BK_PT", "10"))
    B_PR = int(env.get("BK_PR", "3"))
    B_OB = int(env.get("BK_OB", "8"))
    LOOK = int(env.get("BK_LOOK", "3"))
    ILV = env.get("BK_ILV", "1") == "1"
    CPY = env.get("BK_CPY", "vvs")  # per-fo copy engine cycle: v=DVE s=Act
    OBQ = env.get("BK_OBQ", "0") == "1"  # alternate ob DMA queues
    ILVN = int(env.get("BK_ILVN", "1"))  # oproj units per 2 j-steps

    hsh = nc.dram_tensor("hsh", [HID, BT], f8, kind="ExternalInput")
    hsl = nc.dram_tensor("hsl", [HID, BT], f8, kind="ExternalInput")
    wqh = nc.dram_tensor("wqh", [HID, 256], f8, kind="ExternalInput")
    wql = nc.dram_tensor("wql", [HID, 256], f8, kind="ExternalInput")
    wkh = nc.dram_tensor("wkh", [HID, 256], f8, kind="ExternalInput")
    wkl = nc.dram_tensor("wkl", [HID, 256], f8, kind="ExternalInput")
    wvh = nc.dram_tensor("wvh", [HID, 256], f8, kind="ExternalInput")
    wvl = nc.dram_tensor("wvl", [HID, 256], f8, kind="ExternalInput")
    woh = nc.dram_tensor("woh", [256, HID], f8, kind="ExternalInput")
    wol = nc.dram_tensor("wol", [256, HID], f8, kind="ExternalInput")
    cosd = nc.dram_tensor("cosT", [128, T], f32, kind="ExternalInput")
    sind = nc.dram_tensor("sinT", [128, T], f32, kind="ExternalInput")
    maskd = nc.dram_tensor("masks", [128, 2048], bf16, kind="ExternalInput")

    outT = nc.dram_tensor("outT", [HID, BT], bf16, kind="ExternalOutput")
    abard = nc.dram_tensor("abar", [4, 128, 16], f32, kind="ExternalOutput")
    sumsd = nc.dram_tensor("sums", [4, 2, T], f32, kind="ExternalOutput")

    with tile.TileContext(nc) as tc:
        with (
            tc.tile_pool(name="singles", bufs=1) as singles,
            tc.tile_pool(name="res", bufs=1) as res,
            tc.tile_pool(name="stats", bufs=1) as stats,
        ):
            # --- small constants (no DMA deps) for the PE warm-up ---
            ones_a = singles.tile([128, 2], bf16, tag="onesa")  # [1, 0]
            ones_b = singles.tile([128, 2], bf16, tag="onesb")  # [1, 1]
            nc.vector.memset(ones_a[:, 0:1], 1.0)
            nc.vector.memset(ones_a[:, 1:2], 0.0)
            nc.vector.memset(ones_b, 1.0)
            wsrc = singles.tile([128, 16], bf16, tag="wsrc")
            nc.vector.memset(wsrc, 0.0)

            # --- constant tiles (fp8 hi/lo weight pairs) ---
            wq_sb = [singles.tile([128, NF, 256], f8, tag=f"wq{i}",
                                   name=f"wq{i}")
                     for i in range(2)]
            wk_sb = [singles.tile([128, NF, 256], f8, tag=f"wk{i}",
                                   name=f"wk{i}")
                     for i in range(2)]
            wv_sb = [singles.tile([128, NF, 256], f8, tag=f"wv{i}",
                                   name=f"wv{i}")
                     for i in range(2)]
            wo_sb = [singles.tile([128, 2, HID], f8, tag=f"wo{i}",
                                   name=f"wo{i}")
                     for i in range(2)]
            cos_sb = singles.tile([128, T], f32, tag="cos")
            sin_sb = singles.tile([128, T], f32, tag="sin")
            mask_sb = singles.tile([128, 4, 512], bf16, tag="mask")

            # --- residents ---
            qt = [res.tile([128, BT], bf16, tag=f"qt{h}", name=f"qt{h}")
                  for h in range(HL)]
            kt = [res.tile([128, BT], bf16, tag=f"kt{h}", name=f"kt{h}")
                  for h in range(HL)]
            vres = res.tile([128, 32, 256], bf16, tag="vres")
            # attn_out hi/lo fp8, [d, head, t] per batch
            aoh = [res.tile([128, 2, T], f8, tag=f"aoh{b}", name=f"aoh{b}")
                   for b in range(B)]
            aol = [res.tile([128, 2, T], f8, tag=f"aol{b}", name=f"aol{b}")
                   for b in range(B)]
            abar_raw = [stats.tile([128, 16], f32, tag=f"ab{p}", name=f"ab{p}")
                        for p in range(4)]

            # ================= Phase 1: QKV projections + RoPE ================
            with (
                tc.tile_pool(name="hsp", bufs=B_HSP) as hsp,
                tc.tile_pool(name="rope", bufs=B_ROPE) as rope,
                tc.tile_pool(name="qkps", bufs=B_QK, space="PSUM") as qkps,
                tc.tile_pool(name="vps", bufs=2, space="PSUM") as vps,
                tc.tile_pool(name="wps", bufs=1, space="PSUM") as wps,
            ):
                # PE warm-up: a tiny matmul at t~0 starts the p-state ramp so
                # the real matmuls (gated on DMA) run at full clock.
                warm = wps.tile([2, 16], f32, tag="warm")
                nc.tensor.matmul(warm, lhsT=ones_a, rhs=wsrc,
                                 start=True, stop=True)

                # DMA issue order is the sync-queue service order; front-load
                # exactly what the first matmuls need (wq_hi + hs_hi chunk 0).
                hs0 = [hsp.tile([128, NF, 512], f8, tag=f"hs{i}",
                                name=f"hs0{i}") for i in range(2)]
                nc.sync.dma_start(
                    out=wq_sb[0][:, 0:8, :],
                    in_=wqh[0:1024, :].rearrange("(nf p) d -> p nf d", p=128))
                nc.sync.dma_start(
                    out=hs0[0][:, 0:8, :],
                    in_=hsh[0:1024, 0:512].rearrange("(nf p) t -> p nf t",
                                                     p=128))
                nc.sync.dma_start(
                    out=wq_sb[0][:, 8:16, :],
                    in_=wqh[1024:2048, :].rearrange("(nf p) d -> p nf d",
                                                    p=128))
                nc.sync.dma_start(
                    out=hs0[0][:, 8:16, :],
                    in_=hsh[1024:2048, 0:512].rearrange("(nf p) t -> p nf t",
                                                        p=128))
                nc.sync.dma_start(
                    out=wq_sb[1],
                    in_=wql.rearrange("(nf p) d -> p nf d", p=128))
                nc.sync.dma_start(
                    out=hs0[1],
                    in_=hsl[:, 0:512].rearrange("(nf p) t -> p nf t", p=128))
                nc.sync.dma_start(
                    out=wv_sb[0],
                    in_=wvh.rearrange("(nf p) d -> p nf d", p=128))
                nc.sync.dma_start(
                    out=wv_sb[1],
                    in_=wvl.rearrange("(nf p) d -> p nf d", p=128))
                nc.sync.dma_start(
                    out=wk_sb[0],
                    in_=wkh.rearrange("(nf p) d -> p nf d", p=128))
                nc.sync.dma_start(
                    out=wk_sb[1],
                    in_=wkl.rearrange("(nf p) d -> p nf d", p=128))
                # hs chunk 1 must beat the chunk-0 compute tail; cos/sin only
                # gate RoPE (vector work), mask/wo only phase 2.
                hs1 = [hsp.tile([128, NF, 512], f8, tag=f"hs{i}",
                                name=f"hs1{i}") for i in range(2)]
                nc.sync.dma_start(
                    out=hs1[0],
                    in_=hsh[:, 512:1024].rearrange("(nf p) t -> p nf t",
                                                   p=128))
                nc.sync.dma_start(
                    out=hs1[1],
                    in_=hsl[:, 512:1024].rearrange("(nf p) t -> p nf t",
                                                   p=128))
                nc.sync.dma_start(out=cos_sb, in_=cosd[:, :])
                nc.sync.dma_start(out=sin_sb, in_=sind[:, :])
                nc.sync.dma_start(
                    out=mask_sb, in_=maskd.rearrange("p (v t) -> p v t", v=4))
                nc.sync.dma_start(
                    out=wo_sb[0], in_=woh.rearrange("(kt p) f -> p kt f",
                                                    p=128))
                nc.sync.dma_start(
                    out=wo_sb[1], in_=wol.rearrange("(kt p) f -> p kt f",
                                                    p=128))

                def rope_apply(ps, dest, tl, swap_eng=nc.gpsimd,
                               late=False):
                    qf = rope.tile([128, 512], f32, tag="qf")
                    if late:
                        nc.vector.tensor_copy(qf, ps)
                    else:
                        nc.scalar.copy(qf, ps)
                    rot = rope.tile([128, 512], f32, tag="rot")
                    swap_eng.dma_start(out=rot[0:64, :], in_=qf[64:128, :])
                    swap_eng.dma_start(out=rot[64:128, :], in_=qf[0:64, :])
                    t1 = rope.tile([128, 512], f32, tag="t1")
                    nc.vector.tensor_mul(t1, rot, sin_sb[:, tl])
                    t2 = rope.tile([128, 512], f32, tag="t2")
                    nc.vector.tensor_mul(t2, qf, cos_sb[:, tl])
                    nc.vector.tensor_add(dest, t1, t2)

                def qk_matmuls(ps, w_pair, hs_pair, h):
                    # 3-term fp8 DoubleRow: hi@hi, lo(w)@hi, hi(w)@lo
                    hsel = slice(h * 128, (h + 1) * 128)
                    terms = [(0, 0), (1, 0), (0, 1)]
                    for ti, (wi, xi) in enumerate(terms):
                        for g in range(NG):
                            nc.tensor.matmul(
                                ps,
                                lhsT=w_pair[wi][:, 2 * g:2 * g + 2, hsel],
                                rhs=hs_pair[xi][:, 2 * g:2 * g + 2, :],
                                start=(ti == 0 and g == 0),
                                stop=(ti == 2 and g == NG - 1),
                                perf_mode=DR)

                for c in range(8):
                    cs = slice(c * 512, (c + 1) * 512)
                    tl = slice((c % 4) * 512, (c % 4) * 512 + 512)
                    if c == 0:
                        hs_t = hs0
                    elif c == 1:
                        hs_t = hs1
                    else:
                        hs_t = [hsp.tile([128, NF, 512], f8, tag=f"hs{i}",
                                         name=f"hs{i}")
                                for i in range(2)]
                        nc.sync.dma_start(
                            out=hs_t[0],
                            in_=hsh[:, cs].rearrange("(nf p) t -> p nf t",
                                                     p=128))
                        nc.sync.dma_start(
                            out=hs_t[1],
                            in_=hsl[:, cs].rearrange("(nf p) t -> p nf t",
                                                     p=128))
                    # Q (h0, h1) -> V -> K (h0, h1): matches DMA arrivals.
                    # Chunk 0: emit hi@hi g-halves first (first-half DMAs
                    # only), then the lo terms which need wql/hsl.
                    if c == 0:
                        qps = [qkps.tile([128, 512], f32, tag="qk",
                                         name=f"qps{h}") for h in range(HL)]
                        for gh in range(2):
                            for h in range(HL):
                                for g in range(gh * 4, gh * 4 + 4):
                                    nc.tensor.matmul(
                                        qps[h],
                                        lhsT=wq_sb[0][:, 2 * g:2 * g + 2,
                                                      h * 128:(h + 1) * 128],
                                        rhs=hs_t[0][:, 2 * g:2 * g + 2, :],
                                        start=(g == 0), stop=False,
                                        perf_mode=DR)
                        for h in range(HL):
                            for (wi, xi) in [(1, 0), (0, 1)]:
                                for g in range(NG):
                                    nc.tensor.matmul(
                                        qps[h],
                                        lhsT=wq_sb[wi][:, 2 * g:2 * g + 2,
                                                       h * 128:(h + 1) * 128],
                                        rhs=hs_t[xi][:, 2 * g:2 * g + 2, :],
                                        start=False,
                                        stop=(wi == 0 and g == NG - 1),
                                        perf_mode=DR)
                        for h in range(HL):
                            rope_apply(qps[h], qt[h][:, cs], tl, nc.gpsimd)
                    else:
                        for h in range(HL):
                            ps = qkps.tile([128, 512], f32, tag="qk")
                            qk_matmuls(ps, wq_sb, hs_t, h)
                            rope_apply(ps, qt[h][:, cs], tl,
                                       nc.gpsimd if c < 5 else nc.sync,
                                       late=(c >= 6))
                    for s in range(4):
                        vp = vps.tile([128, 256], f32, tag="v")
                        ssel = slice(s * 128, (s + 1) * 128)
                        terms = [(0, 0), (1, 0), (0, 1)]
                        for ti, (xi, wi) in enumerate(terms):
                            for g in range(NG):
                                nc.tensor.matmul(
                                    vp,
                                    lhsT=hs_t[xi][:, 2 * g:2 * g + 2, ssel],
                                    rhs=wv_sb[wi][:, 2 * g:2 * g + 2, :],
                                    start=(ti == 0 and g == 0),
                                    stop=(ti == 2 and g == NG - 1),
                                    perf_mode=DR)
                        # keep Act free near the phase boundary: route the
                        # last chunks' V copies through DVE
                        if c >= 6:
                            nc.vector.tensor_copy(vres[:, c * 4 + s, :], vp)
                        else:
                            nc.scalar.copy(vres[:, c * 4 + s, :], vp)
                    for h in range(HL):
                        ps = qkps.tile([128, 512], f32, tag="qk")
                        qk_matmuls(ps, wk_sb, hs_t, h)
                        rope_apply(ps, kt[h][:, cs], tl,
                                   nc.gpsimd if c < 5 else nc.sync,
                                   late=(c >= 6))

            # ========== Phase 2+3: attention + interleaved o_proj ==========
            with (
                tc.tile_pool(name="sps", bufs=B_SPS, space="PSUM") as sps,
                tc.tile_pool(name="ops", bufs=B_OPS, space="PSUM") as ops,
                tc.tile_pool(name="msc", bufs=B_MSC, space="PSUM") as msc,
                tc.tile_pool(name="smp", bufs=1, space="PSUM") as smp,
                tc.tile_pool(name="pt", bufs=B_PT) as ptp,
                tc.tile_pool(name="pr", bufs=B_PR) as prp,
                tc.tile_pool(name="att_sm", bufs=int(env.get("BK_SM", "4"))) as atsm,
                tc.tile_pool(name="ob", bufs=B_OB) as obp,
            ):
                pending = []

                def emit_unit():
                    if pending:
                        pending.pop(0)()

                def flush_units():
                    while pending:
                        pending.pop(0)()

                def enqueue_oproj(b, c, gsz=4):
                    tl = slice(c * 512, (c + 1) * 512)
                    state = {}
                    # GPSIMD cannot read PSUM; mix DVE/Act (Act carries exps)
                    copy_engines = [
                        ((lambda o, i: nc.vector.tensor_scalar_mul(o, i, ODESC))
                         if ch == "v" else
                         (lambda o, i: nc.scalar.mul(o, i, ODESC)))
                        for ch in CPY]

                    def unit(fo):
                        def f():
                            g = fo // gsz
                            if g not in state:
                                state[g] = obp.tile([128, gsz, 512], bf16,
                                                    tag=f"ob{gsz}",
                                                    name="ob_t")
                            ob_t = state[g]
                            fs = slice(fo * 128, (fo + 1) * 128)
                            pp = msc.tile([128, 512], f32, tag="pp")
                            nc.tensor.matmul(
                                pp, lhsT=wo_sb[0][:, :, fs],
                                rhs=aoh[b][:, :, tl],
                                start=True, stop=False, perf_mode=DR)
                            nc.tensor.matmul(
                                pp, lhsT=wo_sb[1][:, :, fs],
                                rhs=aoh[b][:, :, tl],
                                start=False, stop=False, perf_mode=DR)
                            nc.tensor.matmul(
                                pp, lhsT=wo_sb[0][:, :, fs],
                                rhs=aol[b][:, :, tl],
                                start=False, stop=True, perf_mode=DR)
                            copy_engines[fo % len(CPY)](
                                ob_t[:, fo % gsz, :], pp)
                            if fo % gsz == gsz - 1:
                                rows = slice(g * gsz * 128,
                                             (g + 1) * gsz * 128)
                                cg = slice((b * 4 + c) * 512,
                                           (b * 4 + c + 1) * 512)
                                eng = (nc.gpsimd if (OBQ and g % 2 == 1)
                                       else nc.sync)
                                eng.dma_start(
                                    out=outT[rows, cg].rearrange(
                                        "(nf p) t -> p nf t", p=128),
                                    in_=ob_t)
                        return f

                    for fo in range(16):
                        pending.append(unit(fo))

                # Global step stream: score-matmul lookahead crosses chunk
                # boundaries so the next chunk's exps run during the previous
                # chunk's tail (norm chain / oproj flush) with no PE bubble.
                class Chunk:
                    def __init__(self, p, c):
                        self.p, self.c = p, c
                        self.b, self.h = p // 2, p % 2
                        self.jmax = 4 * (c + 1)
                        self.o_ps = None
                        self.sm_ps = None
                        self.sm_started = False
                        self.pts = {}
                        self.prs = []
                        self.sq = []

                def tile_off(ck, j):
                    # Diagonal k-tile v=1..3: first 128v query cols are fully
                    # causal-masked -> compute only cols [128v:512]. Exact.
                    v = j - 4 * ck.c
                    return 128 * v if 1 <= v <= 3 else 0

                def emit_s(ck, j):
                    b, c = ck.b, ck.c
                    off = tile_off(ck, j)
                    sp = sps.tile([128, 512], f32, tag="s", name="sp")
                    nc.tensor.matmul(
                        sp[:, off:],
                        lhsT=kt[ck.h][:, b * T + j * 128:
                                      b * T + (j + 1) * 128],
                        rhs=qt[ck.h][:, b * T + c * 512 + off:
                                     b * T + (c + 1) * 512],
                        start=True, stop=True)
                    ck.sq.append(sp)

                def emit_epv(ck, j):
                    p, c, b, h = ck.p, ck.c, ck.b, ck.h
                    off = tile_off(ck, j)
                    sp = ck.sq[j]
                    pt_t = ptp.tile([128, 512], bf16, tag="p", name="pt_t")
                    nc.scalar.activation(pt_t[:, off:], sp[:, off:],
                                         EXP, scale=SCALE)
                    if j >= 4 * c:
                        nc.vector.tensor_mul(pt_t[:, off:], pt_t[:, off:],
                                             mask_sb[:, j - 4 * c, off:])
                    if c == 3:
                        nc.vector.tensor_copy(
                            abar_raw[p][:, j:j + 1], sp[:, 511:512])
                    if ck.o_ps is None:
                        ck.o_ps = ops.tile([128, 512], f32, tag="o",
                                           name="o_ps")
                    nc.tensor.matmul(
                        ck.o_ps[:, off:],
                        lhsT=vres[:, b * 16 + j, h * 128:(h + 1) * 128],
                        rhs=pt_t[:, off:],
                        start=(j == 0), stop=(j == ck.jmax - 1))
                    if ck.sm_ps is None:
                        ck.sm_ps = smp.tile([2, 512], f32, tag="sm",
                                            name="sm_ps")
                    if j >= 4 * c:
                        # diagonal tile: individual (possibly trimmed) rowsum
                        nc.tensor.matmul(
                            ck.sm_ps[:, off:],
                            lhsT=(ones_b if c == 3 else ones_a),
                            rhs=pt_t[:, off:],
                            start=(j == 4 * c and c == 0),
                            stop=(j == ck.jmax - 1))
                        return
                    ck.pts[j] = pt_t
                    if j % 2 == 1:
                        pr = prp.tile([128, 512], bf16, tag="pr", name="pr")
                        nc.vector.tensor_add(pr, ck.pts[j - 1], ck.pts[j])
                        nc.tensor.matmul(
                            ck.sm_ps,
                            lhsT=ones_a,
                            rhs=pr,
                            start=(j == 1), stop=False)
                        del ck.pts[j - 1], ck.pts[j]

                def epilogue(ck):
                    p, c, b, hl = ck.p, ck.c, ck.b, ck.h
                    cl = slice(c * 512, (c + 1) * 512)
                    rec = atsm.tile([1, 512], f32, tag="rec", name="rec")
                    nc.vector.reciprocal(rec, ck.sm_ps[0:1, :])
                    bc = atsm.tile([128, 512], f32, tag="bc", name="bc")
                    nc.gpsimd.partition_broadcast(bc, rec)
                    full = atsm.tile([128, 512], bf16, tag="full",
                                     name="full")
                    nc.vector.tensor_mul(full, ck.o_ps, bc)
                    # fp8 hi/lo split on Pool (Act does exps, DVE the rest)
                    nc.gpsimd.tensor_copy(aoh[b][:, hl, cl], full)
                    nc.gpsimd.tensor_sub(aol[b][:, hl, cl], full,
                                         aoh[b][:, hl, cl])
                    sm_sb = atsm.tile([2, 512], f32, tag="smsb", name="sm_sb")
                    nc.vector.tensor_copy(sm_sb, ck.sm_ps)
                    nc.sync.dma_start(out=sumsd[p, :, cl], in_=sm_sb)
                    if c == 3:
                        ab_exp = atsm.tile([128, 16], f32, tag="abe",
                                           name="ab_exp")
                        nc.scalar.activation(
                            ab_exp, abar_raw[p], EXP, scale=SCALE)
                        nc.sync.dma_start(out=abard[p], in_=ab_exp)

                chunks = [Chunk(b * 2 + hl, c)
                          for b in range(B) for c in range(4)
                          for hl in range(HL)]
                steps = [(ck, j) for ck in chunks for j in range(ck.jmax)]
                for k in range(LOOK):
                    emit_s(*steps[k])
                for i, (ck, j) in enumerate(steps):
                    if i + LOOK < len(steps):
                        emit_s(*steps[i + LOOK])
                    emit_epv(ck, j)
                    if ILV and j % 2 == 1:
                        for _ in range(ILVN):
                            emit_unit()
                    if j == ck.jmax - 1:
                        epilogue(ck)
                        if ck.h == 1:
                            flush_units()
                            enqueue_oproj(
                                ck.b, ck.c,
                                gsz=(2 if (ck.b, ck.c) == (1, 3) else 4))
                flush_units()

    nc.compile()
    return nc


def _get_nc():
    if "nc" not in _NC_CACHE:
        _NC_CACHE["nc"] = build_nc()
    return _NC_CACHE["nc"]


def _split8(x):
    hi = x.astype(F8)
    lo = (x - hi.astype(np.float32)).astype(F8)
    return hi, lo


def _host_inputs(hidden_states, q_w, k_w, v_w, o_w):
    """Per-core input dicts."""
    hsT = np.ascontiguousarray(hidden_states.reshape(BT, HID).T)
    hs_hi, hs_lo = _split8(hsT)
    inv = 10000.0 ** (-np.arange(64, dtype=np.float64) / 64.0)
    t = np.arange(T, dtype=np.float64)
    fr = t[None, :] * inv[:, None]  # [64, T]
    # 1/WSCL descale of the x64-scaled Q/K baked into the rope tables
    cosT = (np.cos(np.concatenate([fr, fr], 0)) / WSCL).astype(np.float32)
    sinT = (np.sin(np.concatenate([fr, fr], 0)) / WSCL).astype(np.float32)
    sinT[:64] *= -1.0  # sign-baked for swap-halves rotate
    masks = np.zeros((128, 4, 512), dtype=np.float32)
    kk = np.arange(128)[:, None]
    tt = np.arange(512)[None, :]
    for v in range(4):
        masks[:, v, :] = (tt >= 128 * v + kk).astype(np.float32)
    masks = masks.reshape(128, 2048).astype(BF16)

    in_maps = []
    for core in range(NCORES):
        rs = slice(core * 256, (core + 1) * 256)
        wq_hi, wq_lo = _split8(
            WSCL * np.ascontiguousarray(q_w[rs, :].T))
        wk_hi, wk_lo = _split8(
            WSCL * np.ascontiguousarray(k_w[rs, :].T))
        wv_hi, wv_lo = _split8(
            VSCL * np.ascontiguousarray(v_w[rs, :].T))
        wo_hi, wo_lo = _split8(
            WSCL * np.ascontiguousarray(o_w[:, rs].T))
        in_maps.append({
            "hsh": hs_hi,
            "hsl": hs_lo,
            "wqh": wq_hi, "wql": wq_lo,
            "wkh": wk_hi, "wkl": wk_lo,
            "wvh": wv_hi, "wvl": wv_lo,
            "woh": wo_hi, "wol": wo_lo,
            "cosT": cosT,
            "sinT": sinT,
            "masks": masks,
        })
    return in_maps


def _epilogue(out, results, hidden_states, v_w, o_w):
    """Add the CaM rank-1 correction per (b, h) on host."""
    for core in range(NCORES):
        r = results[core]
        for p in range(4):
            b, hl = p // 2, p % 2
            h = core * HL + hl
            rowsum = r["sums"][p][0]  # [T] unnormalized exp row sums
            tails = r["sums"][p][1]
            a_exp = np.asarray(r["abar"][p], np.float64).T.reshape(2048)
            a_bar = a_exp / max(float(rowsum[T - 1]), 1e-30)
            avg_w = max(float(np.mean(a_bar[WS:])), 1e-6)
            prob = float(np.clip(a_bar[EVICT] / avg_w, 0.0, 1.0))
            prob = float(np.nan_to_num(prob, nan=0.0, posinf=1.0, neginf=0.0))
            m = 1.0 if U_CONST[b, h] < prob else 0.0
            if m == 0.0:
                continue
            # exact v_e from fp32 inputs
            v_row = hidden_states[b, EVICT, :] @ v_w[h * D:(h + 1) * D, :].T
            v_e = v_row * (m / RB)  # [D]
            w_e = o_w[:, h * D:(h + 1) * D] @ v_e  # [HID]
            s_tail = (tails / np.maximum(rowsum, 1e-30)).astype(np.float32)
            out[b] += np.outer(s_tail, w_e).astype(np.float32)
    return out


def kernel(hidden_states, attention_mask, q_w, k_w, v_w, o_w):
    from concourse.bass_utils import run_bass_kernel_spmd

    nc = _get_nc()
    in_maps = _host_inputs(hidden_states, q_w, k_w, v_w, o_w)
    trace = bool(int(os.environ.get("BK_TRACE", "0")))
    res = run_bass_kernel_spmd(
        nc, in_maps, core_ids=list(range(NCORES)), trace=trace,
    )
    if trace and res.exec_time_ns is not None:
        print(f"HW exec time: {res.exec_time_ns} ns")
        _NC_CACHE["last_exec_ns"] = res.exec_time_ns
        _NC_CACHE["last_trace"] = res.instructions_and_trace
    results = res.results

    acc = np.zeros((HID, BT), dtype=np.float32)
    for core in range(NCORES):
        acc += np.asarray(results[core]["outT"], np.float32)
    out = np.ascontiguousarray(acc.T).reshape(B, T, HID)
    out = _epilogue(out, results, hidden_states, v_w, o_w)
    return out.astype(np.float32)


# revision 6
# speedup vs baseline: 1.0518x; 1.0067x over previous
"""Trainium2 Bass kernel for nn_LlamaAttention_cam (sparse_attention).

Sharding: 16 heads across 8 cores (2 heads/core), both batches per core.
Q/K/V projections column-parallel over heads; o_proj row-parallel (per-core
partial outputs summed on host). The CaM merge is a rank-1 correction
(s_tail outer v_e) applied on host from tiny device-side statistics.

The projection GEMMs (QKV + o_proj) run as fp8e4 DoubleRow matmuls with
3-term error compensation: X*W ~ Xh*Wh + Xl*Wh + Xh*Wl where Xh = fp8(X),
Xl = fp8(X - Xh). DoubleRow packs a 256-deep contraction at 0.5 cyc/col,
so 3 terms cost 75% of the bf16 equivalent. hs and all weights are split
on the host (free); attn_out is split on-device (Pool engine). Weights are
pre-scaled by 64 into fp8 range; V inherits x64 which puts attn_out in
fp8 range too; the o_proj epilogue copy descales by 1/4096.

Self-contained: hardcodes all shapes; takes full inputs, returns full output.
"""

import math
import os

import numpy as np
import ml_dtypes

B, T, HID, H = 2, 2048, 2048, 16
D = 128
NCORES = 8
HL = H // NCORES  # heads per core = 2
BT = B * T  # 4096
NF = HID // 128  # 16 f-tiles
NG = NF // 2  # 8 f-tile pairs for DoubleRow
SCALE = 1.0 / math.sqrt(D)
RB = int(0.25 * T)  # 512 recent budget
WS = T - RB  # 1536
EVICT = WS - 1  # 1535
WSCL = 64.0  # fp8 pre-scale on wq/wk/wo
VSCL = 16.0  # fp8 pre-scale on wv: max |attn_out*VSCL| ~ 5sigma*16 = 72 < 240
ODESC = 1.0 / (VSCL * WSCL)  # o_proj descale: V carries x16, wo carries x64

# jax.random.uniform(jax.random.key(42), (2,16), float32); bernoulli(key,p) == u < p
U_CONST = np.array(
    [[0.59400654, 0.43801308, 0.6285691, 0.00791204, 0.27834702,
      0.7976179, 0.8521497, 0.9625306, 0.67656493, 0.11104441,
      0.4959929, 0.7311437, 0.18970704, 0.1544199, 0.03802836,
      0.33559263],
     [0.92825687, 0.6123972, 0.49262476, 0.733806, 0.18920851,
      0.15386605, 0.037136197, 0.32930005, 0.9372028, 0.5957513,
      0.4615929, 0.6695677, 0.07019377, 0.39408123, 0.55786455,
      0.35412872]], dtype=np.float32)

BF16 = ml_dtypes.bfloat16
F8 = ml_dtypes.float8_e4m3

_NC_CACHE = {}


def build_nc():
    import concourse.bacc as bacc
    import concourse.mybir as mybir
    import concourse.tile as tile

    f32 = mybir.dt.float32
    bf16 = mybir.dt.bfloat16
    f8 = mybir.dt.float8e4
    EXP = mybir.ActivationFunctionType.Exp
    DR = mybir.MatmulPerfMode.DoubleRow

    nc = bacc.Bacc("TRN2", target_bir_lowering=False, debug=False)
    env = os.environ
    B_QK = int(env.get("BK_QK", "5"))
    B_HSP = int(env.get("BK_HSP", "2"))
    B_ROPE = int(env.get("BK_ROPE", "3"))
    B_SPS = int(env.get("BK_SPS", "3"))
    B_OPS = int(env.get("BK_OPS", "1"))
    B_MSC = int(env.get("BK_MSC", "3"))
    B_PT = int(env.get("BK_PT", "10"))
    B_PR = int(env.get("BK_PR", "3"))
    B_OB = int(env.get("BK_OB", "8"))
    LOOK = int(env.get("BK_LOOK", "3"))
    ILV = env.get("BK_ILV", "1") == "1"
    CPY = env.get("BK_CPY", "vs")  # per-fo copy engine cycle: v=DVE s=Act
    OBQ = env.get("BK_OBQ", "0") == "1"  # alternate ob DMA queues
    ILVN = int(env.get("BK_ILVN", "1"))  # oproj units per 2 j-steps

    hsh = nc.dram_tensor("hsh", [HID, BT], f8, kind="ExternalInput")
    hsl = nc.dram_tensor("hsl", [HID, BT], f8, kind="ExternalInput")
    wqh = nc.dram_tensor("wqh", [HID, 256], f8, kind="ExternalInput")
    wql = nc.dram_tensor("wql", [HID, 256], f8, kind="ExternalInput")
    wkh = nc.dram_tensor("wkh", [HID, 256], f8, kind="ExternalInput")
    wkl = nc.dram_tensor("wkl", [HID, 256], f8, kind="ExternalInput")
    wvh = nc.dram_tensor("wvh", [HID, 256], f8, kind="ExternalInput")
    wvl = nc.dram_tensor("wvl", [HID, 256], f8, kind="ExternalInput")
    woh = nc.dram_tensor("woh", [256, HID], f8, kind="ExternalInput")
    wol = nc.dram_tensor("wol", [256, HID], f8, kind="ExternalInput")
    cosd = nc.dram_tensor("cosT", [128, T], f32, kind="ExternalInput")
    sind = nc.dram_tensor("sinT", [128, T], f32, kind="ExternalInput")
    maskd = nc.dram_tensor("masks", [128, 2048], bf16, kind="ExternalInput")

    outT = nc.dram_tensor("outT", [HID, BT], bf16, kind="ExternalOutput")
    abard = nc.dram_tensor("abar", [4, 128, 16], f32, kind="ExternalOutput")
    sumsd = nc.dram_tensor("sums", [4, 2, T], f32, kind="ExternalOutput")

    with tile.TileContext(nc) as tc:
        with (
            tc.tile_pool(name="singles", bufs=1) as singles,
            tc.tile_pool(name="res", bufs=1) as res,
            tc.tile_pool(name="stats", bufs=1) as stats,
        ):
            # --- small constants (no DMA deps) for the PE warm-up ---
            ones_a = singles.tile([128, 2], bf16, tag="onesa")  # [1, 0]
            ones_b = singles.tile([128, 2], bf16, tag="onesb")  # [1, 1]
            nc.vector.memset(ones_a[:, 0:1], 1.0)
            nc.vector.memset(ones_a[:, 1:2], 0.0)
            nc.vector.memset(ones_b, 1.0)
            wsrc = singles.tile([128, 16], bf16, tag="wsrc")
            nc.vector.memset(wsrc, 0.0)

            # --- constant tiles (fp8 hi/lo weight pairs) ---
            wq_sb = [singles.tile([128, NF, 256], f8, tag=f"wq{i}",
                                   name=f"wq{i}")
                     for i in range(2)]
            wk_sb = [singles.tile([128, NF, 256], f8, tag=f"wk{i}",
                                   name=f"wk{i}")
                     for i in range(2)]
            wv_sb = [singles.tile([128, NF, 256], f8, tag=f"wv{i}",
                                   name=f"wv{i}")
                     for i in range(2)]
            wo_sb = [singles.tile([128, 2, HID], f8, tag=f"wo{i}",
                                   name=f"wo{i}")
                     for i in range(2)]
            cos_sb = singles.tile([128, T], f32, tag="cos")
            sin_sb = singles.tile([128, T], f32, tag="sin")
            mask_sb = singles.tile([128, 4, 512], bf16, tag="mask")

            # --- residents ---
            qt = [res.tile([128, BT], bf16, tag=f"qt{h}", name=f"qt{h}")
                  for h in range(HL)]
            kt = [res.tile([128, BT], bf16, tag=f"kt{h}", name=f"kt{h}")
                  for h in range(HL)]
            vres = res.tile([128, 32, 256], bf16, tag="vres")
            # attn_out hi/lo fp8, [d, head, t] per batch
            aoh = [res.tile([128, 2, T], f8, tag=f"aoh{b}", name=f"aoh{b}")
                   for b in range(B)]
            aol = [res.tile([128, 2, T], f8, tag=f"aol{b}", name=f"aol{b}")
                   for b in range(B)]
            abar_raw = [stats.tile([128, 16], f32, tag=f"ab{p}", name=f"ab{p}")
                        for p in range(4)]

            # ================= Phase 1: QKV projections + RoPE ================
            with (
                tc.tile_pool(name="hsp", bufs=B_HSP) as hsp,
                tc.tile_pool(name="rope", bufs=B_ROPE) as rope,
                tc.tile_pool(name="qkps", bufs=B_QK, space="PSUM") as qkps,
                tc.tile_pool(name="vps", bufs=2, space="PSUM") as vps,
                tc.tile_pool(name="wps", bufs=1, space="PSUM") as wps,
            ):
                # PE warm-up: a tiny matmul at t~0 starts the p-state ramp so
                # the real matmuls (gated on DMA) run at full clock.
                warm = wps.tile([2, 16], f32, tag="warm")
                nc.tensor.matmul(warm, lhsT=ones_a, rhs=wsrc,
                                 start=True, stop=True)

                # DMA issue order is the sync-queue service order; front-load
                # exactly what the first matmuls need (wq_hi + hs_hi chunk 0).
                hs0 = [hsp.tile([128, NF, 512], f8, tag=f"hs{i}",
                                name=f"hs0{i}") for i in range(2)]
                nc.sync.dma_start(
                    out=wq_sb[0][:, 0:8, :],
                    in_=wqh[0:1024, :].rearrange("(nf p) d -> p nf d", p=128))
                nc.sync.dma_start(
                    out=hs0[0][:, 0:8, :],
                    in_=hsh[0:1024, 0:512].rearrange("(nf p) t -> p nf t",
                                                     p=128))
                nc.sync.dma_start(
                    out=wq_sb[0][:, 8:16, :],
                    in_=wqh[1024:2048, :].rearrange("(nf p) d -> p nf d",
                                                    p=128))
                nc.sync.dma_start(
                    out=hs0[0][:, 8:16, :],
                    in_=hsh[1024:2048, 0:512].rearrange("(nf p) t -> p nf t",
                                                        p=128))
                nc.sync.dma_start(
                    out=wq_sb[1],
                    in_=wql.rearrange("(nf p) d -> p nf d", p=128))
                nc.sync.dma_start(
                    out=hs0[1],
                    in_=hsl[:, 0:512].rearrange("(nf p) t -> p nf t", p=128))
                nc.sync.dma_start(
                    out=wv_sb[0],
                    in_=wvh.rearrange("(nf p) d -> p nf d", p=128))
                nc.sync.dma_start(
                    out=wv_sb[1],
                    in_=wvl.rearrange("(nf p) d -> p nf d", p=128))
                nc.sync.dma_start(
                    out=wk_sb[0],
                    in_=wkh.rearrange("(nf p) d -> p nf d", p=128))
                nc.sync.dma_start(
                    out=wk_sb[1],
                    in_=wkl.rearrange("(nf p) d -> p nf d", p=128))
                # hs chunk 1 must beat the chunk-0 compute tail; cos/sin only
                # gate RoPE (vector work), mask/wo only phase 2.
                hs1 = [hsp.tile([128, NF, 512], f8, tag=f"hs{i}",
                                name=f"hs1{i}") for i in range(2)]
                nc.sync.dma_start(
                    out=hs1[0],
                    in_=hsh[:, 512:1024].rearrange("(nf p) t -> p nf t",
                                                   p=128))
                nc.sync.dma_start(
                    out=hs1[1],
                    in_=hsl[:, 512:1024].rearrange("(nf p) t -> p nf t",
                                                   p=128))
                nc.gpsimd.dma_start(out=cos_sb, in_=cosd[:, :])
                nc.gpsimd.dma_start(out=sin_sb, in_=sind[:, :])

                def rope_apply(ps, dest, tl, swap_eng=nc.gpsimd,
                               late=False):
                    qf = rope.tile([128, 512], f32, tag="qf")
                    if late:
                        nc.vector.tensor_copy(qf, ps)
                    else:
                        nc.scalar.copy(qf, ps)
                    rot = rope.tile([128, 512], f32, tag="rot")
                    swap_eng.dma_start(out=rot[0:64, :], in_=qf[64:128, :])
                    swap_eng.dma_start(out=rot[64:128, :], in_=qf[0:64, :])
                    t1 = rope.tile([128, 512], f32, tag="t1")
                    nc.vector.tensor_mul(t1, rot, sin_sb[:, tl])
                    t2 = rope.tile([128, 512], f32, tag="t2")
                    nc.vector.tensor_mul(t2, qf, cos_sb[:, tl])
                    nc.vector.tensor_add(dest, t1, t2)

                def qk_matmuls(ps, w_pair, hs_pair, h):
                    # 3-term fp8 DoubleRow: hi@hi, lo(w)@hi, hi(w)@lo
                    hsel = slice(h * 128, (h + 1) * 128)
                    terms = [(0, 0), (1, 0), (0, 1)]
                    for ti, (wi, xi) in enumerate(terms):
                        for g in range(NG):
                            nc.tensor.matmul(
                                ps,
                                lhsT=w_pair[wi][:, 2 * g:2 * g + 2, hsel],
                                rhs=hs_pair[xi][:, 2 * g:2 * g + 2, :],
                                start=(ti == 0 and g == 0),
                                stop=(ti == 2 and g == NG - 1),
                                perf_mode=DR)

                for c in range(8):
                    cs = slice(c * 512, (c + 1) * 512)
                    tl = slice((c % 4) * 512, (c % 4) * 512 + 512)
                    if c == 0:
                        hs_t = hs0
                    elif c == 1:
                        hs_t = hs1
                    else:
                        hs_t = [hsp.tile([128, NF, 512], f8, tag=f"hs{i}",
                                         name=f"hs{i}")
                                for i in range(2)]
                        nc.sync.dma_start(
                            out=hs_t[0],
                            in_=hsh[:, cs].rearrange("(nf p) t -> p nf t",
                                                     p=128))
                        nc.sync.dma_start(
                            out=hs_t[1],
                            in_=hsl[:, cs].rearrange("(nf p) t -> p nf t",
                                                     p=128))
                        if c == 2:
                            nc.sync.dma_start(
                                out=mask_sb,
                                in_=maskd.rearrange("p (v t) -> p v t", v=4))
                            nc.sync.dma_start(
                                out=wo_sb[0],
                                in_=woh.rearrange("(kt p) f -> p kt f",
                                                  p=128))
                            nc.sync.dma_start(
                                out=wo_sb[1],
                                in_=wol.rearrange("(kt p) f -> p kt f",
                                                  p=128))
                    # Q (h0, h1) -> V -> K (h0, h1): matches DMA arrivals.
                    # Chunk 0: emit hi@hi g-halves first (first-half DMAs
                    # only), then the lo terms which need wql/hsl.
                    if c == 0:
                        qps = [qkps.tile([128, 512], f32, tag="qk",
                                         name=f"qps{h}") for h in range(HL)]
                        for gh in range(2):
                            for h in range(HL):
                                for g in range(gh * 4, gh * 4 + 4):
                                    nc.tensor.matmul(
                                        qps[h],
                                        lhsT=wq_sb[0][:, 2 * g:2 * g + 2,
                                                      h * 128:(h + 1) * 128],
                                        rhs=hs_t[0][:, 2 * g:2 * g + 2, :],
                                        start=(g == 0), stop=False,
                                        perf_mode=DR)
                        for h in range(HL):
                            for (wi, xi) in [(1, 0), (0, 1)]:
                                for g in range(NG):
                                    nc.tensor.matmul(
                                        qps[h],
                                        lhsT=wq_sb[wi][:, 2 * g:2 * g + 2,
                                                       h * 128:(h + 1) * 128],
                                        rhs=hs_t[xi][:, 2 * g:2 * g + 2, :],
                                        start=False,
                                        stop=(wi == 0 and g == NG - 1),
                                        perf_mode=DR)
                        for h in range(HL):
                            rope_apply(qps[h], qt[h][:, cs], tl, nc.gpsimd)
                    else:
                        for h in range(HL):
                            ps = qkps.tile([128, 512], f32, tag="qk")
                            qk_matmuls(ps, wq_sb, hs_t, h)
                            rope_apply(ps, qt[h][:, cs], tl,
                                       nc.gpsimd if c < 5 else nc.sync,
                                       late=(c >= 6))
                    for s in range(4):
                        vp = vps.tile([128, 256], f32, tag="v")
                        ssel = slice(s * 128, (s + 1) * 128)
                        terms = [(0, 0), (1, 0), (0, 1)]
                        for ti, (xi, wi) in enumerate(terms):
                            for g in range(NG):
                                nc.tensor.matmul(
                                    vp,
                                    lhsT=hs_t[xi][:, 2 * g:2 * g + 2, ssel],
                                    rhs=wv_sb[wi][:, 2 * g:2 * g + 2, :],
                                    start=(ti == 0 and g == 0),
                                    stop=(ti == 2 and g == NG - 1),
                                    perf_mode=DR)
                        # keep Act free near the phase boundary: route the
                        # last chunks' V copies through DVE
                        if c >= 6:
                            nc.vector.tensor_copy(vres[:, c * 4 + s, :], vp)
                        else:
                            nc.scalar.copy(vres[:, c * 4 + s, :], vp)
                    for h in range(HL):
                        ps = qkps.tile([128, 512], f32, tag="qk")
                        qk_matmuls(ps, wk_sb, hs_t, h)
                        rope_apply(ps, kt[h][:, cs], tl,
                                   nc.gpsimd if c < 5 else nc.sync,
                                   late=(c >= 6))

            # ========== Phase 2+3: attention + interleaved o_proj ==========
            with (
                tc.tile_pool(name="sps", bufs=B_SPS, space="PSUM") as sps,
                tc.tile_pool(name="ops", bufs=B_OPS, space="PSUM") as ops,
                tc.tile_pool(name="msc", bufs=B_MSC, space="PSUM") as msc,
                tc.tile_pool(name="smp", bufs=1, space="PSUM") as smp,
                tc.tile_pool(name="pt", bufs=B_PT) as ptp,
                tc.tile_pool(name="pr", bufs=B_PR) as prp,
                tc.tile_pool(name="att_sm", bufs=int(env.get("BK_SM", "4"))) as atsm,
                tc.tile_pool(name="ob", bufs=B_OB) as obp,
            ):
                pending = []

                def emit_unit():
                    if pending:
                        pending.pop(0)()

                def flush_units():
                    while pending:
                        pending.pop(0)()

                def enqueue_oproj(b, c, gsz=4):
                    tl = slice(c * 512, (c + 1) * 512)
                    state = {}
                    # GPSIMD cannot read PSUM; mix DVE/Act (Act carries exps)
                    copy_engines = [
                        ((lambda o, i: nc.vector.tensor_scalar_mul(o, i, ODESC))
                         if ch == "v" else
                         (lambda o, i: nc.scalar.mul(o, i, ODESC)))
                        for ch in CPY]

                    def unit(fo):
                        def f():
                            g = fo // gsz
                            if g not in state:
                                state[g] = obp.tile([128, gsz, 512], bf16,
                                                    tag=f"ob{gsz}",
                                                    name="ob_t")
                            ob_t = state[g]
                            fs = slice(fo * 128, (fo + 1) * 128)
                            pp = msc.tile([128, 512], f32, tag="pp")
                            nc.tensor.matmul(
                                pp, lhsT=wo_sb[0][:, :, fs],
                                rhs=aoh[b][:, :, tl],
                                start=True, stop=False, perf_mode=DR)
                            nc.tensor.matmul(
                                pp, lhsT=wo_sb[1][:, :, fs],
                                rhs=aoh[b][:, :, tl],
                                start=False, stop=False, perf_mode=DR)
                            nc.tensor.matmul(
                                pp, lhsT=wo_sb[0][:, :, fs],
                                rhs=aol[b][:, :, tl],
                                start=False, stop=True, perf_mode=DR)
                            copy_engines[fo % len(CPY)](
                                ob_t[:, fo % gsz, :], pp)
                            if fo % gsz == gsz - 1:
                                rows = slice(g * gsz * 128,
                                             (g + 1) * gsz * 128)
                                cg = slice((b * 4 + c) * 512,
                                           (b * 4 + c + 1) * 512)
                                eng = (nc.gpsimd if (OBQ and g % 2 == 1)
                                       else nc.sync)
                                eng.dma_start(
                                    out=outT[rows, cg].rearrange(
                                        "(nf p) t -> p nf t", p=128),
                                    in_=ob_t)
                        return f

                    for fo in range(16):
                        pending.append(unit(fo))

                # Global step stream: score-matmul lookahead crosses chunk
                # boundaries so the next chunk's exps run during the previous
                # chunk's tail (norm chain / oproj flush) with no PE bubble.
                class Chunk:
                    def __init__(self, p, c):
                        self.p, self.c = p, c
                        self.b, self.h = p // 2, p % 2
                        self.jmax = 4 * (c + 1)
                        self.o_ps = None
                        self.sm_ps = None
                        self.sm_started = False
                        self.pts = {}
                        self.prs = []
                        self.sq = []

                def tile_off(ck, j):
                    # Diagonal k-tile v=1..3: first 128v query cols are fully
                    # causal-masked -> compute only cols [128v:512]. Exact.
                    v = j - 4 * ck.c
                    return 128 * v if 1 <= v <= 3 else 0

                def emit_s(ck, j):
                    b, c = ck.b, ck.c
                    off = tile_off(ck, j)
                    sp = sps.tile([128, 512], f32, tag="s", name="sp")
                    nc.tensor.matmul(
                        sp[:, off:],
                        lhsT=kt[ck.h][:, b * T + j * 128:
                                      b * T + (j + 1) * 128],
                        rhs=qt[ck.h][:, b * T + c * 512 + off:
                                     b * T + (c + 1) * 512],
                        start=True, stop=True)
                    ck.sq.append(sp)

                def emit_epv(ck, j):
                    p, c, b, h = ck.p, ck.c, ck.b, ck.h
                    off = tile_off(ck, j)
                    sp = ck.sq[j]
                    pt_t = ptp.tile([128, 512], bf16, tag="p", name="pt_t")
                    nc.scalar.activation(pt_t[:, off:], sp[:, off:],
                                         EXP, scale=SCALE)
                    if j >= 4 * c:
                        nc.vector.tensor_mul(pt_t[:, off:], pt_t[:, off:],
                                             mask_sb[:, j - 4 * c, off:])
                    if c == 3:
                        nc.vector.tensor_copy(
                            abar_raw[p][:, j:j + 1], sp[:, 511:512])
                    if ck.o_ps is None:
                        ck.o_ps = ops.tile([128, 512], f32, tag="o",
                                           name="o_ps")
                    nc.tensor.matmul(
                        ck.o_ps[:, off:],
                        lhsT=vres[:, b * 16 + j, h * 128:(h + 1) * 128],
                        rhs=pt_t[:, off:],
                        start=(j == 0), stop=(j == ck.jmax - 1))
                    if ck.sm_ps is None:
                        ck.sm_ps = smp.tile([2, 512], f32, tag="sm",
                                            name="sm_ps")
                    if j >= 4 * c:
                        # diagonal tile: individual (possibly trimmed) rowsum
                        nc.tensor.matmul(
                            ck.sm_ps[:, off:],
                            lhsT=(ones_b if c == 3 else ones_a),
                            rhs=pt_t[:, off:],
                            start=(j == 4 * c and c == 0),
                            stop=(j == ck.jmax - 1))
                        return
                    ck.pts[j] = pt_t
                    if j % 2 == 1:
                        pr = prp.tile([128, 512], bf16, tag="pr", name="pr")
                        nc.vector.tensor_add(pr, ck.pts[j - 1], ck.pts[j])
                        nc.tensor.matmul(
                            ck.sm_ps,
                            lhsT=ones_a,
                            rhs=pr,
                            start=(j == 1), stop=False)
                        del ck.pts[j - 1], ck.pts[j]

                def epilogue(ck):
                    p, c, b, hl = ck.p, ck.c, ck.b, ck.h
                    cl = slice(c * 512, (c + 1) * 512)
                    rec = atsm.tile([1, 512], f32, tag="rec", name="rec")
                    nc.vector.reciprocal(rec, ck.sm_ps[0:1, :])
                    bc = atsm.tile([128, 512], f32, tag="bc", name="bc")
                    nc.gpsimd.partition_broadcast(bc, rec)
                    full = atsm.tile([128, 512], bf16, tag="full",
                                     name="full")
                    nc.vector.tensor_mul(full, ck.o_ps, bc)
                    # fp8 hi/lo split on Pool (Act does exps, DVE the rest)
                    nc.gpsimd.tensor_copy(aoh[b][:, hl, cl], full)
                    nc.gpsimd.tensor_sub(aol[b][:, hl, cl], full,
                                         aoh[b][:, hl, cl])
                    nc.sync.dma_start(out=sumsd[p, 0:1, cl], in_=rec)
                    if c == 3:
                        tl_sb = atsm.tile([2, 512], f32, tag="smsb",
                                          name="tl_sb")
                        nc.vector.tensor_copy(tl_sb, ck.sm_ps)
                        nc.sync.dma_start(out=sumsd[p, 1:2, cl],
                                          in_=tl_sb[1:2, :])
                    if c == 3:
                        ab_exp = atsm.tile([128, 16], f32, tag="abe",
                                           name="ab_exp")
                        nc.scalar.activation(
                            ab_exp, abar_raw[p], EXP, scale=SCALE)
                        nc.sync.dma_start(out=abard[p], in_=ab_exp)

                chunks = [Chunk(b * 2 + hl, c)
                          for b in range(B) for c in range(4)
                          for hl in range(HL)]
                steps = [(ck, j) for ck in chunks for j in range(ck.jmax)]
                for k in range(LOOK):
                    emit_s(*steps[k])
                for i, (ck, j) in enumerate(steps):
                    if i + LOOK < len(steps):
                        emit_s(*steps[i + LOOK])
                    emit_epv(ck, j)
                    if ILV and j % 2 == 1:
                        for _ in range(ILVN):
                            emit_unit()
                    if j == ck.jmax - 1:
                        epilogue(ck)
                        if ck.h == 1:
                            flush_units()
                            enqueue_oproj(
                                ck.b, ck.c,
                                gsz=(2 if (ck.b, ck.c) == (1, 3) else 4))
                flush_units()

    nc.compile()
    return nc


def _get_nc():
    if "nc" not in _NC_CACHE:
        _NC_CACHE["nc"] = build_nc()
    return _NC_CACHE["nc"]


def _split8(x):
    hi = x.astype(F8)
    lo = (x - hi.astype(np.float32)).astype(F8)
    return hi, lo


def _host_inputs(hidden_states, q_w, k_w, v_w, o_w):
    """Per-core input dicts."""
    hsT = np.ascontiguousarray(hidden_states.reshape(BT, HID).T)
    hs_hi, hs_lo = _split8(hsT)
    inv = 10000.0 ** (-np.arange(64, dtype=np.float64) / 64.0)
    t = np.arange(T, dtype=np.float64)
    fr = t[None, :] * inv[:, None]  # [64, T]
    # 1/WSCL descale of the x64-scaled Q/K baked into the rope tables
    cosT = (np.cos(np.concatenate([fr, fr], 0)) / WSCL).astype(np.float32)
    sinT = (np.sin(np.concatenate([fr, fr], 0)) / WSCL).astype(np.float32)
    sinT[:64] *= -1.0  # sign-baked for swap-halves rotate
    masks = np.zeros((128, 4, 512), dtype=np.float32)
    kk = np.arange(128)[:, None]
    tt = np.arange(512)[None, :]
    for v in range(4):
        masks[:, v, :] = (tt >= 128 * v + kk).astype(np.float32)
    masks = masks.reshape(128, 2048).astype(BF16)

    in_maps = []
    for core in range(NCORES):
        rs = slice(core * 256, (core + 1) * 256)
        wq_hi, wq_lo = _split8(
            WSCL * np.ascontiguousarray(q_w[rs, :].T))
        wk_hi, wk_lo = _split8(
            WSCL * np.ascontiguousarray(k_w[rs, :].T))
        wv_hi, wv_lo = _split8(
            VSCL * np.ascontiguousarray(v_w[rs, :].T))
        wo_hi, wo_lo = _split8(
            WSCL * np.ascontiguousarray(o_w[:, rs].T))
        in_maps.append({
            "hsh": hs_hi,
            "hsl": hs_lo,
            "wqh": wq_hi, "wql": wq_lo,
            "wkh": wk_hi, "wkl": wk_lo,
            "wvh": wv_hi, "wvl": wv_lo,
            "woh": wo_hi, "wol": wo_lo,
            "cosT": cosT,
            "sinT": sinT,
            "masks": masks,
        })
    return in_maps


def _epilogue(out, results, hidden_states, v_w, o_w):
    """Add the CaM rank-1 correction per (b, h) on host."""
    for core in range(NCORES):
        r = results[core]
        for p in range(4):
            b, hl = p // 2, p % 2
            h = core * HL + hl
            rec = np.asarray(r["sums"][p][0], np.float64)  # 1/rowsum
            rowsum = 1.0 / np.maximum(rec, 1e-30)
            tails = np.zeros(T, np.float64)
            tails[WS:] = np.asarray(r["sums"][p][1][WS:], np.float64)
            a_exp = np.asarray(r["abar"][p], np.float64).T.reshape(2048)
            a_bar = a_exp / max(float(rowsum[T - 1]), 1e-30)
            avg_w = max(float(np.mean(a_bar[WS:])), 1e-6)
            prob = float(np.clip(a_bar[EVICT] / avg_w, 0.0, 1.0))
            prob = float(np.nan_to_num(prob, nan=0.0, posinf=1.0, neginf=0.0))
            m = 1.0 if U_CONST[b, h] < prob else 0.0
            if m == 0.0:
                continue
            # exact v_e from fp32 inputs
            v_row = hidden_states[b, EVICT, :] @ v_w[h * D:(h + 1) * D, :].T
            v_e = v_row * (m / RB)  # [D]
            w_e = o_w[:, h * D:(h + 1) * D] @ v_e  # [HID]
            s_tail = (tails / np.maximum(rowsum, 1e-30)).astype(np.float32)
            out[b] += np.outer(s_tail, w_e).astype(np.float32)
    return out


def kernel(hidden_states, attention_mask, q_w, k_w, v_w, o_w):
    from concourse.bass_utils import run_bass_kernel_spmd

    nc = _get_nc()
    in_maps = _host_inputs(hidden_states, q_w, k_w, v_w, o_w)
    trace = bool(int(os.environ.get("BK_TRACE", "0")))
    res = run_bass_kernel_spmd(
        nc, in_maps, core_ids=list(range(NCORES)), trace=trace,
    )
    if trace and res.exec_time_ns is not None:
        print(f"HW exec time: {res.exec_time_ns} ns")
        _NC_CACHE["last_exec_ns"] = res.exec_time_ns
        _NC_CACHE["last_trace"] = res.instructions_and_trace
    results = res.results

    acc = np.zeros((HID, BT), dtype=np.float32)
    for core in range(NCORES):
        acc += np.asarray(results[core]["outT"], np.float32)
    out = np.ascontiguousarray(acc.T).reshape(B, T, HID)
    out = _epilogue(out, results, hidden_states, v_w, o_w)
    return out.astype(np.float32)


# revision 7
# speedup vs baseline: 1.0562x; 1.0043x over previous
"""Trainium2 Bass kernel for nn_LlamaAttention_cam (sparse_attention).

Sharding: 16 heads across 8 cores (2 heads/core), both batches per core.
Q/K/V projections column-parallel over heads; o_proj row-parallel (per-core
partial outputs summed on host). The CaM merge is a rank-1 correction
(s_tail outer v_e) applied on host from tiny device-side statistics.

The projection GEMMs (QKV + o_proj) run as fp8e4 DoubleRow matmuls with
3-term error compensation: X*W ~ Xh*Wh + Xl*Wh + Xh*Wl where Xh = fp8(X),
Xl = fp8(X - Xh). DoubleRow packs a 256-deep contraction at 0.5 cyc/col,
so 3 terms cost 75% of the bf16 equivalent. hs and all weights are split
on the host (free); attn_out is split on-device (Pool engine). Weights are
pre-scaled by 64 into fp8 range; V inherits x64 which puts attn_out in
fp8 range too; the o_proj epilogue copy descales by 1/4096.

Self-contained: hardcodes all shapes; takes full inputs, returns full output.
"""

import math
import os

import numpy as np
import ml_dtypes

B, T, HID, H = 2, 2048, 2048, 16
D = 128
NCORES = 8
HL = H // NCORES  # heads per core = 2
BT = B * T  # 4096
NF = HID // 128  # 16 f-tiles
NG = NF // 2  # 8 f-tile pairs for DoubleRow
SCALE = 1.0 / math.sqrt(D)
RB = int(0.25 * T)  # 512 recent budget
WS = T - RB  # 1536
EVICT = WS - 1  # 1535
WSCL = 64.0  # fp8 pre-scale on wq/wk/wo
VSCL = 16.0  # fp8 pre-scale on wv: max |attn_out*VSCL| ~ 5sigma*16 = 72 < 240
ODESC = 1.0 / (VSCL * WSCL)  # o_proj descale: V carries x16, wo carries x64

# jax.random.uniform(jax.random.key(42), (2,16), float32); bernoulli(key,p) == u < p
U_CONST = np.array(
    [[0.59400654, 0.43801308, 0.6285691, 0.00791204, 0.27834702,
      0.7976179, 0.8521497, 0.9625306, 0.67656493, 0.11104441,
      0.4959929, 0.7311437, 0.18970704, 0.1544199, 0.03802836,
      0.33559263],
     [0.92825687, 0.6123972, 0.49262476, 0.733806, 0.18920851,
      0.15386605, 0.037136197, 0.32930005, 0.9372028, 0.5957513,
      0.4615929, 0.6695677, 0.07019377, 0.39408123, 0.55786455,
      0.35412872]], dtype=np.float32)

BF16 = ml_dtypes.bfloat16
F8 = ml_dtypes.float8_e4m3

_NC_CACHE = {}


def build_nc():
    import concourse.bacc as bacc
    import concourse.mybir as mybir
    import concourse.tile as tile

    f32 = mybir.dt.float32
    bf16 = mybir.dt.bfloat16
    f8 = mybir.dt.float8e4
    EXP = mybir.ActivationFunctionType.Exp
    DR = mybir.MatmulPerfMode.DoubleRow

    nc = bacc.Bacc("TRN2", target_bir_lowering=False, debug=False)
    env = os.environ
    B_QK = int(env.get("BK_QK", "5"))
    B_HSP = int(env.get("BK_HSP", "2"))
    B_ROPE = int(env.get("BK_ROPE", "3"))
    B_SPS = int(env.get("BK_SPS", "3"))
    B_OPS = int(env.get("BK_OPS", "1"))
    B_MSC = int(env.get("BK_MSC", "3"))
    B_PT = int(env.get("BK_PT", "10"))
    B_PR = int(env.get("BK_PR", "3"))
    B_OB = int(env.get("BK_OB", "8"))
    LOOK = int(env.get("BK_LOOK", "3"))
    ILV = env.get("BK_ILV", "1") == "1"
    CPY = env.get("BK_CPY", "vsvvs")  # per-fo copy engine cycle: v=DVE s=Act
    OBQ = env.get("BK_OBQ", "0") == "1"  # alternate ob DMA queues
    ILVN = int(env.get("BK_ILVN", "1"))  # oproj units per 2 j-steps

    hsh = nc.dram_tensor("hsh", [HID, BT], f8, kind="ExternalInput")
    hsl = nc.dram_tensor("hsl", [HID, BT], f8, kind="ExternalInput")
    wqh = nc.dram_tensor("wqh", [HID, 256], f8, kind="ExternalInput")
    wql = nc.dram_tensor("wql", [HID, 256], f8, kind="ExternalInput")
    wkh = nc.dram_tensor("wkh", [HID, 256], f8, kind="ExternalInput")
    wkl = nc.dram_tensor("wkl", [HID, 256], f8, kind="ExternalInput")
    wvh = nc.dram_tensor("wvh", [HID, 256], f8, kind="ExternalInput")
    wvl = nc.dram_tensor("wvl", [HID, 256], f8, kind="ExternalInput")
    woh = nc.dram_tensor("woh", [256, HID], f8, kind="ExternalInput")
    wol = nc.dram_tensor("wol", [256, HID], f8, kind="ExternalInput")
    cosd = nc.dram_tensor("cosT", [128, T], f32, kind="ExternalInput")
    sind = nc.dram_tensor("sinT", [128, T], f32, kind="ExternalInput")
    maskd = nc.dram_tensor("masks", [128, 2048], bf16, kind="ExternalInput")

    outT = nc.dram_tensor("outT", [HID, BT], bf16, kind="ExternalOutput")
    abard = nc.dram_tensor("abar", [4, 128, 16], f32, kind="ExternalOutput")
    sumsd = nc.dram_tensor("sums", [4, 2, T], f32, kind="ExternalOutput")

    with tile.TileContext(nc) as tc:
        with (
            tc.tile_pool(name="singles", bufs=1) as singles,
            tc.tile_pool(name="res", bufs=1) as res,
            tc.tile_pool(name="stats", bufs=1) as stats,
        ):
            # --- small constants (no DMA deps) for the PE warm-up ---
            ones_a = singles.tile([128, 2], bf16, tag="onesa")  # [1, 0]
            ones_b = singles.tile([128, 2], bf16, tag="onesb")  # [1, 1]
            nc.vector.memset(ones_a[:, 0:1], 1.0)
            nc.vector.memset(ones_a[:, 1:2], 0.0)
            nc.vector.memset(ones_b, 1.0)
            wsrc = singles.tile([128, 16], bf16, tag="wsrc")
            nc.vector.memset(wsrc, 0.0)

            # --- constant tiles (fp8 hi/lo weight pairs) ---
            wq_sb = [singles.tile([128, NF, 256], f8, tag=f"wq{i}",
                                   name=f"wq{i}")
                     for i in range(2)]
            wk_sb = [singles.tile([128, NF, 256], f8, tag=f"wk{i}",
                                   name=f"wk{i}")
                     for i in range(2)]
            wv_sb = [singles.tile([128, NF, 256], f8, tag=f"wv{i}",
                                   name=f"wv{i}")
                     for i in range(2)]
            wo_sb = [singles.tile([128, 2, HID], f8, tag=f"wo{i}",
                                   name=f"wo{i}")
                     for i in range(2)]
            cos_sb = singles.tile([128, T], f32, tag="cos")
            sin_sb = singles.tile([128, T], f32, tag="sin")
            mask_sb = singles.tile([128, 4, 512], bf16, tag="mask")

            # --- residents ---
            qt = [res.tile([128, BT], bf16, tag=f"qt{h}", name=f"qt{h}")
                  for h in range(HL)]
            kt = [res.tile([128, BT], bf16, tag=f"kt{h}", name=f"kt{h}")
                  for h in range(HL)]
            vres = res.tile([128, 32, 256], bf16, tag="vres")
            # attn_out hi/lo fp8, [d, head, t] per batch
            aoh = [res.tile([128, 2, T], f8, tag=f"aoh{b}", name=f"aoh{b}")
                   for b in range(B)]
            aol = [res.tile([128, 2, T], f8, tag=f"aol{b}", name=f"aol{b}")
                   for b in range(B)]
            abar_raw = [stats.tile([128, 16], f32, tag=f"ab{p}", name=f"ab{p}")
                        for p in range(4)]

            # ================= Phase 1: QKV projections + RoPE ================
            with (
                tc.tile_pool(name="hsp", bufs=B_HSP) as hsp,
                tc.tile_pool(name="rope", bufs=B_ROPE) as rope,
                tc.tile_pool(name="qkps", bufs=B_QK, space="PSUM") as qkps,
                tc.tile_pool(name="vps", bufs=2, space="PSUM") as vps,
                tc.tile_pool(name="wps", bufs=1, space="PSUM") as wps,
            ):
                # PE warm-up: a tiny matmul at t~0 starts the p-state ramp so
                # the real matmuls (gated on DMA) run at full clock.
                warm = wps.tile([2, 16], f32, tag="warm")
                nc.tensor.matmul(warm, lhsT=ones_a, rhs=wsrc,
                                 start=True, stop=True)

                # DMA issue order is the sync-queue service order; front-load
                # exactly what the first matmuls need (wq_hi + hs_hi chunk 0).
                hs0 = [hsp.tile([128, NF, 512], f8, tag=f"hs{i}",
                                name=f"hs0{i}") for i in range(2)]
                for fh in range(2):
                    for i, (wsrc_d, hsrc_d) in enumerate([(wqh, hsh),
                                                          (wql, hsl)]):
                        rsl = slice(fh * 1024, (fh + 1) * 1024)
                        fsl = slice(fh * 8, (fh + 1) * 8)
                        nc.sync.dma_start(
                            out=wq_sb[i][:, fsl, :],
                            in_=wsrc_d[rsl, :].rearrange(
                                "(nf p) d -> p nf d", p=128))
                        nc.sync.dma_start(
                            out=hs0[i][:, fsl, :],
                            in_=hsrc_d[rsl, 0:512].rearrange(
                                "(nf p) t -> p nf t", p=128))
                nc.sync.dma_start(
                    out=wv_sb[0],
                    in_=wvh.rearrange("(nf p) d -> p nf d", p=128))
                nc.sync.dma_start(
                    out=wv_sb[1],
                    in_=wvl.rearrange("(nf p) d -> p nf d", p=128))
                nc.sync.dma_start(
                    out=wk_sb[0],
                    in_=wkh.rearrange("(nf p) d -> p nf d", p=128))
                nc.sync.dma_start(
                    out=wk_sb[1],
                    in_=wkl.rearrange("(nf p) d -> p nf d", p=128))
                # hs chunk 1 must beat the chunk-0 compute tail; cos/sin only
                # gate RoPE (vector work), mask/wo only phase 2.
                hs1 = [hsp.tile([128, NF, 512], f8, tag=f"hs{i}",
                                name=f"hs1{i}") for i in range(2)]
                nc.sync.dma_start(
                    out=hs1[0],
                    in_=hsh[:, 512:1024].rearrange("(nf p) t -> p nf t",
                                                   p=128))
                nc.sync.dma_start(
                    out=hs1[1],
                    in_=hsl[:, 512:1024].rearrange("(nf p) t -> p nf t",
                                                   p=128))
                nc.gpsimd.dma_start(out=cos_sb, in_=cosd[:, :])
                nc.gpsimd.dma_start(out=sin_sb, in_=sind[:, :])
                nc.gpsimd.dma_start(
                    out=mask_sb, in_=maskd.rearrange("p (v t) -> p v t", v=4))
                nc.gpsimd.dma_start(
                    out=wo_sb[0], in_=woh.rearrange("(kt p) f -> p kt f",
                                                    p=128))
                nc.gpsimd.dma_start(
                    out=wo_sb[1], in_=wol.rearrange("(kt p) f -> p kt f",
                                                    p=128))

                def rope_apply(ps, dest, tl, swap_eng=nc.gpsimd,
                               late=False):
                    qf = rope.tile([128, 512], f32, tag="qf")
                    if late:
                        nc.vector.tensor_copy(qf, ps)
                    else:
                        nc.scalar.copy(qf, ps)
                    rot = rope.tile([128, 512], f32, tag="rot")
                    swap_eng.dma_start(out=rot[0:64, :], in_=qf[64:128, :])
                    swap_eng.dma_start(out=rot[64:128, :], in_=qf[0:64, :])
                    t1 = rope.tile([128, 512], f32, tag="t1")
                    nc.vector.tensor_mul(t1, rot, sin_sb[:, tl])
                    t2 = rope.tile([128, 512], f32, tag="t2")
                    nc.vector.tensor_mul(t2, qf, cos_sb[:, tl])
                    nc.vector.tensor_add(dest, t1, t2)

                def qk_matmuls(ps, w_pair, hs_pair, h):
                    # 3-term fp8 DoubleRow: hi@hi, lo(w)@hi, hi(w)@lo
                    hsel = slice(h * 128, (h + 1) * 128)
                    terms = [(0, 0), (1, 0), (0, 1)]
                    for ti, (wi, xi) in enumerate(terms):
                        for g in range(NG):
                            nc.tensor.matmul(
                                ps,
                                lhsT=w_pair[wi][:, 2 * g:2 * g + 2, hsel],
                                rhs=hs_pair[xi][:, 2 * g:2 * g + 2, :],
                                start=(ti == 0 and g == 0),
                                stop=(ti == 2 and g == NG - 1),
                                perf_mode=DR)

                for c in range(8):
                    cs = slice(c * 512, (c + 1) * 512)
                    tl = slice((c % 4) * 512, (c % 4) * 512 + 512)
                    if c == 0:
                        hs_t = hs0
                    elif c == 1:
                        hs_t = hs1
                    else:
                        hs_t = [hsp.tile([128, NF, 512], f8, tag=f"hs{i}",
                                         name=f"hs{i}")
                                for i in range(2)]
                        nc.sync.dma_start(
                            out=hs_t[0],
                            in_=hsh[:, cs].rearrange("(nf p) t -> p nf t",
                                                     p=128))
                        nc.sync.dma_start(
                            out=hs_t[1],
                            in_=hsl[:, cs].rearrange("(nf p) t -> p nf t",
                                                     p=128))
                    # Q (h0, h1) -> V -> K (h0, h1): matches DMA arrivals.
                    # Chunk 0: emit hi@hi g-halves first (first-half DMAs
                    # only), then the lo terms which need wql/hsl.
                    if c == 0:
                        qps = [qkps.tile([128, 512], f32, tag="qk",
                                         name=f"qps{h}") for h in range(HL)]
                        for gh in range(2):
                            for ti, (wi, xi) in enumerate(
                                    [(0, 0), (1, 0), (0, 1)]):
                                for h in range(HL):
                                    for g in range(gh * 4, gh * 4 + 4):
                                        nc.tensor.matmul(
                                            qps[h],
                                            lhsT=wq_sb[wi][
                                                :, 2 * g:2 * g + 2,
                                                h * 128:(h + 1) * 128],
                                            rhs=hs_t[xi][:, 2 * g:2 * g + 2,
                                                         :],
                                            start=(gh == 0 and ti == 0
                                                   and g == 0),
                                            stop=(gh == 1 and ti == 2
                                                  and g == 7),
                                            perf_mode=DR)
                        for h in range(HL):
                            rope_apply(qps[h], qt[h][:, cs], tl, nc.gpsimd)
                    else:
                        for h in range(HL):
                            ps = qkps.tile([128, 512], f32, tag="qk")
                            qk_matmuls(ps, wq_sb, hs_t, h)
                            rope_apply(ps, qt[h][:, cs], tl,
                                       nc.gpsimd if c < 5 else nc.sync,
                                       late=(c >= 6))
                    for s in range(4):
                        vp = vps.tile([128, 256], f32, tag="v")
                        ssel = slice(s * 128, (s + 1) * 128)
                        terms = [(0, 0), (1, 0), (0, 1)]
                        for ti, (xi, wi) in enumerate(terms):
                            for g in range(NG):
                                nc.tensor.matmul(
                                    vp,
                                    lhsT=hs_t[xi][:, 2 * g:2 * g + 2, ssel],
                                    rhs=wv_sb[wi][:, 2 * g:2 * g + 2, :],
                                    start=(ti == 0 and g == 0),
                                    stop=(ti == 2 and g == NG - 1),
                                    perf_mode=DR)
                        # keep Act free near the phase boundary: route the
                        # last chunks' V copies through DVE
                        if c >= 6:
                            nc.vector.tensor_copy(vres[:, c * 4 + s, :], vp)
                        else:
                            nc.scalar.copy(vres[:, c * 4 + s, :], vp)
                    for h in range(HL):
                        ps = qkps.tile([128, 512], f32, tag="qk")
                        qk_matmuls(ps, wk_sb, hs_t, h)
                        rope_apply(ps, kt[h][:, cs], tl,
                                   nc.gpsimd if c < 5 else nc.sync,
                                   late=(c >= 6))

            # ========== Phase 2+3: attention + interleaved o_proj ==========
            with (
                tc.tile_pool(name="sps", bufs=B_SPS, space="PSUM") as sps,
                tc.tile_pool(name="ops", bufs=B_OPS, space="PSUM") as ops,
                tc.tile_pool(name="msc", bufs=B_MSC, space="PSUM") as msc,
                tc.tile_pool(name="smp", bufs=1, space="PSUM") as smp,
                tc.tile_pool(name="pt", bufs=B_PT) as ptp,
                tc.tile_pool(name="pr", bufs=B_PR) as prp,
                tc.tile_pool(name="att_sm", bufs=int(env.get("BK_SM", "4"))) as atsm,
                tc.tile_pool(name="ob", bufs=B_OB) as obp,
            ):
                pending = []

                def emit_unit():
                    if pending:
                        pending.pop(0)()

                def flush_units():
                    while pending:
                        pending.pop(0)()

                def enqueue_oproj(b, c, gsz=4):
                    tl = slice(c * 512, (c + 1) * 512)
                    state = {}
                    # GPSIMD cannot read PSUM; mix DVE/Act (Act carries exps)
                    copy_engines = [
                        ((lambda o, i: nc.vector.tensor_scalar_mul(o, i, ODESC))
                         if ch == "v" else
                         (lambda o, i: nc.scalar.mul(o, i, ODESC)))
                        for ch in CPY]

                    def unit(fo):
                        def f():
                            g = fo // gsz
                            if g not in state:
                                state[g] = obp.tile([128, gsz, 512], bf16,
                                                    tag=f"ob{gsz}",
                                                    name="ob_t")
                            ob_t = state[g]
                            fs = slice(fo * 128, (fo + 1) * 128)
                            pp = msc.tile([128, 512], f32, tag="pp")
                            nc.tensor.matmul(
                                pp, lhsT=wo_sb[0][:, :, fs],
                                rhs=aoh[b][:, :, tl],
                                start=True, stop=False, perf_mode=DR)
                            nc.tensor.matmul(
                                pp, lhsT=wo_sb[1][:, :, fs],
                                rhs=aoh[b][:, :, tl],
                                start=False, stop=False, perf_mode=DR)
                            nc.tensor.matmul(
                                pp, lhsT=wo_sb[0][:, :, fs],
                                rhs=aol[b][:, :, tl],
                                start=False, stop=True, perf_mode=DR)
                            copy_engines[fo % len(CPY)](
                                ob_t[:, fo % gsz, :], pp)
                            if fo % gsz == gsz - 1:
                                rows = slice(g * gsz * 128,
                                             (g + 1) * gsz * 128)
                                cg = slice((b * 4 + c) * 512,
                                           (b * 4 + c + 1) * 512)
                                eng = (nc.gpsimd if (OBQ and g % 2 == 1)
                                       else nc.sync)
                                eng.dma_start(
                                    out=outT[rows, cg].rearrange(
                                        "(nf p) t -> p nf t", p=128),
                                    in_=ob_t)
                        return f

                    for fo in range(16):
                        pending.append(unit(fo))

                # Global step stream: score-matmul lookahead crosses chunk
                # boundaries so the next chunk's exps run during the previous
                # chunk's tail (norm chain / oproj flush) with no PE bubble.
                class Chunk:
                    def __init__(self, p, c):
                        self.p, self.c = p, c
                        self.b, self.h = p // 2, p % 2
                        self.jmax = 4 * (c + 1)
                        self.o_ps = None
                        self.sm_ps = None
                        self.sm_started = False
                        self.pts = {}
                        self.prs = []
                        self.sq = []

                def tile_off(ck, j):
                    # Diagonal k-tile v=1..3: first 128v query cols are fully
                    # causal-masked -> compute only cols [128v:512]. Exact.
                    v = j - 4 * ck.c
                    return 128 * v if 1 <= v <= 3 else 0

                def emit_s(ck, j):
                    b, c = ck.b, ck.c
                    off = tile_off(ck, j)
                    sp = sps.tile([128, 512], f32, tag="s", name="sp")
                    nc.tensor.matmul(
                        sp[:, off:],
                        lhsT=kt[ck.h][:, b * T + j * 128:
                                      b * T + (j + 1) * 128],
                        rhs=qt[ck.h][:, b * T + c * 512 + off:
                                     b * T + (c + 1) * 512],
                        start=True, stop=True)
                    ck.sq.append(sp)

                def emit_epv(ck, j):
                    p, c, b, h = ck.p, ck.c, ck.b, ck.h
                    off = tile_off(ck, j)
                    sp = ck.sq[j]
                    pt_t = ptp.tile([128, 512], bf16, tag="p", name="pt_t")
                    nc.scalar.activation(pt_t[:, off:], sp[:, off:],
                                         EXP, scale=SCALE)
                    if j >= 4 * c:
                        nc.vector.tensor_mul(pt_t[:, off:], pt_t[:, off:],
                                             mask_sb[:, j - 4 * c, off:])
                    if c == 3:
                        nc.vector.tensor_copy(
                            abar_raw[p][:, j:j + 1], sp[:, 511:512])
                    if ck.o_ps is None:
                        ck.o_ps = ops.tile([128, 512], f32, tag="o",
                                           name="o_ps")
                    nc.tensor.matmul(
                        ck.o_ps[:, off:],
                        lhsT=vres[:, b * 16 + j, h * 128:(h + 1) * 128],
                        rhs=pt_t[:, off:],
                        start=(j == 0), stop=(j == ck.jmax - 1))
                    if ck.sm_ps is None:
                        ck.sm_ps = smp.tile([2, 512], f32, tag="sm",
                                            name="sm_ps")
                    if j >= 4 * c:
                        # diagonal tile: individual (possibly trimmed) rowsum
                        nc.tensor.matmul(
                            ck.sm_ps[:, off:],
                            lhsT=(ones_b if c == 3 else ones_a),
                            rhs=pt_t[:, off:],
                            start=(j == 4 * c and c == 0),
                            stop=(j == ck.jmax - 1))
                        return
                    ck.pts[j] = pt_t
                    if j % 2 == 1:
                        pr = prp.tile([128, 512], bf16, tag="pr", name="pr")
                        nc.vector.tensor_add(pr, ck.pts[j - 1], ck.pts[j])
                        nc.tensor.matmul(
                            ck.sm_ps,
                            lhsT=ones_a,
                            rhs=pr,
                            start=(j == 1), stop=False)
                        del ck.pts[j - 1], ck.pts[j]

                def epilogue(ck):
                    p, c, b, hl = ck.p, ck.c, ck.b, ck.h
                    cl = slice(c * 512, (c + 1) * 512)
                    rec = atsm.tile([1, 512], f32, tag="rec", name="rec")
                    nc.vector.reciprocal(rec, ck.sm_ps[0:1, :])
                    bc = atsm.tile([128, 512], f32, tag="bc", name="bc")
                    nc.gpsimd.partition_broadcast(bc, rec)
                    full = atsm.tile([128, 512], bf16, tag="full",
                                     name="full")
                    nc.vector.tensor_mul(full, ck.o_ps, bc)
                    # fp8 hi/lo split on Pool (Act does exps, DVE the rest)
                    nc.gpsimd.tensor_copy(aoh[b][:, hl, cl], full)
                    nc.gpsimd.tensor_sub(aol[b][:, hl, cl], full,
                                         aoh[b][:, hl, cl])
                    nc.sync.dma_start(out=sumsd[p, 0:1, cl], in_=rec)
                    if c == 3:
                        tl_sb = atsm.tile([2, 512], f32, tag="smsb",
                                          name="tl_sb")
                        nc.vector.tensor_copy(tl_sb, ck.sm_ps)
                        nc.sync.dma_start(out=sumsd[p, 1:2, cl],
                                          in_=tl_sb[1:2, :])
                    if c == 3:
                        ab_exp = atsm.tile([128, 16], f32, tag="abe",
                                           name="ab_exp")
                        nc.scalar.activation(
                            ab_exp, abar_raw[p], EXP, scale=SCALE)
                        nc.sync.dma_start(out=abard[p], in_=ab_exp)

                chunks = [Chunk(b * 2 + hl, c)
                          for b in range(B) for c in range(4)
                          for hl in range(HL)]
                steps = [(ck, j) for ck in chunks for j in range(ck.jmax)]
                for k in range(LOOK):
                    emit_s(*steps[k])
                for i, (ck, j) in enumerate(steps):
                    if i + LOOK < len(steps):
                        emit_s(*steps[i + LOOK])
                    emit_epv(ck, j)
                    if ILV and j % 2 == 1:
                        for _ in range(ILVN):
                            emit_unit()
                    if j == ck.jmax - 1:
                        epilogue(ck)
                        if ck.h == 1:
                            flush_units()
                            enqueue_oproj(
                                ck.b, ck.c,
                                gsz=(2 if (ck.b, ck.c) == (1, 3) else 4))
                flush_units()

    nc.compile()
    return nc


def _get_nc():
    if "nc" not in _NC_CACHE:
        _NC_CACHE["nc"] = build_nc()
    return _NC_CACHE["nc"]


def _split8(x):
    hi = x.astype(F8)
    lo = (x - hi.astype(np.float32)).astype(F8)
    return hi, lo


def _host_inputs(hidden_states, q_w, k_w, v_w, o_w):
    """Per-core input dicts."""
    hsT = np.ascontiguousarray(hidden_states.reshape(BT, HID).T)
    hs_hi, hs_lo = _split8(hsT)
    inv = 10000.0 ** (-np.arange(64, dtype=np.float64) / 64.0)
    t = np.arange(T, dtype=np.float64)
    fr = t[None, :] * inv[:, None]  # [64, T]
    # 1/WSCL descale of the x64-scaled Q/K baked into the rope tables
    cosT = (np.cos(np.concatenate([fr, fr], 0)) / WSCL).astype(np.float32)
    sinT = (np.sin(np.concatenate([fr, fr], 0)) / WSCL).astype(np.float32)
    sinT[:64] *= -1.0  # sign-baked for swap-halves rotate
    masks = np.zeros((128, 4, 512), dtype=np.float32)
    kk = np.arange(128)[:, None]
    tt = np.arange(512)[None, :]
    for v in range(4):
        masks[:, v, :] = (tt >= 128 * v + kk).astype(np.float32)
    masks = masks.reshape(128, 2048).astype(BF16)

    in_maps = []
    for core in range(NCORES):
        rs = slice(core * 256, (core + 1) * 256)
        wq_hi, wq_lo = _split8(
            WSCL * np.ascontiguousarray(q_w[rs, :].T))
        wk_hi, wk_lo = _split8(
            WSCL * np.ascontiguousarray(k_w[rs, :].T))
        wv_hi, wv_lo = _split8(
            VSCL * np.ascontiguousarray(v_w[rs, :].T))
        wo_hi, wo_lo = _split8(
            WSCL * np.ascontiguousarray(o_w[:, rs].T))
        in_maps.append({
            "hsh": hs_hi,
            "hsl": hs_lo,
            "wqh": wq_hi, "wql": wq_lo,
            "wkh": wk_hi, "wkl": wk_lo,
            "wvh": wv_hi, "wvl": wv_lo,
            "woh": wo_hi, "wol": wo_lo,
            "cosT": cosT,
            "sinT": sinT,
            "masks": masks,
        })
    return in_maps


def _epilogue(out, results, hidden_states, v_w, o_w):
    """Add the CaM rank-1 correction per (b, h) on host."""
    for core in range(NCORES):
        r = results[core]
        for p in range(4):
            b, hl = p // 2, p % 2
            h = core * HL + hl
            rec = np.asarray(r["sums"][p][0], np.float64)  # 1/rowsum
            rowsum = 1.0 / np.maximum(rec, 1e-30)
            tails = np.zeros(T, np.float64)
            tails[WS:] = np.asarray(r["sums"][p][1][WS:], np.float64)
            a_exp = np.asarray(r["abar"][p], np.float64).T.reshape(2048)
            a_bar = a_exp / max(float(rowsum[T - 1]), 1e-30)
            avg_w = max(float(np.mean(a_bar[WS:])), 1e-6)
            prob = float(np.clip(a_bar[EVICT] / avg_w, 0.0, 1.0))
            prob = float(np.nan_to_num(prob, nan=0.0, posinf=1.0, neginf=0.0))
            m = 1.0 if U_CONST[b, h] < prob else 0.0
            if m == 0.0:
                continue
            # exact v_e from fp32 inputs
            v_row = hidden_states[b, EVICT, :] @ v_w[h * D:(h + 1) * D, :].T
            v_e = v_row * (m / RB)  # [D]
            w_e = o_w[:, h * D:(h + 1) * D] @ v_e  # [HID]
            s_tail = (tails / np.maximum(rowsum, 1e-30)).astype(np.float32)
            out[b] += np.outer(s_tail, w_e).astype(np.float32)
    return out


def kernel(hidden_states, attention_mask, q_w, k_w, v_w, o_w):
    from concourse.bass_utils import run_bass_kernel_spmd

    nc = _get_nc()
    in_maps = _host_inputs(hidden_states, q_w, k_w, v_w, o_w)
    trace = bool(int(os.environ.get("BK_TRACE", "0")))
    res = run_bass_kernel_spmd(
        nc, in_maps, core_ids=list(range(NCORES)), trace=trace,
    )
    if trace and res.exec_time_ns is not None:
        print(f"HW exec time: {res.exec_time_ns} ns")
        _NC_CACHE["last_exec_ns"] = res.exec_time_ns
        _NC_CACHE["last_trace"] = res.instructions_and_trace
    results = res.results

    acc = np.zeros((HID, BT), dtype=np.float32)
    for core in range(NCORES):
        acc += np.asarray(results[core]["outT"], np.float32)
    out = np.ascontiguousarray(acc.T).reshape(B, T, HID)
    out = _epilogue(out, results, hidden_states, v_w, o_w)
    return out.astype(np.float32)


# revision 15
# speedup vs baseline: 1.0605x; 1.0041x over previous
"""Trainium2 Bass kernel for nn_LlamaAttention_cam (sparse_attention).

Sharding: 16 heads across 8 cores (2 heads/core), both batches per core.
Q/K/V projections column-parallel over heads; o_proj row-parallel (per-core
partial outputs summed on host). The CaM merge is a rank-1 correction
(s_tail outer v_e) applied on host from tiny device-side statistics.

The projection GEMMs (QKV + o_proj) run as fp8e4 DoubleRow matmuls with
3-term error compensation: X*W ~ Xh*Wh + Xl*Wh + Xh*Wl where Xh = fp8(X),
Xl = fp8(X - Xh). DoubleRow packs a 256-deep contraction at 0.5 cyc/col,
so 3 terms cost 75% of the bf16 equivalent. hs and all weights are split
on the host (free); attn_out is split on-device (Pool engine). Weights are
pre-scaled by 64 into fp8 range; V inherits x64 which puts attn_out in
fp8 range too; the o_proj epilogue copy descales by 1/4096.

Self-contained: hardcodes all shapes; takes full inputs, returns full output.
"""

import math
import os

import numpy as np
import ml_dtypes

B, T, HID, H = 2, 2048, 2048, 16
D = 128
NCORES = 8
HL = H // NCORES  # heads per core = 2
BT = B * T  # 4096
NF = HID // 128  # 16 f-tiles
NG = NF // 2  # 8 f-tile pairs for DoubleRow
SCALE = 1.0 / math.sqrt(D)
RB = int(0.25 * T)  # 512 recent budget
WS = T - RB  # 1536
EVICT = WS - 1  # 1535
WSCL = 64.0  # fp8 pre-scale on wq/wk/wo
VSCL = 16.0  # fp8 pre-scale on wv: max |attn_out*VSCL| ~ 5sigma*16 = 72 < 240
ODESC = 1.0 / (VSCL * WSCL)  # o_proj descale: V carries x16, wo carries x64

# jax.random.uniform(jax.random.key(42), (2,16), float32); bernoulli(key,p) == u < p
U_CONST = np.array(
    [[0.59400654, 0.43801308, 0.6285691, 0.00791204, 0.27834702,
      0.7976179, 0.8521497, 0.9625306, 0.67656493, 0.11104441,
      0.4959929, 0.7311437, 0.18970704, 0.1544199, 0.03802836,
      0.33559263],
     [0.92825687, 0.6123972, 0.49262476, 0.733806, 0.18920851,
      0.15386605, 0.037136197, 0.32930005, 0.9372028, 0.5957513,
      0.4615929, 0.6695677, 0.07019377, 0.39408123, 0.55786455,
      0.35412872]], dtype=np.float32)

BF16 = ml_dtypes.bfloat16
F8 = ml_dtypes.float8_e4m3

_NC_CACHE = {}


def build_nc():
    import concourse.bacc as bacc
    import concourse.mybir as mybir
    import concourse.tile as tile

    f32 = mybir.dt.float32
    bf16 = mybir.dt.bfloat16
    f8 = mybir.dt.float8e4
    EXP = mybir.ActivationFunctionType.Exp
    DR = mybir.MatmulPerfMode.DoubleRow

    nc = bacc.Bacc("TRN2", target_bir_lowering=False, debug=False)
    env = os.environ
    B_QK = int(env.get("BK_QK", "5"))
    B_HSP = int(env.get("BK_HSP", "2"))
    B_ROPE = int(env.get("BK_ROPE", "3"))
    B_SPS = int(env.get("BK_SPS", "3"))
    B_OPS = int(env.get("BK_OPS", "1"))
    B_MSC = int(env.get("BK_MSC", "3"))
    B_PT = int(env.get("BK_PT", "10"))
    B_PR = int(env.get("BK_PR", "3"))
    B_OB = int(env.get("BK_OB", "8"))
    LOOK = int(env.get("BK_LOOK", "4"))
    ILV = env.get("BK_ILV", "1") == "1"
    CPY = env.get("BK_CPY", "vsvvs")  # per-fo copy engine cycle: v=DVE s=Act
    OBQ = env.get("BK_OBQ", "0") == "1"  # alternate ob DMA queues
    ILVN = int(env.get("BK_ILVN", "1"))  # oproj units per 2 j-steps

    hsh = nc.dram_tensor("hsh", [HID, BT], f8, kind="ExternalInput")
    hsl = nc.dram_tensor("hsl", [HID, BT], f8, kind="ExternalInput")
    wqh = nc.dram_tensor("wqh", [HID, 256], f8, kind="ExternalInput")
    wql = nc.dram_tensor("wql", [HID, 256], f8, kind="ExternalInput")
    wkh = nc.dram_tensor("wkh", [HID, 256], f8, kind="ExternalInput")
    wkl = nc.dram_tensor("wkl", [HID, 256], f8, kind="ExternalInput")
    wvh = nc.dram_tensor("wvh", [HID, 256], f8, kind="ExternalInput")
    wvl = nc.dram_tensor("wvl", [HID, 256], f8, kind="ExternalInput")
    woh = nc.dram_tensor("woh", [256, HID], f8, kind="ExternalInput")
    wol = nc.dram_tensor("wol", [256, HID], f8, kind="ExternalInput")
    cosd = nc.dram_tensor("cosT", [128, T], f32, kind="ExternalInput")
    sind = nc.dram_tensor("sinT", [128, T], f32, kind="ExternalInput")
    maskd = nc.dram_tensor("masks", [128, 2048], bf16, kind="ExternalInput")

    outT = nc.dram_tensor("outT", [HID, BT], bf16, kind="ExternalOutput")
    abard = nc.dram_tensor("abar", [4, 128, 16], f32, kind="ExternalOutput")
    sumsd = nc.dram_tensor("sums", [4, 2, T], f32, kind="ExternalOutput")

    with tile.TileContext(nc) as tc:
        with (
            tc.tile_pool(name="singles", bufs=1) as singles,
            tc.tile_pool(name="res", bufs=1) as res,
            tc.tile_pool(name="stats", bufs=1) as stats,
        ):
            # --- small constants (no DMA deps) for the PE warm-up ---
            ones_a = singles.tile([128, 2], bf16, tag="onesa")  # [1, 0]
            ones_b = singles.tile([128, 2], bf16, tag="onesb")  # [1, 1]
            nc.vector.memset(ones_a[:, 0:1], 1.0)
            nc.vector.memset(ones_a[:, 1:2], 0.0)
            nc.vector.memset(ones_b, 1.0)
            wsrc = singles.tile([128, 16], bf16, tag="wsrc")
            nc.vector.memset(wsrc, 0.0)

            # --- constant tiles (fp8 hi/lo weight pairs) ---
            wq_sb = [singles.tile([128, NF, 256], f8, tag=f"wq{i}",
                                   name=f"wq{i}")
                     for i in range(2)]
            wk_sb = [singles.tile([128, NF, 256], f8, tag=f"wk{i}",
                                   name=f"wk{i}")
                     for i in range(2)]
            wv_sb = [singles.tile([128, NF, 256], f8, tag=f"wv{i}",
                                   name=f"wv{i}")
                     for i in range(2)]
            wo_sb = [singles.tile([128, 2, HID], f8, tag=f"wo{i}",
                                   name=f"wo{i}")
                     for i in range(2)]
            cos_sb = singles.tile([128, T], f32, tag="cos")
            sin_sb = singles.tile([128, T], f32, tag="sin")
            mask_sb = singles.tile([128, 4, 512], bf16, tag="mask")

            # --- residents ---
            qt = [res.tile([128, BT], bf16, tag=f"qt{h}", name=f"qt{h}")
                  for h in range(HL)]
            kt = [res.tile([128, BT], bf16, tag=f"kt{h}", name=f"kt{h}")
                  for h in range(HL)]
            vres = res.tile([128, 32, 256], bf16, tag="vres")
            # attn_out hi/lo fp8, [d, head, t] per batch
            aoh = [res.tile([128, 2, T], f8, tag=f"aoh{b}", name=f"aoh{b}")
                   for b in range(B)]
            aol = [res.tile([128, 2, T], f8, tag=f"aol{b}", name=f"aol{b}")
                   for b in range(B)]
            abar_raw = [stats.tile([128, 16], f32, tag=f"ab{p}", name=f"ab{p}")
                        for p in range(4)]

            # ================= Phase 1: QKV projections + RoPE ================
            with (
                tc.tile_pool(name="hsp", bufs=B_HSP) as hsp,
                tc.tile_pool(name="rope", bufs=B_ROPE) as rope,
                tc.tile_pool(name="qkps", bufs=B_QK, space="PSUM") as qkps,
                tc.tile_pool(name="vps", bufs=2, space="PSUM") as vps,
                tc.tile_pool(name="wps", bufs=1, space="PSUM") as wps,
            ):
                # PE warm-up: a tiny matmul at t~0 starts the p-state ramp so
                # the real matmuls (gated on DMA) run at full clock.
                warm = wps.tile([2, 16], f32, tag="warm")
                nc.tensor.matmul(warm, lhsT=ones_a, rhs=wsrc,
                                 start=True, stop=True)

                # DMA issue order is the sync-queue service order; front-load
                # exactly what the first matmuls need (wq_hi + hs_hi chunk 0).
                hs0 = [hsp.tile([128, NF, 512], f8, tag=f"hs{i}",
                                name=f"hs0{i}") for i in range(2)]
                for fh in range(2):
                    for i, (wsrc_d, hsrc_d) in enumerate([(wqh, hsh),
                                                          (wql, hsl)]):
                        rsl = slice(fh * 1024, (fh + 1) * 1024)
                        fsl = slice(fh * 8, (fh + 1) * 8)
                        nc.sync.dma_start(
                            out=wq_sb[i][:, fsl, :],
                            in_=wsrc_d[rsl, :].rearrange(
                                "(nf p) d -> p nf d", p=128))
                        nc.sync.dma_start(
                            out=hs0[i][:, fsl, :],
                            in_=hsrc_d[rsl, 0:512].rearrange(
                                "(nf p) t -> p nf t", p=128))
                nc.sync.dma_start(
                    out=wv_sb[0],
                    in_=wvh.rearrange("(nf p) d -> p nf d", p=128))
                nc.sync.dma_start(
                    out=wv_sb[1],
                    in_=wvl.rearrange("(nf p) d -> p nf d", p=128))
                nc.sync.dma_start(
                    out=wk_sb[0],
                    in_=wkh.rearrange("(nf p) d -> p nf d", p=128))
                nc.sync.dma_start(
                    out=wk_sb[1],
                    in_=wkl.rearrange("(nf p) d -> p nf d", p=128))
                # hs chunk 1 must beat the chunk-0 compute tail; cos/sin only
                # gate RoPE (vector work), mask/wo only phase 2.
                hs1 = [hsp.tile([128, NF, 512], f8, tag=f"hs{i}",
                                name=f"hs1{i}") for i in range(2)]
                nc.sync.dma_start(
                    out=hs1[0],
                    in_=hsh[:, 512:1024].rearrange("(nf p) t -> p nf t",
                                                   p=128))
                nc.sync.dma_start(
                    out=hs1[1],
                    in_=hsl[:, 512:1024].rearrange("(nf p) t -> p nf t",
                                                   p=128))
                nc.gpsimd.dma_start(out=cos_sb, in_=cosd[:, :])
                nc.gpsimd.dma_start(out=sin_sb, in_=sind[:, :])
                nc.gpsimd.dma_start(
                    out=mask_sb, in_=maskd.rearrange("p (v t) -> p v t", v=4))
                nc.gpsimd.dma_start(
                    out=wo_sb[0], in_=woh.rearrange("(kt p) f -> p kt f",
                                                    p=128))
                nc.gpsimd.dma_start(
                    out=wo_sb[1], in_=wol.rearrange("(kt p) f -> p kt f",
                                                    p=128))

                def rope_apply(ps, dest, tl, swap_eng=nc.gpsimd,
                               late=False):
                    qf = rope.tile([128, 512], f32, tag="qf")
                    if late:
                        nc.vector.tensor_copy(qf, ps)
                    else:
                        nc.scalar.copy(qf, ps)
                    rot = rope.tile([128, 512], f32, tag="rot")
                    swap_eng.dma_start(out=rot[0:64, :], in_=qf[64:128, :])
                    swap_eng.dma_start(out=rot[64:128, :], in_=qf[0:64, :])
                    t1 = rope.tile([128, 512], f32, tag="t1")
                    nc.vector.tensor_mul(t1, rot, sin_sb[:, tl])
                    t2 = rope.tile([128, 512], f32, tag="t2")
                    nc.vector.tensor_mul(t2, qf, cos_sb[:, tl])
                    nc.vector.tensor_add(dest, t1, t2)

                def qk_matmuls(ps, w_pair, hs_pair, h):
                    # 3-term fp8 DoubleRow: hi@hi, lo(w)@hi, hi(w)@lo
                    hsel = slice(h * 128, (h + 1) * 128)
                    terms = [(0, 0), (1, 0), (0, 1)]
                    for ti, (wi, xi) in enumerate(terms):
                        for g in range(NG):
                            nc.tensor.matmul(
                                ps,
                                lhsT=w_pair[wi][:, 2 * g:2 * g + 2, hsel],
                                rhs=hs_pair[xi][:, 2 * g:2 * g + 2, :],
                                start=(ti == 0 and g == 0),
                                stop=(ti == 2 and g == NG - 1),
                                perf_mode=DR)

                for c in range(8):
                    cs = slice(c * 512, (c + 1) * 512)
                    tl = slice((c % 4) * 512, (c % 4) * 512 + 512)
                    if c == 0:
                        hs_t = hs0
                    elif c == 1:
                        hs_t = hs1
                    else:
                        hs_t = [hsp.tile([128, NF, 512], f8, tag=f"hs{i}",
                                         name=f"hs{i}")
                                for i in range(2)]
                        nc.sync.dma_start(
                            out=hs_t[0],
                            in_=hsh[:, cs].rearrange("(nf p) t -> p nf t",
                                                     p=128))
                        nc.sync.dma_start(
                            out=hs_t[1],
                            in_=hsl[:, cs].rearrange("(nf p) t -> p nf t",
                                                     p=128))
                    # Q (h0, h1) -> V -> K (h0, h1): matches DMA arrivals.
                    # Chunk 0: emit hi@hi g-halves first (first-half DMAs
                    # only), then the lo terms which need wql/hsl.
                    if c == 0:
                        qps = [qkps.tile([128, 512], f32, tag="qk",
                                         name=f"qps{h}") for h in range(HL)]
                        for gh in range(2):
                            for ti, (wi, xi) in enumerate(
                                    [(0, 0), (1, 0), (0, 1)]):
                                for h in range(HL):
                                    for g in range(gh * 4, gh * 4 + 4):
                                        nc.tensor.matmul(
                                            qps[h],
                                            lhsT=wq_sb[wi][
                                                :, 2 * g:2 * g + 2,
                                                h * 128:(h + 1) * 128],
                                            rhs=hs_t[xi][:, 2 * g:2 * g + 2,
                                                         :],
                                            start=(gh == 0 and ti == 0
                                                   and g == 0),
                                            stop=(gh == 1 and ti == 2
                                                  and g == 7),
                                            perf_mode=DR)
                        for h in range(HL):
                            rope_apply(qps[h], qt[h][:, cs], tl, nc.gpsimd)
                    else:
                        for h in range(HL):
                            ps = qkps.tile([128, 512], f32, tag="qk")
                            qk_matmuls(ps, wq_sb, hs_t, h)
                            rope_apply(ps, qt[h][:, cs], tl,
                                       nc.gpsimd if c < 5 else nc.sync,
                                       late=(c >= 6))
                    for s in range(4):
                        vp = vps.tile([128, 256], f32, tag="v")
                        ssel = slice(s * 128, (s + 1) * 128)
                        terms = [(0, 0), (1, 0), (0, 1)]
                        for ti, (xi, wi) in enumerate(terms):
                            for g in range(NG):
                                nc.tensor.matmul(
                                    vp,
                                    lhsT=hs_t[xi][:, 2 * g:2 * g + 2, ssel],
                                    rhs=wv_sb[wi][:, 2 * g:2 * g + 2, :],
                                    start=(ti == 0 and g == 0),
                                    stop=(ti == 2 and g == NG - 1),
                                    perf_mode=DR)
                        # keep Act free near the phase boundary: route the
                        # last chunks' V copies through DVE
                        if c >= 6:
                            nc.vector.tensor_copy(vres[:, c * 4 + s, :], vp)
                        else:
                            nc.scalar.copy(vres[:, c * 4 + s, :], vp)
                    for h in range(HL):
                        ps = qkps.tile([128, 512], f32, tag="qk")
                        qk_matmuls(ps, wk_sb, hs_t, h)
                        rope_apply(ps, kt[h][:, cs], tl,
                                   nc.gpsimd if c < 5 else nc.sync,
                                   late=(c >= 6))

            # ========== Phase 2+3: attention + interleaved o_proj ==========
            with (
                tc.tile_pool(name="sps", bufs=B_SPS, space="PSUM") as sps,
                tc.tile_pool(name="ops", bufs=B_OPS, space="PSUM") as ops,
                tc.tile_pool(name="msc", bufs=B_MSC, space="PSUM") as msc,
                tc.tile_pool(name="smp", bufs=1, space="PSUM") as smp,
                tc.tile_pool(name="pt", bufs=B_PT) as ptp,
                tc.tile_pool(name="pr", bufs=B_PR) as prp,
                tc.tile_pool(name="att_sm", bufs=int(env.get("BK_SM", "4"))) as atsm,
                tc.tile_pool(name="ob", bufs=B_OB) as obp,
            ):
                pending = []
                msc_rot = [msc]  # +sps at the final flush (banks free then)
                ppi = [0]

                def emit_unit():
                    if pending:
                        pending.pop(0)()

                def flush_units():
                    while pending:
                        pending.pop(0)()

                def enqueue_oproj(b, c, gsz=4):
                    tl = slice(c * 512, (c + 1) * 512)
                    state = {}
                    # GPSIMD cannot read PSUM; mix DVE/Act (Act carries exps)
                    copy_engines = [
                        ((lambda o, i: nc.vector.tensor_scalar_mul(o, i, ODESC))
                         if ch == "v" else
                         (lambda o, i: nc.scalar.mul(o, i, ODESC)))
                        for ch in CPY]

                    def unit(fo):
                        def f():
                            g = fo // gsz
                            if g not in state:
                                state[g] = obp.tile([128, gsz, 512], bf16,
                                                    tag=f"ob{gsz}",
                                                    name="ob_t")
                            ob_t = state[g]
                            fs = slice(fo * 128, (fo + 1) * 128)
                            pool = msc_rot[ppi[0] % len(msc_rot)]
                            ppi[0] += 1
                            pp = pool.tile(
                                [128, 512], f32,
                                tag=("pp" if pool is msc else "s"),
                                name="pp")
                            nc.tensor.matmul(
                                pp, lhsT=wo_sb[0][:, :, fs],
                                rhs=aoh[b][:, :, tl],
                                start=True, stop=False, perf_mode=DR)
                            nc.tensor.matmul(
                                pp, lhsT=wo_sb[1][:, :, fs],
                                rhs=aoh[b][:, :, tl],
                                start=False, stop=False, perf_mode=DR)
                            nc.tensor.matmul(
                                pp, lhsT=wo_sb[0][:, :, fs],
                                rhs=aol[b][:, :, tl],
                                start=False, stop=True, perf_mode=DR)
                            copy_engines[fo % len(CPY)](
                                ob_t[:, fo % gsz, :], pp)
                            if fo % gsz == gsz - 1:
                                rows = slice(g * gsz * 128,
                                             (g + 1) * gsz * 128)
                                cg = slice((b * 4 + c) * 512,
                                           (b * 4 + c + 1) * 512)
                                eng = (nc.gpsimd if (OBQ and g % 2 == 1)
                                       else nc.sync)
                                eng.dma_start(
                                    out=outT[rows, cg].rearrange(
                                        "(nf p) t -> p nf t", p=128),
                                    in_=ob_t)
                        return f

                    for fo in range(16):
                        pending.append(unit(fo))

                # Global step stream: score-matmul lookahead crosses chunk
                # boundaries so the next chunk's exps run during the previous
                # chunk's tail (norm chain / oproj flush) with no PE bubble.
                class Chunk:
                    def __init__(self, p, c):
                        self.p, self.c = p, c
                        self.b, self.h = p // 2, p % 2
                        self.jmax = 4 * (c + 1)
                        self.o_ps = None
                        self.sm_ps = None
                        self.sm_started = False
                        self.pts = {}
                        self.prs = []
                        self.sq = []

                def tile_off(ck, j):
                    # Diagonal k-tile v=1..3: first 128v query cols are fully
                    # causal-masked -> compute only cols [128v:512]. Exact.
                    v = j - 4 * ck.c
                    return 128 * v if 1 <= v <= 3 else 0

                def emit_s(ck, j):
                    b, c = ck.b, ck.c
                    off = tile_off(ck, j)
                    sp = sps.tile([128, 512], f32, tag="s", name="sp")
                    nc.tensor.matmul(
                        sp[:, off:],
                        lhsT=kt[ck.h][:, b * T + j * 128:
                                      b * T + (j + 1) * 128],
                        rhs=qt[ck.h][:, b * T + c * 512 + off:
                                     b * T + (c + 1) * 512],
                        start=True, stop=True)
                    ck.sq.append(sp)

                def emit_epv(ck, j):
                    p, c, b, h = ck.p, ck.c, ck.b, ck.h
                    off = tile_off(ck, j)
                    sp = ck.sq[j]
                    pt_t = ptp.tile([128, 512], bf16, tag="p", name="pt_t")
                    nc.scalar.activation(pt_t[:, off:], sp[:, off:],
                                         EXP, scale=SCALE)
                    if j >= 4 * c:
                        nc.vector.tensor_mul(pt_t[:, off:], pt_t[:, off:],
                                             mask_sb[:, j - 4 * c, off:])
                    if c == 3:
                        nc.vector.tensor_copy(
                            abar_raw[p][:, j:j + 1], sp[:, 511:512])
                    if ck.o_ps is None:
                        ck.o_ps = ops.tile([128, 512], f32, tag="o",
                                           name="o_ps")
                    nc.tensor.matmul(
                        ck.o_ps[:, off:],
                        lhsT=vres[:, b * 16 + j, h * 128:(h + 1) * 128],
                        rhs=pt_t[:, off:],
                        start=(j == 0), stop=(j == ck.jmax - 1))
                    if ck.sm_ps is None:
                        ck.sm_ps = smp.tile([2, 512], f32, tag="sm",
                                            name="sm_ps")
                    if j >= 4 * c:
                        # diagonal tile: individual (possibly trimmed) rowsum
                        nc.tensor.matmul(
                            ck.sm_ps[:, off:],
                            lhsT=(ones_b if c == 3 else ones_a),
                            rhs=pt_t[:, off:],
                            start=(j == 4 * c and c == 0),
                            stop=(j == ck.jmax - 1))
                        return
                    ck.pts[j] = pt_t
                    if j % 2 == 1:
                        pr = prp.tile([128, 512], bf16, tag="pr", name="pr")
                        nc.vector.tensor_add(pr, ck.pts[j - 1], ck.pts[j])
                        nc.tensor.matmul(
                            ck.sm_ps,
                            lhsT=ones_a,
                            rhs=pr,
                            start=(j == 1), stop=False)
                        del ck.pts[j - 1], ck.pts[j]

                def epilogue(ck):
                    p, c, b, hl = ck.p, ck.c, ck.b, ck.h
                    cl = slice(c * 512, (c + 1) * 512)
                    rec = atsm.tile([1, 512], f32, tag="rec", name="rec")
                    nc.vector.reciprocal(rec, ck.sm_ps[0:1, :])
                    bc = atsm.tile([128, 512], f32, tag="bc", name="bc")
                    nc.gpsimd.partition_broadcast(bc, rec)
                    full = atsm.tile([128, 512], bf16, tag="full",
                                     name="full")
                    nc.vector.tensor_mul(full, ck.o_ps, bc)
                    # fp8 hi/lo split on Pool (Act does exps, DVE the rest)
                    nc.gpsimd.tensor_copy(aoh[b][:, hl, cl], full)
                    nc.gpsimd.tensor_sub(aol[b][:, hl, cl], full,
                                         aoh[b][:, hl, cl])
                    nc.sync.dma_start(out=sumsd[p, 0:1, cl], in_=rec)
                    if c == 3:
                        tl_sb = atsm.tile([2, 512], f32, tag="smsb",
                                          name="tl_sb")
                        nc.vector.tensor_copy(tl_sb, ck.sm_ps)
                        nc.sync.dma_start(out=sumsd[p, 1:2, cl],
                                          in_=tl_sb[1:2, :])
                    if c == 3:
                        ab_exp = atsm.tile([128, 16], f32, tag="abe",
                                           name="ab_exp")
                        nc.scalar.activation(
                            ab_exp, abar_raw[p], EXP, scale=SCALE)
                        nc.sync.dma_start(out=abard[p], in_=ab_exp)

                chunks = [Chunk(b * 2 + hl, c)
                          for b in range(B) for c in range(4)
                          for hl in range(HL)]
                steps = [(ck, j) for ck in chunks for j in range(ck.jmax)]
                for k in range(LOOK):
                    emit_s(*steps[k])
                for i, (ck, j) in enumerate(steps):
                    if i + LOOK < len(steps):
                        emit_s(*steps[i + LOOK])
                    emit_epv(ck, j)
                    if ILV and j % 2 == 1:
                        for _ in range(ILVN):
                            emit_unit()
                    if j == ck.jmax - 1:
                        epilogue(ck)
                        if ck.h == 1:
                            flush_units()
                            enqueue_oproj(
                                ck.b, ck.c,
                                gsz=(2 if (ck.b, ck.c) == (1, 3) else 4))
                if env.get("BK_TROT", "0") == "1":
                    # scores done: reuse their banks for the tail flush
                    msc_rot.append(sps)
                flush_units()

    nc.compile()
    return nc


def _get_nc():
    if "nc" not in _NC_CACHE:
        _NC_CACHE["nc"] = build_nc()
    return _NC_CACHE["nc"]


def _split8(x):
    hi = x.astype(F8)
    lo = (x - hi.astype(np.float32)).astype(F8)
    return hi, lo


def _host_inputs(hidden_states, q_w, k_w, v_w, o_w):
    """Per-core input dicts."""
    hsT = np.ascontiguousarray(hidden_states.reshape(BT, HID).T)
    hs_hi, hs_lo = _split8(hsT)
    inv = 10000.0 ** (-np.arange(64, dtype=np.float64) / 64.0)
    t = np.arange(T, dtype=np.float64)
    fr = t[None, :] * inv[:, None]  # [64, T]
    # 1/WSCL descale of the x64-scaled Q/K baked into the rope tables
    cosT = (np.cos(np.concatenate([fr, fr], 0)) / WSCL).astype(np.float32)
    sinT = (np.sin(np.concatenate([fr, fr], 0)) / WSCL).astype(np.float32)
    sinT[:64] *= -1.0  # sign-baked for swap-halves rotate
    masks = np.zeros((128, 4, 512), dtype=np.float32)
    kk = np.arange(128)[:, None]
    tt = np.arange(512)[None, :]
    for v in range(4):
        masks[:, v, :] = (tt >= 128 * v + kk).astype(np.float32)
    masks = masks.reshape(128, 2048).astype(BF16)

    in_maps = []
    for core in range(NCORES):
        rs = slice(core * 256, (core + 1) * 256)
        wq_hi, wq_lo = _split8(
            WSCL * np.ascontiguousarray(q_w[rs, :].T))
        wk_hi, wk_lo = _split8(
            WSCL * np.ascontiguousarray(k_w[rs, :].T))
        wv_hi, wv_lo = _split8(
            VSCL * np.ascontiguousarray(v_w[rs, :].T))
        wo_hi, wo_lo = _split8(
            WSCL * np.ascontiguousarray(o_w[:, rs].T))
        in_maps.append({
            "hsh": hs_hi,
            "hsl": hs_lo,
            "wqh": wq_hi, "wql": wq_lo,
            "wkh": wk_hi, "wkl": wk_lo,
            "wvh": wv_hi, "wvl": wv_lo,
            "woh": wo_hi, "wol": wo_lo,
            "cosT": cosT,
            "sinT": sinT,
            "masks": masks,
        })
    return in_maps


def _epilogue(out, results, hidden_states, v_w, o_w):
    """Add the CaM rank-1 correction per (b, h) on host."""
    for core in range(NCORES):
        r = results[core]
        for p in range(4):
            b, hl = p // 2, p % 2
            h = core * HL + hl
            rec = np.asarray(r["sums"][p][0], np.float64)  # 1/rowsum
            rowsum = 1.0 / np.maximum(rec, 1e-30)
            tails = np.zeros(T, np.float64)
            tails[WS:] = np.asarray(r["sums"][p][1][WS:], np.float64)
            a_exp = np.asarray(r["abar"][p], np.float64).T.reshape(2048)
            a_bar = a_exp / max(float(rowsum[T - 1]), 1e-30)
            avg_w = max(float(np.mean(a_bar[WS:])), 1e-6)
            prob = float(np.clip(a_bar[EVICT] / avg_w, 0.0, 1.0))
            prob = float(np.nan_to_num(prob, nan=0.0, posinf=1.0, neginf=0.0))
            m = 1.0 if U_CONST[b, h] < prob else 0.0
            if m == 0.0:
                continue
            # exact v_e from fp32 inputs
            v_row = hidden_states[b, EVICT, :] @ v_w[h * D:(h + 1) * D, :].T
            v_e = v_row * (m / RB)  # [D]
            w_e = o_w[:, h * D:(h + 1) * D] @ v_e  # [HID]
            s_tail = (tails / np.maximum(rowsum, 1e-30)).astype(np.float32)
            out[b] += np.outer(s_tail, w_e).astype(np.float32)
    return out


def kernel(hidden_states, attention_mask, q_w, k_w, v_w, o_w):
    from concourse.bass_utils import run_bass_kernel_spmd

    nc = _get_nc()
    in_maps = _host_inputs(hidden_states, q_w, k_w, v_w, o_w)
    trace = bool(int(os.environ.get("BK_TRACE", "0")))
    res = run_bass_kernel_spmd(
        nc, in_maps, core_ids=list(range(NCORES)), trace=trace,
    )
    if trace and res.exec_time_ns is not None:
        print(f"HW exec time: {res.exec_time_ns} ns")
        _NC_CACHE["last_exec_ns"] = res.exec_time_ns
        _NC_CACHE["last_trace"] = res.instructions_and_trace
    results = res.results

    acc = np.zeros((HID, BT), dtype=np.float32)
    for core in range(NCORES):
        acc += np.asarray(results[core]["outT"], np.float32)
    out = np.ascontiguousarray(acc.T).reshape(B, T, HID)
    out = _epilogue(out, results, hidden_states, v_w, o_w)
    return out.astype(np.float32)


# revision 22
# speedup vs baseline: 1.1154x; 1.0517x over previous
"""Trainium2 Bass kernel for nn_LlamaAttention_cam (sparse_attention).

Sharding: 8 cores = 2 batches x 4 head-groups. Core k handles batch k//4
and heads 4*(k%4)..4*(k%4)+3, so each core streams only its batch's
hidden_states (half the DMA of batch-replicated sharding). Q/K/V
projections column-parallel over heads; o_proj row-parallel within each
batch group (4-core partial sums added on host). The CaM merge is a
rank-1 correction (s_tail outer v_e) applied on host from tiny
device-side statistics.

The projection GEMMs (QKV + o_proj) run as fp8e4 DoubleRow matmuls with
3-term error compensation: X*W ~ Xh*Wh + Xl*Wh + Xh*Wl where Xh = fp8(X),
Xl = fp8(X - Xh). DoubleRow packs a 256-deep contraction at 0.5 cyc/col,
so 3 terms cost 75% of the bf16 equivalent. hs and all weights are split
on the host (free); attn_out is split on-device. Weights are pre-scaled
into fp8 range (x64, V x16); the o_proj epilogue copy descales by 1/1024.

Self-contained: hardcodes all shapes; takes full inputs, returns full output.
"""

import math
import os

import numpy as np
import ml_dtypes

B, T, HID, H = 2, 2048, 2048, 16
D = 128
NCORES = 8
HL = 4  # heads per core
HG = H // HL  # head groups = 4
NF = HID // 128  # 16 f-tiles
NG = NF // 2  # 8 f-tile pairs for DoubleRow
SCALE = 1.0 / math.sqrt(D)
RB = int(0.25 * T)  # 512 recent budget
WS = T - RB  # 1536
EVICT = WS - 1  # 1535
WSCL = 64.0  # fp8 pre-scale on wq/wk/wo
VSCL = 16.0  # fp8 pre-scale on wv: max |attn_out*VSCL| ~ 5sigma*16 = 72 < 240
ODESC = 1.0 / (VSCL * WSCL)  # o_proj descale: V carries x16, wo carries x64

# jax.random.uniform(jax.random.key(42), (2,16), float32); bernoulli(key,p) == u < p
U_CONST = np.array(
    [[0.59400654, 0.43801308, 0.6285691, 0.00791204, 0.27834702,
      0.7976179, 0.8521497, 0.9625306, 0.67656493, 0.11104441,
      0.4959929, 0.7311437, 0.18970704, 0.1544199, 0.03802836,
      0.33559263],
     [0.92825687, 0.6123972, 0.49262476, 0.733806, 0.18920851,
      0.15386605, 0.037136197, 0.32930005, 0.9372028, 0.5957513,
      0.4615929, 0.6695677, 0.07019377, 0.39408123, 0.55786455,
      0.35412872]], dtype=np.float32)

BF16 = ml_dtypes.bfloat16
F8 = ml_dtypes.float8_e4m3

_NC_CACHE = {}


def build_nc():
    import concourse.bacc as bacc
    import concourse.mybir as mybir
    import concourse.tile as tile

    f32 = mybir.dt.float32
    bf16 = mybir.dt.bfloat16
    f8 = mybir.dt.float8e4
    EXP = mybir.ActivationFunctionType.Exp
    DR = mybir.MatmulPerfMode.DoubleRow

    nc = bacc.Bacc("TRN2", target_bir_lowering=False, debug=False)
    env = os.environ
    B_QK = int(env.get("BK_QK", "5"))
    B_HSP = int(env.get("BK_HSP", "2"))
    B_ROPE = int(env.get("BK_ROPE", "4"))
    B_SPS = int(env.get("BK_SPS", "3"))
    B_OPS = int(env.get("BK_OPS", "2"))
    B_MSC = int(env.get("BK_MSC", "2"))
    B_PT = int(env.get("BK_PT", "10"))
    B_PR = int(env.get("BK_PR", "3"))
    B_OB = int(env.get("BK_OB", "8"))
    LOOK = int(env.get("BK_LOOK", "4"))
    ILV = env.get("BK_ILV", "1") == "1"
    CPY = env.get("BK_CPY", "vsvvs")  # per-fo copy engine cycle: v=DVE s=Act
    OBQ = env.get("BK_OBQ", "0") == "1"  # alternate ob DMA queues
    ILVN = int(env.get("BK_ILVN", "1"))  # oproj units per 2 j-steps
    TROT = env.get("BK_TROT", "1") == "1"

    hsh = nc.dram_tensor("hsh", [HID, T], f8, kind="ExternalInput")
    hsl = nc.dram_tensor("hsl", [HID, T], f8, kind="ExternalInput")
    wqh = nc.dram_tensor("wqh", [HID, 512], f8, kind="ExternalInput")
    wql = nc.dram_tensor("wql", [HID, 512], f8, kind="ExternalInput")
    wkh = nc.dram_tensor("wkh", [HID, 512], f8, kind="ExternalInput")
    wkl = nc.dram_tensor("wkl", [HID, 512], f8, kind="ExternalInput")
    wvh = nc.dram_tensor("wvh", [HID, 512], f8, kind="ExternalInput")
    wvl = nc.dram_tensor("wvl", [HID, 512], f8, kind="ExternalInput")
    woh = nc.dram_tensor("woh", [512, HID], f8, kind="ExternalInput")
    wol = nc.dram_tensor("wol", [512, HID], f8, kind="ExternalInput")
    cosd = nc.dram_tensor("cosT", [128, T], f32, kind="ExternalInput")
    sind = nc.dram_tensor("sinT", [128, T], f32, kind="ExternalInput")
    maskd = nc.dram_tensor("masks", [128, 2048], bf16, kind="ExternalInput")

    outT = nc.dram_tensor("outT", [HID, T], bf16, kind="ExternalOutput")
    abard = nc.dram_tensor("abar", [4, 128, 16], f32, kind="ExternalOutput")
    sumsd = nc.dram_tensor("sums", [4, 2, T], f32, kind="ExternalOutput")

    with tile.TileContext(nc) as tc:
        with (
            tc.tile_pool(name="singles", bufs=1) as singles,
            tc.tile_pool(name="res", bufs=1) as res,
            tc.tile_pool(name="stats", bufs=1) as stats,
        ):
            # --- small constants (no DMA deps) for the PE warm-up ---
            ones_a = singles.tile([128, 2], bf16, tag="onesa")  # [1, 0]
            ones_b = singles.tile([128, 2], bf16, tag="onesb")  # [1, 1]
            nc.vector.memset(ones_a[:, 0:1], 1.0)
            nc.vector.memset(ones_a[:, 1:2], 0.0)
            nc.vector.memset(ones_b, 1.0)
            wsrc2 = singles.tile([128, 512], bf16, tag="wsrc2")
            nc.vector.memset(wsrc2, 0.0)

            cos_sb = singles.tile([128, T], f32, tag="cos")
            sin_sb = singles.tile([128, T], f32, tag="sin")
            mask_sb = singles.tile([128, 4, 512], bf16, tag="mask")

            # --- residents ---
            qt = [res.tile([128, T], bf16, tag=f"qt{h}", name=f"qt{h}")
                  for h in range(HL)]
            kt = [res.tile([128, T], bf16, tag=f"kt{h}", name=f"kt{h}")
                  for h in range(HL)]
            vres = res.tile([128, 16, 512], bf16, tag="vres")
            # attn_out hi/lo fp8, [d, head, t]
            aoh = res.tile([128, HL, T], f8, tag="aoh")
            aol = res.tile([128, HL, T], f8, tag="aol")
            abar_raw = [stats.tile([128, 16], f32, tag=f"ab{p}", name=f"ab{p}")
                        for p in range(HL)]

            # ================= Phase 1: QKV projections + RoPE ================
            with (
                tc.tile_pool(name="wqkv", bufs=1) as wpool,
                tc.tile_pool(name="hsp", bufs=B_HSP) as hsp,
                tc.tile_pool(name="rope", bufs=B_ROPE) as rope,
                tc.tile_pool(name="qkps", bufs=B_QK, space="PSUM") as qkps,
                tc.tile_pool(name="vps", bufs=2, space="PSUM") as vps,
                tc.tile_pool(name="wps", bufs=1, space="PSUM") as wps,
            ):
                wq_sb = [wpool.tile([128, NF, 512], f8, tag=f"wq{i}",
                                    name=f"wq{i}") for i in range(2)]
                wk_sb = [wpool.tile([128, NF, 512], f8, tag=f"wk{i}",
                                    name=f"wk{i}") for i in range(2)]
                wv_sb = [wpool.tile([128, NF, 512], f8, tag=f"wv{i}",
                                    name=f"wv{i}") for i in range(2)]

                # PE warm-up: dependency-free matmuls at t~0 start the
                # p-state ramp while the first DMAs land.
                warm = wps.tile([2, 512], f32, tag="warm")

                def warm_fill(k):
                    for _ in range(k):
                        nc.tensor.matmul(warm, lhsT=ones_a, rhs=wsrc2,
                                         start=True, stop=True)

                warm_fill(int(env.get("BK_NWARM", "4")))

                # DMA issue order is the sync-queue service order; front-load
                # exactly what the first matmuls need (wq_hi + hs_hi chunk 0).
                hs0 = [hsp.tile([128, NF, 512], f8, tag=f"hs{i}",
                                name=f"hs0{i}") for i in range(2)]
                for fh in range(2):
                    for i, (wsrc_d, hsrc_d) in enumerate([(wqh, hsh),
                                                          (wql, hsl)]):
                        rsl = slice(fh * 1024, (fh + 1) * 1024)
                        fsl = slice(fh * 8, (fh + 1) * 8)
                        nc.sync.dma_start(
                            out=wq_sb[i][:, fsl, :],
                            in_=wsrc_d[rsl, :].rearrange(
                                "(nf p) d -> p nf d", p=128))
                        nc.sync.dma_start(
                            out=hs0[i][:, fsl, :],
                            in_=hsrc_d[rsl, 0:512].rearrange(
                                "(nf p) t -> p nf t", p=128))
                nc.sync.dma_start(
                    out=wv_sb[0],
                    in_=wvh.rearrange("(nf p) d -> p nf d", p=128))
                nc.sync.dma_start(
                    out=wv_sb[1],
                    in_=wvl.rearrange("(nf p) d -> p nf d", p=128))
                nc.sync.dma_start(
                    out=wk_sb[0],
                    in_=wkh.rearrange("(nf p) d -> p nf d", p=128))
                nc.sync.dma_start(
                    out=wk_sb[1],
                    in_=wkl.rearrange("(nf p) d -> p nf d", p=128))
                hs1 = [hsp.tile([128, NF, 512], f8, tag=f"hs{i}",
                                name=f"hs1{i}") for i in range(2)]
                nc.sync.dma_start(
                    out=hs1[0],
                    in_=hsh[:, 512:1024].rearrange("(nf p) t -> p nf t",
                                                   p=128))
                nc.sync.dma_start(
                    out=hs1[1],
                    in_=hsl[:, 512:1024].rearrange("(nf p) t -> p nf t",
                                                   p=128))
                nc.gpsimd.dma_start(out=cos_sb, in_=cosd[:, :])
                nc.gpsimd.dma_start(out=sin_sb, in_=sind[:, :])
                nc.gpsimd.dma_start(
                    out=mask_sb, in_=maskd.rearrange("p (v t) -> p v t", v=4))

                def rope_apply(ps, dest, tl):
                    qf = rope.tile([128, 512], f32, tag="qf")
                    nc.scalar.copy(qf, ps)
                    rot = rope.tile([128, 512], f32, tag="rot")
                    nc.gpsimd.dma_start(out=rot[0:64, :], in_=qf[64:128, :])
                    nc.gpsimd.dma_start(out=rot[64:128, :], in_=qf[0:64, :])
                    t1 = rope.tile([128, 512], f32, tag="t1")
                    nc.vector.tensor_mul(t1, rot, sin_sb[:, tl])
                    t2 = rope.tile([128, 512], f32, tag="t2")
                    nc.vector.tensor_mul(t2, qf, cos_sb[:, tl])
                    nc.vector.tensor_add(dest, t1, t2)

                def qk_matmuls(ps, w_pair, hs_pair, h):
                    # 3-term fp8 DoubleRow: hi@hi, lo(w)@hi, hi(w)@lo
                    hsel = slice(h * 128, (h + 1) * 128)
                    terms = [(0, 0), (1, 0), (0, 1)]
                    for ti, (wi, xi) in enumerate(terms):
                        for g in range(NG):
                            nc.tensor.matmul(
                                ps,
                                lhsT=w_pair[wi][:, 2 * g:2 * g + 2, hsel],
                                rhs=hs_pair[xi][:, 2 * g:2 * g + 2, :],
                                start=(ti == 0 and g == 0),
                                stop=(ti == 2 and g == NG - 1),
                                perf_mode=DR)

                for c in range(4):
                    cs = slice(c * 512, (c + 1) * 512)
                    if c == 0:
                        hs_t = hs0
                    elif c == 1:
                        hs_t = hs1
                    else:
                        hs_t = [hsp.tile([128, NF, 512], f8, tag=f"hs{i}",
                                         name=f"hs{i}")
                                for i in range(2)]
                        nc.sync.dma_start(
                            out=hs_t[0],
                            in_=hsh[:, cs].rearrange("(nf p) t -> p nf t",
                                                     p=128))
                        nc.sync.dma_start(
                            out=hs_t[1],
                            in_=hsl[:, cs].rearrange("(nf p) t -> p nf t",
                                                     p=128))
                    # Q (h0-h3) -> V -> K (h0-h3): matches DMA arrivals.
                    # Chunk 0: emit hi@hi g-halves first (first-half DMAs
                    # only), then the lo terms which need wql/hsl.
                    if c == 0:
                        qps = [qkps.tile([128, 512], f32, tag="qk",
                                         name=f"qps{h}") for h in range(HL)]
                        for gh in range(2):
                            for ti, (wi, xi) in enumerate(
                                    [(0, 0), (1, 0), (0, 1)]):
                                for h in range(HL):
                                    for g in range(gh * 4, gh * 4 + 4):
                                        nc.tensor.matmul(
                                            qps[h],
                                            lhsT=wq_sb[wi][
                                                :, 2 * g:2 * g + 2,
                                                h * 128:(h + 1) * 128],
                                            rhs=hs_t[xi][:, 2 * g:2 * g + 2,
                                                         :],
                                            start=(gh == 0 and ti == 0
                                                   and g == 0),
                                            stop=(gh == 1 and ti == 2
                                                  and g == 7),
                                            perf_mode=DR)
                        warm_fill(int(env.get("BK_NW2", "2")))
                        for h in range(HL):
                            rope_apply(qps[h], qt[h][:, cs], cs)
                    else:
                        for h in range(HL):
                            ps = qkps.tile([128, 512], f32, tag="qk")
                            qk_matmuls(ps, wq_sb, hs_t, h)
                            rope_apply(ps, qt[h][:, cs], cs)
                    for s in range(4):
                        vp = vps.tile([128, 512], f32, tag="v")
                        ssel = slice(s * 128, (s + 1) * 128)
                        terms = [(0, 0), (1, 0), (0, 1)]
                        for ti, (xi, wi) in enumerate(terms):
                            for g in range(NG):
                                nc.tensor.matmul(
                                    vp,
                                    lhsT=hs_t[xi][:, 2 * g:2 * g + 2, ssel],
                                    rhs=wv_sb[wi][:, 2 * g:2 * g + 2, :],
                                    start=(ti == 0 and g == 0),
                                    stop=(ti == 2 and g == NG - 1),
                                    perf_mode=DR)
                        nc.scalar.copy(vres[:, c * 4 + s, :], vp)
                        if c <= 1:
                            warm_fill(int(env.get("BK_NW3", "1")))
                    for h in range(HL):
                        ps = qkps.tile([128, 512], f32, tag="qk")
                        qk_matmuls(ps, wk_sb, hs_t, h)
                        rope_apply(ps, kt[h][:, cs], cs)

            # ========== Phase 2+3: attention + interleaved o_proj ==========
            with (
                tc.tile_pool(name="wop", bufs=1) as wop,
                tc.tile_pool(name="sps", bufs=B_SPS, space="PSUM") as sps,
                tc.tile_pool(name="ops", bufs=B_OPS, space="PSUM") as ops,
                tc.tile_pool(name="msc", bufs=B_MSC, space="PSUM") as msc,
                tc.tile_pool(name="smp", bufs=1, space="PSUM") as smp,
                tc.tile_pool(name="pt", bufs=B_PT) as ptp,
                tc.tile_pool(name="pr", bufs=B_PR) as prp,
                tc.tile_pool(name="att_sm", bufs=int(env.get("BK_SM", "4"))) as atsm,
                tc.tile_pool(name="ob", bufs=B_OB) as obp,
            ):
                wo_sb = [wop.tile([128, HL, HID], f8, tag=f"wo{i}",
                                  name=f"wo{i}") for i in range(2)]
                nc.sync.dma_start(
                    out=wo_sb[0], in_=woh.rearrange("(kt p) f -> p kt f",
                                                    p=128))
                nc.sync.dma_start(
                    out=wo_sb[1], in_=wol.rearrange("(kt p) f -> p kt f",
                                                    p=128))

                pending = []
                msc_rot = [msc]  # +sps at the final flush (banks free then)
                ppi = [0]

                def emit_unit():
                    if pending:
                        pending.pop(0)()

                def flush_units():
                    while pending:
                        pending.pop(0)()

                def enqueue_oproj(c, gsz=4):
                    tl = slice(c * 512, (c + 1) * 512)
                    state = {}
                    # GPSIMD cannot read PSUM; mix DVE/Act (Act carries exps)
                    copy_engines = [
                        ((lambda o, i: nc.vector.tensor_scalar_mul(o, i,
                                                                   ODESC))
                         if ch == "v" else
                         (lambda o, i: nc.scalar.mul(o, i, ODESC)))
                        for ch in CPY]

                    def unit(fo):
                        def f():
                            g = fo // gsz
                            if g not in state:
                                state[g] = obp.tile([128, gsz, 512], bf16,
                                                    tag=f"ob{gsz}",
                                                    name="ob_t")
                            ob_t = state[g]
                            fs = slice(fo * 128, (fo + 1) * 128)
                            pool = msc_rot[ppi[0] % len(msc_rot)]
                            ppi[0] += 1
                            pp = pool.tile(
                                [128, 512], f32,
                                tag=("pp" if pool is msc else "s"),
                                name="pp")
                            # 3 terms x 2 head k-pairs, aol term last
                            first = True
                            for wi, src, last in ((0, aoh, False),
                                                  (1, aoh, False),
                                                  (0, aol, True)):
                                for kp in range(2):
                                    ksl = slice(2 * kp, 2 * kp + 2)
                                    nc.tensor.matmul(
                                        pp,
                                        lhsT=wo_sb[wi][:, ksl, fs],
                                        rhs=src[:, ksl, tl],
                                        start=first,
                                        stop=(last and kp == 1),
                                        perf_mode=DR)
                                    first = False
                            copy_engines[fo % len(CPY)](
                                ob_t[:, fo % gsz, :], pp)
                            if fo % gsz == gsz - 1:
                                rows = slice(g * gsz * 128,
                                             (g + 1) * gsz * 128)
                                cg = slice(c * 512, (c + 1) * 512)
                                eng = (nc.gpsimd if (OBQ and g % 2 == 1)
                                       else nc.sync)
                                eng.dma_start(
                                    out=outT[rows, cg].rearrange(
                                        "(nf p) t -> p nf t", p=128),
                                    in_=ob_t)
                        return f

                    for fo in range(16):
                        pending.append(unit(fo))

                # Global step stream: score-matmul lookahead crosses chunk
                # boundaries so the next chunk's exps run during the previous
                # chunk's tail (norm chain / oproj flush) with no PE bubble.
                class Chunk:
                    def __init__(self, p, c):
                        self.p, self.c = p, c  # p = local head
                        self.jmax = 4 * (c + 1)
                        self.o_ps = None
                        self.sm_ps = None
                        self.pts = {}
                        self.sq = []

                def tile_off(ck, j):
                    # Diagonal k-tile v=1..3: first 128v query cols are fully
                    # causal-masked -> compute only cols [128v:512]. Exact.
                    v = j - 4 * ck.c
                    return 128 * v if 1 <= v <= 3 else 0

                def emit_s(ck, j):
                    c = ck.c
                    off = tile_off(ck, j)
                    sp = sps.tile([128, 512], f32, tag="s", name="sp")
                    nc.tensor.matmul(
                        sp[:, off:],
                        lhsT=kt[ck.p][:, j * 128:(j + 1) * 128],
                        rhs=qt[ck.p][:, c * 512 + off:(c + 1) * 512],
                        start=True, stop=True)
                    ck.sq.append(sp)

                def emit_epv(ck, j):
                    p, c = ck.p, ck.c
                    off = tile_off(ck, j)
                    sp = ck.sq[j]
                    pt_t = ptp.tile([128, 512], bf16, tag="p", name="pt_t")
                    nc.scalar.activation(pt_t[:, off:], sp[:, off:],
                                         EXP, scale=SCALE)
                    if j >= 4 * c:
                        nc.vector.tensor_mul(pt_t[:, off:], pt_t[:, off:],
                                             mask_sb[:, j - 4 * c, off:])
                    if c == 3:
                        nc.vector.tensor_copy(
                            abar_raw[p][:, j:j + 1], sp[:, 511:512])
                    if ck.o_ps is None:
                        ck.o_ps = ops.tile([128, 512], f32, tag="o",
                                           name="o_ps")
                    nc.tensor.matmul(
                        ck.o_ps[:, off:],
                        lhsT=vres[:, j, p * 128:(p + 1) * 128],
                        rhs=pt_t[:, off:],
                        start=(j == 0), stop=(j == ck.jmax - 1))
                    if ck.sm_ps is None:
                        ck.sm_ps = smp.tile([2, 512], f32, tag="sm",
                                            name="sm_ps")
                    if j >= 4 * c:
                        # diagonal tile: individual (possibly trimmed) rowsum
                        nc.tensor.matmul(
                            ck.sm_ps[:, off:],
                            lhsT=(ones_b if c == 3 else ones_a),
                            rhs=pt_t[:, off:],
                            start=(j == 4 * c and c == 0),
                            stop=(j == ck.jmax - 1))
                        return
                    ck.pts[j] = pt_t
                    if j % 2 == 1:
                        pr = prp.tile([128, 512], bf16, tag="pr", name="pr")
                        nc.vector.tensor_add(pr, ck.pts[j - 1], ck.pts[j])
                        nc.tensor.matmul(
                            ck.sm_ps,
                            lhsT=ones_a,
                            rhs=pr,
                            start=(j == 1), stop=False)
                        del ck.pts[j - 1], ck.pts[j]

                def epilogue(ck):
                    p, c = ck.p, ck.c
                    cl = slice(c * 512, (c + 1) * 512)
                    rec = atsm.tile([1, 512], f32, tag="rec", name="rec")
                    nc.vector.reciprocal(rec, ck.sm_ps[0:1, :])
                    bc = atsm.tile([128, 512], f32, tag="bc", name="bc")
                    nc.gpsimd.partition_broadcast(bc, rec)
                    full = atsm.tile([128, 512], bf16, tag="full",
                                     name="full")
                    nc.vector.tensor_mul(full, ck.o_ps, bc)
                    # fp8 hi/lo split on Pool (Act does exps, DVE the rest)
                    nc.gpsimd.tensor_copy(aoh[:, p, cl], full)
                    nc.gpsimd.tensor_sub(aol[:, p, cl], full,
                                         aoh[:, p, cl])
                    nc.sync.dma_start(out=sumsd[p, 0:1, cl], in_=rec)
                    if c == 3:
                        tl_sb = atsm.tile([2, 512], f32, tag="smsb",
                                          name="tl_sb")
                        nc.vector.tensor_copy(tl_sb, ck.sm_ps)
                        nc.sync.dma_start(out=sumsd[p, 1:2, cl],
                                          in_=tl_sb[1:2, :])
                        ab_exp = atsm.tile([128, 16], f32, tag="abe",
                                           name="ab_exp")
                        nc.scalar.activation(
                            ab_exp, abar_raw[p], EXP, scale=SCALE)
                        nc.sync.dma_start(out=abard[p], in_=ab_exp)

                chunks = [Chunk(hl, c)
                          for c in range(4) for hl in range(HL)]
                steps = [(ck, j) for ck in chunks for j in range(ck.jmax)]
                for k in range(LOOK):
                    emit_s(*steps[k])
                for i, (ck, j) in enumerate(steps):
                    if i + LOOK < len(steps):
                        emit_s(*steps[i + LOOK])
                    emit_epv(ck, j)
                    if ILV and j % 2 == 1:
                        for _ in range(ILVN):
                            emit_unit()
                    if j == ck.jmax - 1:
                        epilogue(ck)
                        if ck.p == HL - 1:
                            flush_units()
                            enqueue_oproj(ck.c,
                                          gsz=(2 if ck.c == 3 else 4))
                if TROT:
                    # scores done: reuse their banks for the tail flush
                    msc_rot.append(sps)
                flush_units()

    nc.compile()
    return nc


def _get_nc():
    if "nc" not in _NC_CACHE:
        _NC_CACHE["nc"] = build_nc()
    return _NC_CACHE["nc"]


def _split8(x):
    hi = x.astype(F8)
    lo = (x - hi.astype(np.float32)).astype(F8)
    return hi, lo


def _host_inputs(hidden_states, q_w, k_w, v_w, o_w):
    """Per-core input dicts. Core k: batch k//4, heads 4*(k%4)..4*(k%4)+3."""
    inv = 10000.0 ** (-np.arange(64, dtype=np.float64) / 64.0)
    t = np.arange(T, dtype=np.float64)
    fr = t[None, :] * inv[:, None]  # [64, T]
    # 1/WSCL descale of the x64-scaled Q/K baked into the rope tables
    cosT = (np.cos(np.concatenate([fr, fr], 0)) / WSCL).astype(np.float32)
    sinT = (np.sin(np.concatenate([fr, fr], 0)) / WSCL).astype(np.float32)
    sinT[:64] *= -1.0  # sign-baked for swap-halves rotate
    masks = np.zeros((128, 4, 512), dtype=np.float32)
    kk = np.arange(128)[:, None]
    tt = np.arange(512)[None, :]
    for v in range(4):
        masks[:, v, :] = (tt >= 128 * v + kk).astype(np.float32)
    masks = masks.reshape(128, 2048).astype(BF16)

    hs_b = []
    for b in range(B):
        hsT = np.ascontiguousarray(hidden_states[b].T)  # [HID, T]
        hs_b.append(_split8(hsT))
    w_g = []
    for g in range(HG):
        rs = slice(g * 512, (g + 1) * 512)
        w_g.append((
            _split8(WSCL * np.ascontiguousarray(q_w[rs, :].T)),
            _split8(WSCL * np.ascontiguousarray(k_w[rs, :].T)),
            _split8(VSCL * np.ascontiguousarray(v_w[rs, :].T)),
            _split8(WSCL * np.ascontiguousarray(o_w[:, rs].T)),
        ))

    in_maps = []
    for core in range(NCORES):
        b, g = core // HG, core % HG
        (wq_hi, wq_lo), (wk_hi, wk_lo), (wv_hi, wv_lo), (wo_hi, wo_lo) = \
            w_g[g]
        in_maps.append({
            "hsh": hs_b[b][0],
            "hsl": hs_b[b][1],
            "wqh": wq_hi, "wql": wq_lo,
            "wkh": wk_hi, "wkl": wk_lo,
            "wvh": wv_hi, "wvl": wv_lo,
            "woh": wo_hi, "wol": wo_lo,
            "cosT": cosT,
            "sinT": sinT,
            "masks": masks,
        })
    return in_maps


def _epilogue(out, results, hidden_states, v_w, o_w):
    """Add the CaM rank-1 correction per (b, h) on host."""
    for core in range(NCORES):
        r = results[core]
        b = core // HG
        for p in range(HL):
            h = (core % HG) * HL + p
            rec = np.asarray(r["sums"][p][0], np.float64)  # 1/rowsum
            rowsum = 1.0 / np.maximum(rec, 1e-30)
            tails = np.zeros(T, np.float64)
            tails[WS:] = np.asarray(r["sums"][p][1][WS:], np.float64)
            a_exp = np.asarray(r["abar"][p], np.float64).T.reshape(2048)
            a_bar = a_exp / max(float(rowsum[T - 1]), 1e-30)
            avg_w = max(float(np.mean(a_bar[WS:])), 1e-6)
            prob = float(np.clip(a_bar[EVICT] / avg_w, 0.0, 1.0))
            prob = float(np.nan_to_num(prob, nan=0.0, posinf=1.0, neginf=0.0))
            m = 1.0 if U_CONST[b, h] < prob else 0.0
            if m == 0.0:
                continue
            # exact v_e from fp32 inputs
            v_row = hidden_states[b, EVICT, :] @ v_w[h * D:(h + 1) * D, :].T
            v_e = v_row * (m / RB)  # [D]
            w_e = o_w[:, h * D:(h + 1) * D] @ v_e  # [HID]
            s_tail = (tails / np.maximum(rowsum, 1e-30)).astype(np.float32)
            out[b] += np.outer(s_tail, w_e).astype(np.float32)
    return out


def kernel(hidden_states, attention_mask, q_w, k_w, v_w, o_w):
    from concourse.bass_utils import run_bass_kernel_spmd

    nc = _get_nc()
    in_maps = _host_inputs(hidden_states, q_w, k_w, v_w, o_w)
    trace = bool(int(os.environ.get("BK_TRACE", "0")))
    res = run_bass_kernel_spmd(
        nc, in_maps, core_ids=list(range(NCORES)), trace=trace,
    )
    if trace and res.exec_time_ns is not None:
        print(f"HW exec time: {res.exec_time_ns} ns")
        _NC_CACHE["last_exec_ns"] = res.exec_time_ns
        _NC_CACHE["last_trace"] = res.instructions_and_trace
    results = res.results

    out = np.zeros((B, T, HID), dtype=np.float32)
    for core in range(NCORES):
        b = core // HG
        out[b] += np.asarray(results[core]["outT"], np.float32).T
    out = _epilogue(out, results, hidden_states, v_w, o_w)
    return out.astype(np.float32)


# revision 23
# speedup vs baseline: 1.1433x; 1.0250x over previous
"""Trainium2 Bass kernel for nn_LlamaAttention_cam (sparse_attention).

Sharding: 8 cores = 2 batches x 4 head-groups. Core k handles batch k//4
and heads 4*(k%4)..4*(k%4)+3, so each core streams only its batch's
hidden_states (half the DMA of batch-replicated sharding). Q/K/V
projections column-parallel over heads; o_proj row-parallel within each
batch group (4-core partial sums added on host). The CaM merge is a
rank-1 correction (s_tail outer v_e) applied on host from tiny
device-side statistics.

The projection GEMMs (QKV + o_proj) run as fp8e4 DoubleRow matmuls with
3-term error compensation: X*W ~ Xh*Wh + Xl*Wh + Xh*Wl where Xh = fp8(X),
Xl = fp8(X - Xh). DoubleRow packs a 256-deep contraction at 0.5 cyc/col,
so 3 terms cost 75% of the bf16 equivalent. hs and all weights are split
on the host (free); attn_out is split on-device. Weights are pre-scaled
into fp8 range (x64, V x16); the o_proj epilogue copy descales by 1/1024.

Self-contained: hardcodes all shapes; takes full inputs, returns full output.
"""

import math
import os

import numpy as np
import ml_dtypes

B, T, HID, H = 2, 2048, 2048, 16
D = 128
NCORES = 8
HL = 4  # heads per core
HG = H // HL  # head groups = 4
NF = HID // 128  # 16 f-tiles
NG = NF // 2  # 8 f-tile pairs for DoubleRow
SCALE = 1.0 / math.sqrt(D)
RB = int(0.25 * T)  # 512 recent budget
WS = T - RB  # 1536
EVICT = WS - 1  # 1535
WSCL = 64.0  # fp8 pre-scale on wq/wk/wo
VSCL = 16.0  # fp8 pre-scale on wv: max |attn_out*VSCL| ~ 5sigma*16 = 72 < 240
ODESC = 1.0 / (VSCL * WSCL)  # o_proj descale: V carries x16, wo carries x64

# jax.random.uniform(jax.random.key(42), (2,16), float32); bernoulli(key,p) == u < p
U_CONST = np.array(
    [[0.59400654, 0.43801308, 0.6285691, 0.00791204, 0.27834702,
      0.7976179, 0.8521497, 0.9625306, 0.67656493, 0.11104441,
      0.4959929, 0.7311437, 0.18970704, 0.1544199, 0.03802836,
      0.33559263],
     [0.92825687, 0.6123972, 0.49262476, 0.733806, 0.18920851,
      0.15386605, 0.037136197, 0.32930005, 0.9372028, 0.5957513,
      0.4615929, 0.6695677, 0.07019377, 0.39408123, 0.55786455,
      0.35412872]], dtype=np.float32)

BF16 = ml_dtypes.bfloat16
F8 = ml_dtypes.float8_e4m3

_NC_CACHE = {}


def build_nc():
    import concourse.bacc as bacc
    import concourse.mybir as mybir
    import concourse.tile as tile

    f32 = mybir.dt.float32
    bf16 = mybir.dt.bfloat16
    f8 = mybir.dt.float8e4
    EXP = mybir.ActivationFunctionType.Exp
    DR = mybir.MatmulPerfMode.DoubleRow

    nc = bacc.Bacc("TRN2", target_bir_lowering=False, debug=False)
    env = os.environ
    B_QK = int(env.get("BK_QK", "5"))
    B_HSP = int(env.get("BK_HSP", "2"))
    B_ROPE = int(env.get("BK_ROPE", "4"))
    B_SPS = int(env.get("BK_SPS", "3"))
    B_OPS = int(env.get("BK_OPS", "2"))
    B_MSC = int(env.get("BK_MSC", "2"))
    B_PT = int(env.get("BK_PT", "34"))
    B_PR = int(env.get("BK_PR", "3"))
    B_OB = int(env.get("BK_OB", "6"))
    LOOK = int(env.get("BK_LOOK", "28"))
    ILV = env.get("BK_ILV", "1") == "1"
    CPY = env.get("BK_CPY", "vsvvs")  # per-fo copy engine cycle: v=DVE s=Act
    OBQ = env.get("BK_OBQ", "0") == "1"  # alternate ob DMA queues
    ILVN = int(env.get("BK_ILVN", "1"))  # oproj units per 2 j-steps
    TROT = env.get("BK_TROT", "1") == "1"

    hsh = nc.dram_tensor("hsh", [HID, T], f8, kind="ExternalInput")
    hsl = nc.dram_tensor("hsl", [HID, T], f8, kind="ExternalInput")
    wqh = nc.dram_tensor("wqh", [HID, 512], f8, kind="ExternalInput")
    wql = nc.dram_tensor("wql", [HID, 512], f8, kind="ExternalInput")
    wkh = nc.dram_tensor("wkh", [HID, 512], f8, kind="ExternalInput")
    wkl = nc.dram_tensor("wkl", [HID, 512], f8, kind="ExternalInput")
    wvh = nc.dram_tensor("wvh", [HID, 512], f8, kind="ExternalInput")
    wvl = nc.dram_tensor("wvl", [HID, 512], f8, kind="ExternalInput")
    woh = nc.dram_tensor("woh", [512, HID], f8, kind="ExternalInput")
    wol = nc.dram_tensor("wol", [512, HID], f8, kind="ExternalInput")
    cosd = nc.dram_tensor("cosT", [128, T], f32, kind="ExternalInput")
    sind = nc.dram_tensor("sinT", [128, T], f32, kind="ExternalInput")
    maskd = nc.dram_tensor("masks", [128, 2048], bf16, kind="ExternalInput")

    outT = nc.dram_tensor("outT", [HID, T], bf16, kind="ExternalOutput")
    abard = nc.dram_tensor("abar", [4, 128, 16], f32, kind="ExternalOutput")
    sumsd = nc.dram_tensor("sums", [4, 2, T], f32, kind="ExternalOutput")

    with tile.TileContext(nc) as tc:
        with (
            tc.tile_pool(name="singles", bufs=1) as singles,
            tc.tile_pool(name="res", bufs=1) as res,
            tc.tile_pool(name="stats", bufs=1) as stats,
        ):
            # --- small constants (no DMA deps) for the PE warm-up ---
            ones_a = singles.tile([128, 2], bf16, tag="onesa")  # [1, 0]
            ones_b = singles.tile([128, 2], bf16, tag="onesb")  # [1, 1]
            nc.vector.memset(ones_a[:, 0:1], 1.0)
            nc.vector.memset(ones_a[:, 1:2], 0.0)
            nc.vector.memset(ones_b, 1.0)
            wsrc2 = singles.tile([128, 512], bf16, tag="wsrc2")
            nc.vector.memset(wsrc2, 0.0)

            cos_sb = singles.tile([128, T], f32, tag="cos")
            sin_sb = singles.tile([128, T], f32, tag="sin")
            mask_sb = singles.tile([128, 4, 512], bf16, tag="mask")

            # --- residents ---
            qt = [res.tile([128, T], bf16, tag=f"qt{h}", name=f"qt{h}")
                  for h in range(HL)]
            kt = [res.tile([128, T], bf16, tag=f"kt{h}", name=f"kt{h}")
                  for h in range(HL)]
            vres = res.tile([128, 16, 512], bf16, tag="vres")
            # attn_out hi/lo fp8, [d, head, t]
            aoh = res.tile([128, HL, T], f8, tag="aoh")
            aol = res.tile([128, HL, T], f8, tag="aol")
            abar_raw = [stats.tile([128, 16], f32, tag=f"ab{p}", name=f"ab{p}")
                        for p in range(HL)]

            # ================= Phase 1: QKV projections + RoPE ================
            with (
                tc.tile_pool(name="wqkv", bufs=1) as wpool,
                tc.tile_pool(name="hsp", bufs=B_HSP) as hsp,
                tc.tile_pool(name="rope", bufs=B_ROPE) as rope,
                tc.tile_pool(name="qkps", bufs=B_QK, space="PSUM") as qkps,
                tc.tile_pool(name="vps", bufs=2, space="PSUM") as vps,
                tc.tile_pool(name="wps", bufs=1, space="PSUM") as wps,
            ):
                wq_sb = [wpool.tile([128, NF, 512], f8, tag=f"wq{i}",
                                    name=f"wq{i}") for i in range(2)]
                wk_sb = [wpool.tile([128, NF, 512], f8, tag=f"wk{i}",
                                    name=f"wk{i}") for i in range(2)]
                wv_sb = [wpool.tile([128, NF, 512], f8, tag=f"wv{i}",
                                    name=f"wv{i}") for i in range(2)]

                # PE warm-up: dependency-free matmuls at t~0 start the
                # p-state ramp while the first DMAs land.
                warm = wps.tile([2, 512], f32, tag="warm")

                def warm_fill(k):
                    for _ in range(k):
                        nc.tensor.matmul(warm, lhsT=ones_a, rhs=wsrc2,
                                         start=True, stop=True)

                warm_fill(int(env.get("BK_NWARM", "4")))

                # DMA issue order is the sync-queue service order; front-load
                # exactly what the first matmuls need (wq_hi + hs_hi chunk 0).
                hs0 = [hsp.tile([128, NF, 512], f8, tag=f"hs{i}",
                                name=f"hs0{i}") for i in range(2)]
                for fh in range(2):
                    for i, (wsrc_d, hsrc_d) in enumerate([(wqh, hsh),
                                                          (wql, hsl)]):
                        rsl = slice(fh * 1024, (fh + 1) * 1024)
                        fsl = slice(fh * 8, (fh + 1) * 8)
                        nc.sync.dma_start(
                            out=wq_sb[i][:, fsl, :],
                            in_=wsrc_d[rsl, :].rearrange(
                                "(nf p) d -> p nf d", p=128))
                        nc.sync.dma_start(
                            out=hs0[i][:, fsl, :],
                            in_=hsrc_d[rsl, 0:512].rearrange(
                                "(nf p) t -> p nf t", p=128))
                nc.sync.dma_start(
                    out=wv_sb[0],
                    in_=wvh.rearrange("(nf p) d -> p nf d", p=128))
                nc.sync.dma_start(
                    out=wv_sb[1],
                    in_=wvl.rearrange("(nf p) d -> p nf d", p=128))
                nc.sync.dma_start(
                    out=wk_sb[0],
                    in_=wkh.rearrange("(nf p) d -> p nf d", p=128))
                nc.sync.dma_start(
                    out=wk_sb[1],
                    in_=wkl.rearrange("(nf p) d -> p nf d", p=128))
                hs1 = [hsp.tile([128, NF, 512], f8, tag=f"hs{i}",
                                name=f"hs1{i}") for i in range(2)]
                nc.sync.dma_start(
                    out=hs1[0],
                    in_=hsh[:, 512:1024].rearrange("(nf p) t -> p nf t",
                                                   p=128))
                nc.sync.dma_start(
                    out=hs1[1],
                    in_=hsl[:, 512:1024].rearrange("(nf p) t -> p nf t",
                                                   p=128))
                nc.gpsimd.dma_start(out=cos_sb, in_=cosd[:, :])
                nc.gpsimd.dma_start(out=sin_sb, in_=sind[:, :])
                nc.gpsimd.dma_start(
                    out=mask_sb, in_=maskd.rearrange("p (v t) -> p v t", v=4))

                def rope_apply(ps, dest, tl):
                    qf = rope.tile([128, 512], f32, tag="qf")
                    nc.scalar.copy(qf, ps)
                    rot = rope.tile([128, 512], f32, tag="rot")
                    nc.gpsimd.dma_start(out=rot[0:64, :], in_=qf[64:128, :])
                    nc.gpsimd.dma_start(out=rot[64:128, :], in_=qf[0:64, :])
                    t1 = rope.tile([128, 512], f32, tag="t1")
                    nc.vector.tensor_mul(t1, rot, sin_sb[:, tl])
                    t2 = rope.tile([128, 512], f32, tag="t2")
                    nc.vector.tensor_mul(t2, qf, cos_sb[:, tl])
                    nc.vector.tensor_add(dest, t1, t2)

                def qk_matmuls(ps, w_pair, hs_pair, h):
                    # 3-term fp8 DoubleRow: hi@hi, lo(w)@hi, hi(w)@lo
                    hsel = slice(h * 128, (h + 1) * 128)
                    terms = [(0, 0), (1, 0), (0, 1)]
                    for ti, (wi, xi) in enumerate(terms):
                        for g in range(NG):
                            nc.tensor.matmul(
                                ps,
                                lhsT=w_pair[wi][:, 2 * g:2 * g + 2, hsel],
                                rhs=hs_pair[xi][:, 2 * g:2 * g + 2, :],
                                start=(ti == 0 and g == 0),
                                stop=(ti == 2 and g == NG - 1),
                                perf_mode=DR)

                for c in range(4):
                    cs = slice(c * 512, (c + 1) * 512)
                    if c == 0:
                        hs_t = hs0
                    elif c == 1:
                        hs_t = hs1
                    else:
                        hs_t = [hsp.tile([128, NF, 512], f8, tag=f"hs{i}",
                                         name=f"hs{i}")
                                for i in range(2)]
                        nc.sync.dma_start(
                            out=hs_t[0],
                            in_=hsh[:, cs].rearrange("(nf p) t -> p nf t",
                                                     p=128))
                        nc.sync.dma_start(
                            out=hs_t[1],
                            in_=hsl[:, cs].rearrange("(nf p) t -> p nf t",
                                                     p=128))
                    # Q (h0-h3) -> V -> K (h0-h3): matches DMA arrivals.
                    # Chunk 0: emit hi@hi g-halves first (first-half DMAs
                    # only), then the lo terms which need wql/hsl.
                    if c == 0:
                        qps = [qkps.tile([128, 512], f32, tag="qk",
                                         name=f"qps{h}") for h in range(HL)]
                        for gh in range(2):
                            for ti, (wi, xi) in enumerate(
                                    [(0, 0), (1, 0), (0, 1)]):
                                for h in range(HL):
                                    for g in range(gh * 4, gh * 4 + 4):
                                        nc.tensor.matmul(
                                            qps[h],
                                            lhsT=wq_sb[wi][
                                                :, 2 * g:2 * g + 2,
                                                h * 128:(h + 1) * 128],
                                            rhs=hs_t[xi][:, 2 * g:2 * g + 2,
                                                         :],
                                            start=(gh == 0 and ti == 0
                                                   and g == 0),
                                            stop=(gh == 1 and ti == 2
                                                  and g == 7),
                                            perf_mode=DR)
                        warm_fill(int(env.get("BK_NW2", "2")))
                        for h in range(HL):
                            rope_apply(qps[h], qt[h][:, cs], cs)
                    else:
                        for h in range(HL):
                            ps = qkps.tile([128, 512], f32, tag="qk")
                            qk_matmuls(ps, wq_sb, hs_t, h)
                            rope_apply(ps, qt[h][:, cs], cs)
                    for s in range(4):
                        vp = vps.tile([128, 512], f32, tag="v")
                        ssel = slice(s * 128, (s + 1) * 128)
                        terms = [(0, 0), (1, 0), (0, 1)]
                        for ti, (xi, wi) in enumerate(terms):
                            for g in range(NG):
                                nc.tensor.matmul(
                                    vp,
                                    lhsT=hs_t[xi][:, 2 * g:2 * g + 2, ssel],
                                    rhs=wv_sb[wi][:, 2 * g:2 * g + 2, :],
                                    start=(ti == 0 and g == 0),
                                    stop=(ti == 2 and g == NG - 1),
                                    perf_mode=DR)
                        nc.scalar.copy(vres[:, c * 4 + s, :], vp)
                        if c <= 1:
                            warm_fill(int(env.get("BK_NW3", "1")))
                    for h in range(HL):
                        ps = qkps.tile([128, 512], f32, tag="qk")
                        qk_matmuls(ps, wk_sb, hs_t, h)
                        rope_apply(ps, kt[h][:, cs], cs)

            # ========== Phase 2+3: attention + interleaved o_proj ==========
            with (
                tc.tile_pool(name="wop", bufs=1) as wop,
                tc.tile_pool(name="sps", bufs=B_SPS, space="PSUM") as sps,
                tc.tile_pool(name="ops", bufs=B_OPS, space="PSUM") as ops,
                tc.tile_pool(name="msc", bufs=B_MSC, space="PSUM") as msc,
                tc.tile_pool(name="smp", bufs=1, space="PSUM") as smp,
                tc.tile_pool(name="pt", bufs=B_PT) as ptp,
                tc.tile_pool(name="pr", bufs=B_PR) as prp,
                tc.tile_pool(name="att_sm", bufs=int(env.get("BK_SM", "3"))) as atsm,
                tc.tile_pool(name="ob", bufs=B_OB) as obp,
            ):
                wo_sb = [wop.tile([128, HL, HID], f8, tag=f"wo{i}",
                                  name=f"wo{i}") for i in range(2)]
                nc.sync.dma_start(
                    out=wo_sb[0], in_=woh.rearrange("(kt p) f -> p kt f",
                                                    p=128))
                nc.sync.dma_start(
                    out=wo_sb[1], in_=wol.rearrange("(kt p) f -> p kt f",
                                                    p=128))

                pending = []
                msc_rot = [msc]  # +sps at the final flush (banks free then)
                ppi = [0]

                def emit_unit():
                    if pending:
                        pending.pop(0)()

                def flush_units():
                    while pending:
                        pending.pop(0)()

                def enqueue_oproj(c, gsz=4):
                    tl = slice(c * 512, (c + 1) * 512)
                    state = {}
                    # GPSIMD cannot read PSUM; mix DVE/Act (Act carries exps)
                    copy_engines = [
                        ((lambda o, i: nc.vector.tensor_scalar_mul(o, i,
                                                                   ODESC))
                         if ch == "v" else
                         (lambda o, i: nc.scalar.mul(o, i, ODESC)))
                        for ch in CPY]

                    def unit(fo):
                        def f():
                            g = fo // gsz
                            if g not in state:
                                state[g] = obp.tile([128, gsz, 512], bf16,
                                                    tag=f"ob{gsz}",
                                                    name="ob_t")
                            ob_t = state[g]
                            fs = slice(fo * 128, (fo + 1) * 128)
                            pool = msc_rot[ppi[0] % len(msc_rot)]
                            ppi[0] += 1
                            pp = pool.tile(
                                [128, 512], f32,
                                tag=("pp" if pool is msc else "s"),
                                name="pp")
                            # 3 terms x 2 head k-pairs, aol term last
                            first = True
                            for wi, src, last in ((0, aoh, False),
                                                  (1, aoh, False),
                                                  (0, aol, True)):
                                for kp in range(2):
                                    ksl = slice(2 * kp, 2 * kp + 2)
                                    nc.tensor.matmul(
                                        pp,
                                        lhsT=wo_sb[wi][:, ksl, fs],
                                        rhs=src[:, ksl, tl],
                                        start=first,
                                        stop=(last and kp == 1),
                                        perf_mode=DR)
                                    first = False
                            copy_engines[fo % len(CPY)](
                                ob_t[:, fo % gsz, :], pp)
                            if fo % gsz == gsz - 1:
                                rows = slice(g * gsz * 128,
                                             (g + 1) * gsz * 128)
                                cg = slice(c * 512, (c + 1) * 512)
                                eng = (nc.gpsimd if (OBQ and g % 2 == 1)
                                       else nc.sync)
                                eng.dma_start(
                                    out=outT[rows, cg].rearrange(
                                        "(nf p) t -> p nf t", p=128),
                                    in_=ob_t)
                        return f

                    for fo in range(16):
                        pending.append(unit(fo))

                # Global step stream: score-matmul lookahead crosses chunk
                # boundaries so the next chunk's exps run during the previous
                # chunk's tail (norm chain / oproj flush) with no PE bubble.
                class Chunk:
                    def __init__(self, p, c):
                        self.p, self.c = p, c  # p = local head
                        self.jmax = 4 * (c + 1)
                        self.o_ps = None
                        self.sm_ps = None
                        self.pts = {}
                        self.sq = []

                def tile_off(ck, j):
                    # Diagonal k-tile v=1..3: first 128v query cols are fully
                    # causal-masked -> compute only cols [128v:512]. Exact.
                    v = j - 4 * ck.c
                    return 128 * v if 1 <= v <= 3 else 0

                def emit_s(ck, j):
                    c = ck.c
                    off = tile_off(ck, j)
                    sp = sps.tile([128, 512], f32, tag="s", name="sp")
                    nc.tensor.matmul(
                        sp[:, off:],
                        lhsT=kt[ck.p][:, j * 128:(j + 1) * 128],
                        rhs=qt[ck.p][:, c * 512 + off:(c + 1) * 512],
                        start=True, stop=True)
                    ck.sq.append(sp)

                def emit_epv(ck, j):
                    p, c = ck.p, ck.c
                    off = tile_off(ck, j)
                    sp = ck.sq[j]
                    pt_t = ptp.tile([128, 512], bf16, tag="p", name="pt_t")
                    nc.scalar.activation(pt_t[:, off:], sp[:, off:],
                                         EXP, scale=SCALE)
                    if j >= 4 * c:
                        nc.vector.tensor_mul(pt_t[:, off:], pt_t[:, off:],
                                             mask_sb[:, j - 4 * c, off:])
                    if c == 3:
                        nc.vector.tensor_copy(
                            abar_raw[p][:, j:j + 1], sp[:, 511:512])
                    if ck.o_ps is None:
                        ck.o_ps = ops.tile([128, 512], f32, tag="o",
                                           name="o_ps")
                    nc.tensor.matmul(
                        ck.o_ps[:, off:],
                        lhsT=vres[:, j, p * 128:(p + 1) * 128],
                        rhs=pt_t[:, off:],
                        start=(j == 0), stop=(j == ck.jmax - 1))
                    if ck.sm_ps is None:
                        ck.sm_ps = smp.tile([2, 512], f32, tag="sm",
                                            name="sm_ps")
                    if j >= 4 * c:
                        # diagonal tile: individual (possibly trimmed) rowsum
                        nc.tensor.matmul(
                            ck.sm_ps[:, off:],
                            lhsT=(ones_b if c == 3 else ones_a),
                            rhs=pt_t[:, off:],
                            start=(j == 4 * c and c == 0),
                            stop=(j == ck.jmax - 1))
                        return
                    ck.pts[j] = pt_t
                    if j % 2 == 1:
                        pr = prp.tile([128, 512], bf16, tag="pr", name="pr")
                        nc.vector.tensor_add(pr, ck.pts[j - 1], ck.pts[j])
                        nc.tensor.matmul(
                            ck.sm_ps,
                            lhsT=ones_a,
                            rhs=pr,
                            start=(j == 1), stop=False)
                        del ck.pts[j - 1], ck.pts[j]

                def epilogue(ck):
                    p, c = ck.p, ck.c
                    cl = slice(c * 512, (c + 1) * 512)
                    rec = atsm.tile([1, 512], f32, tag="rec", name="rec")
                    nc.vector.reciprocal(rec, ck.sm_ps[0:1, :])
                    bc = atsm.tile([128, 512], f32, tag="bc", name="bc")
                    nc.gpsimd.partition_broadcast(bc, rec)
                    full = atsm.tile([128, 512], bf16, tag="full",
                                     name="full")
                    nc.vector.tensor_mul(full, ck.o_ps, bc)
                    # fp8 hi/lo split on Pool (Act does exps, DVE the rest)
                    nc.gpsimd.tensor_copy(aoh[:, p, cl], full)
                    nc.gpsimd.tensor_sub(aol[:, p, cl], full,
                                         aoh[:, p, cl])
                    nc.sync.dma_start(out=sumsd[p, 0:1, cl], in_=rec)
                    if c == 3:
                        tl_sb = atsm.tile([2, 512], f32, tag="smsb",
                                          name="tl_sb")
                        nc.vector.tensor_copy(tl_sb, ck.sm_ps)
                        nc.sync.dma_start(out=sumsd[p, 1:2, cl],
                                          in_=tl_sb[1:2, :])
                        ab_exp = atsm.tile([128, 16], f32, tag="abe",
                                           name="ab_exp")
                        nc.scalar.activation(
                            ab_exp, abar_raw[p], EXP, scale=SCALE)
                        nc.sync.dma_start(out=abard[p], in_=ab_exp)

                chunks = [Chunk(hl, c)
                          for c in range(4) for hl in range(HL)]
                steps = [(ck, j) for ck in chunks for j in range(ck.jmax)]
                for k in range(LOOK):
                    emit_s(*steps[k])
                for i, (ck, j) in enumerate(steps):
                    if i + LOOK < len(steps):
                        emit_s(*steps[i + LOOK])
                    emit_epv(ck, j)
                    if ILV and j % 2 == 1:
                        for _ in range(ILVN):
                            emit_unit()
                    if j == ck.jmax - 1:
                        epilogue(ck)
                        if ck.p == HL - 1:
                            flush_units()
                            enqueue_oproj(ck.c,
                                          gsz=(2 if ck.c == 3 else 4))
                if TROT:
                    # scores done: reuse their banks for the tail flush
                    msc_rot.append(sps)
                flush_units()

    nc.compile()
    return nc


def _get_nc():
    if "nc" not in _NC_CACHE:
        _NC_CACHE["nc"] = build_nc()
    return _NC_CACHE["nc"]


def _split8(x):
    hi = x.astype(F8)
    lo = (x - hi.astype(np.float32)).astype(F8)
    return hi, lo


def _host_inputs(hidden_states, q_w, k_w, v_w, o_w):
    """Per-core input dicts. Core k: batch k//4, heads 4*(k%4)..4*(k%4)+3."""
    inv = 10000.0 ** (-np.arange(64, dtype=np.float64) / 64.0)
    t = np.arange(T, dtype=np.float64)
    fr = t[None, :] * inv[:, None]  # [64, T]
    # 1/WSCL descale of the x64-scaled Q/K baked into the rope tables
    cosT = (np.cos(np.concatenate([fr, fr], 0)) / WSCL).astype(np.float32)
    sinT = (np.sin(np.concatenate([fr, fr], 0)) / WSCL).astype(np.float32)
    sinT[:64] *= -1.0  # sign-baked for swap-halves rotate
    masks = np.zeros((128, 4, 512), dtype=np.float32)
    kk = np.arange(128)[:, None]
    tt = np.arange(512)[None, :]
    for v in range(4):
        masks[:, v, :] = (tt >= 128 * v + kk).astype(np.float32)
    masks = masks.reshape(128, 2048).astype(BF16)

    hs_b = []
    for b in range(B):
        hsT = np.ascontiguousarray(hidden_states[b].T)  # [HID, T]
        hs_b.append(_split8(hsT))
    w_g = []
    for g in range(HG):
        rs = slice(g * 512, (g + 1) * 512)
        w_g.append((
            _split8(WSCL * np.ascontiguousarray(q_w[rs, :].T)),
            _split8(WSCL * np.ascontiguousarray(k_w[rs, :].T)),
            _split8(VSCL * np.ascontiguousarray(v_w[rs, :].T)),
            _split8(WSCL * np.ascontiguousarray(o_w[:, rs].T)),
        ))

    in_maps = []
    for core in range(NCORES):
        b, g = core // HG, core % HG
        (wq_hi, wq_lo), (wk_hi, wk_lo), (wv_hi, wv_lo), (wo_hi, wo_lo) = \
            w_g[g]
        in_maps.append({
            "hsh": hs_b[b][0],
            "hsl": hs_b[b][1],
            "wqh": wq_hi, "wql": wq_lo,
            "wkh": wk_hi, "wkl": wk_lo,
            "wvh": wv_hi, "wvl": wv_lo,
            "woh": wo_hi, "wol": wo_lo,
            "cosT": cosT,
            "sinT": sinT,
            "masks": masks,
        })
    return in_maps


def _epilogue(out, results, hidden_states, v_w, o_w):
    """Add the CaM rank-1 correction per (b, h) on host."""
    for core in range(NCORES):
        r = results[core]
        b = core // HG
        for p in range(HL):
            h = (core % HG) * HL + p
            rec = np.asarray(r["sums"][p][0], np.float64)  # 1/rowsum
            rowsum = 1.0 / np.maximum(rec, 1e-30)
            tails = np.zeros(T, np.float64)
            tails[WS:] = np.asarray(r["sums"][p][1][WS:], np.float64)
            a_exp = np.asarray(r["abar"][p], np.float64).T.reshape(2048)
            a_bar = a_exp / max(float(rowsum[T - 1]), 1e-30)
            avg_w = max(float(np.mean(a_bar[WS:])), 1e-6)
            prob = float(np.clip(a_bar[EVICT] / avg_w, 0.0, 1.0))
            prob = float(np.nan_to_num(prob, nan=0.0, posinf=1.0, neginf=0.0))
            m = 1.0 if U_CONST[b, h] < prob else 0.0
            if m == 0.0:
                continue
            # exact v_e from fp32 inputs
            v_row = hidden_states[b, EVICT, :] @ v_w[h * D:(h + 1) * D, :].T
            v_e = v_row * (m / RB)  # [D]
            w_e = o_w[:, h * D:(h + 1) * D] @ v_e  # [HID]
            s_tail = (tails / np.maximum(rowsum, 1e-30)).astype(np.float32)
            out[b] += np.outer(s_tail, w_e).astype(np.float32)
    return out


def kernel(hidden_states, attention_mask, q_w, k_w, v_w, o_w):
    from concourse.bass_utils import run_bass_kernel_spmd

    nc = _get_nc()
    in_maps = _host_inputs(hidden_states, q_w, k_w, v_w, o_w)
    trace = bool(int(os.environ.get("BK_TRACE", "0")))
    res = run_bass_kernel_spmd(
        nc, in_maps, core_ids=list(range(NCORES)), trace=trace,
    )
    if trace and res.exec_time_ns is not None:
        print(f"HW exec time: {res.exec_time_ns} ns")
        _NC_CACHE["last_exec_ns"] = res.exec_time_ns
        _NC_CACHE["last_trace"] = res.instructions_and_trace
    results = res.results

    out = np.zeros((B, T, HID), dtype=np.float32)
    for core in range(NCORES):
        b = core // HG
        out[b] += np.asarray(results[core]["outT"], np.float32).T
    out = _epilogue(out, results, hidden_states, v_w, o_w)
    return out.astype(np.float32)


# revision 37
# speedup vs baseline: 1.1958x; 1.0459x over previous
"""Trainium2 Bass kernel for nn_LlamaAttention_cam (sparse_attention).

Sharding: 8 cores = 2 batches x 4 head-groups. Core k handles batch k//4
and heads 4*(k%4)..4*(k%4)+3, so each core streams only its batch's
hidden_states (half the DMA of batch-replicated sharding). Q/K/V
projections column-parallel over heads; o_proj row-parallel within each
batch group (4-core partial sums added on host). The CaM merge is a
rank-1 correction (s_tail outer v_e) applied on host from tiny
device-side statistics.

The projection GEMMs (QKV + o_proj) run as fp8e4 DoubleRow matmuls with
3-term error compensation: X*W ~ Xh*Wh + Xl*Wh + Xh*Wl where Xh = fp8(X),
Xl = fp8(X - Xh). DoubleRow packs a 256-deep contraction at 0.5 cyc/col,
so 3 terms cost 75% of the bf16 equivalent. hs and all weights are split
on the host (free); attn_out is split on-device. Weights are pre-scaled
into fp8 range (x64, V x16); the o_proj epilogue copy descales by 1/1024.

Self-contained: hardcodes all shapes; takes full inputs, returns full output.
"""

import math
import os

import numpy as np
import ml_dtypes

B, T, HID, H = 2, 2048, 2048, 16
D = 128
NCORES = 8
HL = 4  # heads per core
HG = H // HL  # head groups = 4
NF = HID // 128  # 16 f-tiles
NG = NF // 2  # 8 f-tile pairs for DoubleRow
SCALE = 1.0 / math.sqrt(D)
RB = int(0.25 * T)  # 512 recent budget
WS = T - RB  # 1536
EVICT = WS - 1  # 1535
WSCL = 64.0  # fp8 pre-scale on wq/wk/wo
VSCL = 16.0  # fp8 pre-scale on wv: max |attn_out*VSCL| ~ 5sigma*16 = 72 < 240
ODESC = 1.0 / (VSCL * WSCL)  # o_proj descale: V carries x16, wo carries x64

# jax.random.uniform(jax.random.key(42), (2,16), float32); bernoulli(key,p) == u < p
U_CONST = np.array(
    [[0.59400654, 0.43801308, 0.6285691, 0.00791204, 0.27834702,
      0.7976179, 0.8521497, 0.9625306, 0.67656493, 0.11104441,
      0.4959929, 0.7311437, 0.18970704, 0.1544199, 0.03802836,
      0.33559263],
     [0.92825687, 0.6123972, 0.49262476, 0.733806, 0.18920851,
      0.15386605, 0.037136197, 0.32930005, 0.9372028, 0.5957513,
      0.4615929, 0.6695677, 0.07019377, 0.39408123, 0.55786455,
      0.35412872]], dtype=np.float32)

BF16 = ml_dtypes.bfloat16
F8 = ml_dtypes.float8_e4m3

_NC_CACHE = {}


def build_nc():
    import concourse.bacc as bacc
    import concourse.mybir as mybir
    import concourse.tile as tile

    f32 = mybir.dt.float32
    f16 = mybir.dt.float16
    bf16 = mybir.dt.bfloat16
    f8 = mybir.dt.float8e4
    EXP = mybir.ActivationFunctionType.Exp
    DR = mybir.MatmulPerfMode.DoubleRow

    nc = bacc.Bacc("TRN2", target_bir_lowering=False, debug=False)
    env = os.environ
    B_QK = int(env.get("BK_QK", "3"))
    B_HSP = int(env.get("BK_HSP", "2"))
    B_ROPE = int(env.get("BK_ROPE", "4"))
    B_SPS = int(env.get("BK_SPS", "3"))
    B_OPS = int(env.get("BK_OPS", "2"))
    B_MSC = int(env.get("BK_MSC", "2"))
    B_PT = int(env.get("BK_PT", "34"))
    B_PR = int(env.get("BK_PR", "7"))
    B_OB = int(env.get("BK_OB", "6"))
    LOOK = int(env.get("BK_LOOK", "28"))
    ILV = env.get("BK_ILV", "1") == "1"
    CPY = env.get("BK_CPY", "vvvs")  # per-fo copy engine cycle: v=DVE s=Act
    OBQ = env.get("BK_OBQ", "0") == "1"  # alternate ob DMA queues
    ILVN = int(env.get("BK_ILVN", "1"))  # oproj units per 2 j-steps
    TROT = env.get("BK_TROT", "1") == "1"
    SWQ = int(env.get("BK_SWQ", "3"))  # rope swaps via sync HWDGE from c>=SWQ

    hsh = nc.dram_tensor("hsh", [HID, T], f8, kind="ExternalInput")
    hsl = nc.dram_tensor("hsl", [HID, T], f8, kind="ExternalInput")
    wqh = nc.dram_tensor("wqh", [HID, 512], f8, kind="ExternalInput")
    wql = nc.dram_tensor("wql", [HID, 512], f8, kind="ExternalInput")
    wkh = nc.dram_tensor("wkh", [HID, 512], f8, kind="ExternalInput")
    wkl = nc.dram_tensor("wkl", [HID, 512], f8, kind="ExternalInput")
    wvh = nc.dram_tensor("wvh", [HID, 512], f8, kind="ExternalInput")
    wvl = nc.dram_tensor("wvl", [HID, 512], f8, kind="ExternalInput")
    woh = nc.dram_tensor("woh", [512, HID], f8, kind="ExternalInput")
    wol = nc.dram_tensor("wol", [512, HID], f8, kind="ExternalInput")
    cosd = nc.dram_tensor("cosT", [128, T], f32, kind="ExternalInput")
    sind = nc.dram_tensor("sinT", [128, T], f32, kind="ExternalInput")
    maskd = nc.dram_tensor("masks", [128, 2048], bf16, kind="ExternalInput")

    outT = nc.dram_tensor("outT", [HID, T], bf16, kind="ExternalOutput")
    abard = nc.dram_tensor("abar", [4, 128, 16], f32, kind="ExternalOutput")
    sumsd = nc.dram_tensor("sums", [4, 2, T], f32, kind="ExternalOutput")

    with tile.TileContext(nc) as tc:
        with (
            tc.tile_pool(name="singles", bufs=1) as singles,
            tc.tile_pool(name="res", bufs=1) as res,
            tc.tile_pool(name="stats", bufs=1) as stats,
        ):
            # --- small constants (no DMA deps) for the PE warm-up ---
            ones_a = singles.tile([128, 2], bf16, tag="onesa")  # [1, 0]
            ones_b = singles.tile([128, 2], bf16, tag="onesb")  # [1, 1]
            nc.vector.memset(ones_a[:, 0:1], 1.0)
            nc.vector.memset(ones_a[:, 1:2], 0.0)
            nc.vector.memset(ones_b, 1.0)
            wsrc2 = singles.tile([128, 512], bf16, tag="wsrc2")
            nc.vector.memset(wsrc2, 0.0)
            ones_h = singles.tile([128, 2], f16, tag="onesh")  # [1, 0]
            nc.vector.memset(ones_h[:, 0:1], 1.0)
            nc.vector.memset(ones_h[:, 1:2], 0.0)

            cos_sb = singles.tile([128, T], f32, tag="cos")
            sin_sb = singles.tile([128, T], f32, tag="sin")
            mask_sb = singles.tile([128, 4, 512], bf16, tag="mask")

            # --- residents ---
            qt = [res.tile([128, T], bf16, tag=f"qt{h}", name=f"qt{h}")
                  for h in range(HL)]
            kt = [res.tile([128, T], bf16, tag=f"kt{h}", name=f"kt{h}")
                  for h in range(HL)]
            vres = res.tile([128, 16, 512], bf16, tag="vres")
            # attn_out hi/lo fp8, [d, head, t]
            aoh = res.tile([128, HL, T], f8, tag="aoh")
            aol = res.tile([128, HL, T], f8, tag="aol")
            abar_raw = [stats.tile([128, 16], f32, tag=f"ab{p}", name=f"ab{p}")
                        for p in range(HL)]

            # The score pool owns its PSUM banks for the whole kernel so
            # the lookahead score matmuls at the phase boundary don't stall
            # on released-zone deps from the phase-1 pools.
            _sps_cm = tc.tile_pool(name="sps", bufs=B_SPS, space="PSUM")
            sps = _sps_cm.__enter__()

            # ================= Phase 1: QKV projections + RoPE ================
            with (
                tc.tile_pool(name="wqkv", bufs=1) as wpool,
                tc.tile_pool(name="hsp", bufs=B_HSP) as hsp,
                tc.tile_pool(name="rope", bufs=B_ROPE) as rope,
                tc.tile_pool(name="qkps", bufs=B_QK, space="PSUM") as qkps,
                tc.tile_pool(name="vps", bufs=2, space="PSUM") as vps,
            ):
                wq_sb = [wpool.tile([128, NF, 512], f8, tag=f"wq{i}",
                                    name=f"wq{i}") for i in range(2)]
                wk_sb = [wpool.tile([128, NF, 512], f8, tag=f"wk{i}",
                                    name=f"wk{i}") for i in range(2)]
                wv_sb = [wpool.tile([128, NF, 512], f8, tag=f"wv{i}",
                                    name=f"wv{i}") for i in range(2)]

                # PE warm-up: dependency-free matmuls at t~0 start the
                # p-state ramp while the first DMAs land.
                warm = sps.tile([128, 512], f32, tag="s", name="warm")

                def warm_fill(k):
                    for _ in range(k):
                        nc.tensor.matmul(warm[0:2, :], lhsT=ones_a,
                                         rhs=wsrc2, start=True, stop=True)

                warm_fill(int(env.get("BK_NWARM", "4")))

                # DMA issue order is the sync-queue service order; front-load
                # exactly what the first matmuls need (wq_hi + hs_hi chunk 0).
                hs0 = [hsp.tile([128, NF, 512], f8, tag=f"hs{i}",
                                name=f"hs0{i}") for i in range(2)]
                for fh in range(2):
                    for i, (wsrc_d, hsrc_d) in enumerate([(wqh, hsh),
                                                          (wql, hsl)]):
                        rsl = slice(fh * 1024, (fh + 1) * 1024)
                        fsl = slice(fh * 8, (fh + 1) * 8)
                        nc.sync.dma_start(
                            out=wq_sb[i][:, fsl, :],
                            in_=wsrc_d[rsl, :].rearrange(
                                "(nf p) d -> p nf d", p=128))
                        nc.sync.dma_start(
                            out=hs0[i][:, fsl, :],
                            in_=hsrc_d[rsl, 0:512].rearrange(
                                "(nf p) t -> p nf t", p=128))
                nc.sync.dma_start(
                    out=wv_sb[0],
                    in_=wvh.rearrange("(nf p) d -> p nf d", p=128))
                nc.sync.dma_start(
                    out=wv_sb[1],
                    in_=wvl.rearrange("(nf p) d -> p nf d", p=128))
                nc.sync.dma_start(
                    out=wk_sb[0],
                    in_=wkh.rearrange("(nf p) d -> p nf d", p=128))
                nc.sync.dma_start(
                    out=wk_sb[1],
                    in_=wkl.rearrange("(nf p) d -> p nf d", p=128))
                hs1 = [hsp.tile([128, NF, 512], f8, tag=f"hs{i}",
                                name=f"hs1{i}") for i in range(2)]
                nc.sync.dma_start(
                    out=hs1[0],
                    in_=hsh[:, 512:1024].rearrange("(nf p) t -> p nf t",
                                                   p=128))
                nc.sync.dma_start(
                    out=hs1[1],
                    in_=hsl[:, 512:1024].rearrange("(nf p) t -> p nf t",
                                                   p=128))
                nc.gpsimd.dma_start(out=cos_sb, in_=cosd[:, :])
                nc.gpsimd.dma_start(out=sin_sb, in_=sind[:, :])
                nc.gpsimd.dma_start(
                    out=mask_sb, in_=maskd.rearrange("p (v t) -> p v t", v=4))

                def rope_apply(ps, dest, tl, swap_eng=nc.gpsimd):
                    qf = rope.tile([128, 512], f32, tag="qf")
                    nc.scalar.copy(qf, ps)
                    rot = rope.tile([128, 512], f32, tag="rot")
                    swap_eng.dma_start(out=rot[0:64, :], in_=qf[64:128, :])
                    swap_eng.dma_start(out=rot[64:128, :], in_=qf[0:64, :])
                    t1 = rope.tile([128, 512], f32, tag="t1")
                    nc.vector.tensor_mul(t1, rot, sin_sb[:, tl])
                    t2 = rope.tile([128, 512], f32, tag="t2")
                    nc.vector.tensor_mul(t2, qf, cos_sb[:, tl])
                    nc.vector.tensor_add(dest, t1, t2)

                def qk_matmuls(ps, w_pair, hs_pair, h):
                    # 3-term fp8 DoubleRow: hi@hi, lo(w)@hi, hi(w)@lo
                    hsel = slice(h * 128, (h + 1) * 128)
                    terms = [(0, 0), (1, 0), (0, 1)]
                    for ti, (wi, xi) in enumerate(terms):
                        for g in range(NG):
                            nc.tensor.matmul(
                                ps,
                                lhsT=w_pair[wi][:, 2 * g:2 * g + 2, hsel],
                                rhs=hs_pair[xi][:, 2 * g:2 * g + 2, :],
                                start=(ti == 0 and g == 0),
                                stop=(ti == 2 and g == NG - 1),
                                perf_mode=DR)

                for c in range(4):
                    cs = slice(c * 512, (c + 1) * 512)
                    if c == 0:
                        hs_t = hs0
                    elif c == 1:
                        hs_t = hs1
                    else:
                        hs_t = [hsp.tile([128, NF, 512], f8, tag=f"hs{i}",
                                         name=f"hs{i}")
                                for i in range(2)]
                        nc.sync.dma_start(
                            out=hs_t[0],
                            in_=hsh[:, cs].rearrange("(nf p) t -> p nf t",
                                                     p=128))
                        nc.sync.dma_start(
                            out=hs_t[1],
                            in_=hsl[:, cs].rearrange("(nf p) t -> p nf t",
                                                     p=128))
                    # Q (h0-h3) -> V -> K (h0-h3): matches DMA arrivals.
                    # Chunk 0: emit hi@hi g-halves first (first-half DMAs
                    # only), then the lo terms which need wql/hsl.
                    if c == 0:
                        qps = [qkps.tile([128, 512], f32, tag="qk",
                                         name=f"qps{h}") for h in range(HL)]
                        for gh in range(2):
                            for ti, (wi, xi) in enumerate(
                                    [(0, 0), (1, 0), (0, 1)]):
                                for h in range(HL):
                                    for g in range(gh * 4, gh * 4 + 4):
                                        nc.tensor.matmul(
                                            qps[h],
                                            lhsT=wq_sb[wi][
                                                :, 2 * g:2 * g + 2,
                                                h * 128:(h + 1) * 128],
                                            rhs=hs_t[xi][:, 2 * g:2 * g + 2,
                                                         :],
                                            start=(gh == 0 and ti == 0
                                                   and g == 0),
                                            stop=(gh == 1 and ti == 2
                                                  and g == 7),
                                            perf_mode=DR)
                        warm_fill(int(env.get("BK_NW2", "2")))
                        for h in range(HL):
                            rope_apply(qps[h], qt[h][:, cs], cs)
                    else:
                        for h in range(HL):
                            ps = qkps.tile([128, 512], f32, tag="qk")
                            qk_matmuls(ps, wq_sb, hs_t, h)
                            rope_apply(ps, qt[h][:, cs], cs,
                                       nc.gpsimd if c < SWQ else nc.sync)
                    for s in range(4):
                        vp = vps.tile([128, 512], f32, tag="v")
                        ssel = slice(s * 128, (s + 1) * 128)
                        terms = [(0, 0), (1, 0), (0, 1)]
                        for ti, (xi, wi) in enumerate(terms):
                            for g in range(NG):
                                nc.tensor.matmul(
                                    vp,
                                    lhsT=hs_t[xi][:, 2 * g:2 * g + 2, ssel],
                                    rhs=wv_sb[wi][:, 2 * g:2 * g + 2, :],
                                    start=(ti == 0 and g == 0),
                                    stop=(ti == 2 and g == NG - 1),
                                    perf_mode=DR)
                        nc.scalar.copy(vres[:, c * 4 + s, :], vp)
                        if c <= 1:
                            warm_fill(int(env.get("BK_NW3", "1")))
                    for h in range(HL):
                        ps = qkps.tile([128, 512], f32, tag="qk")
                        qk_matmuls(ps, wk_sb, hs_t, h)
                        rope_apply(ps, kt[h][:, cs], cs,
                                   nc.gpsimd if c < SWQ else nc.sync)

            # ========== Phase 2+3: attention + interleaved o_proj ==========
            with (
                tc.tile_pool(name="wop", bufs=1) as wop,
                tc.tile_pool(name="ops", bufs=B_OPS, space="PSUM") as ops,
                tc.tile_pool(name="msc", bufs=B_MSC, space="PSUM") as msc,
                tc.tile_pool(name="smp", bufs=1, space="PSUM") as smp,
                tc.tile_pool(name="pt", bufs=B_PT) as ptp,
                tc.tile_pool(name="pr", bufs=B_PR) as prp,
                tc.tile_pool(name="att_sm", bufs=int(env.get("BK_SM", "3"))) as atsm,
                tc.tile_pool(name="ob", bufs=B_OB) as obp,
            ):
                wo_sb = [wop.tile([128, HL, HID], f8, tag=f"wo{i}",
                                  name=f"wo{i}") for i in range(2)]
                nc.sync.dma_start(
                    out=wo_sb[0], in_=woh.rearrange("(kt p) f -> p kt f",
                                                    p=128))
                nc.sync.dma_start(
                    out=wo_sb[1], in_=wol.rearrange("(kt p) f -> p kt f",
                                                    p=128))

                pending = []
                msc_rot = [msc]  # +sps at the final flush (banks free then)
                ppi = [0]

                def emit_unit():
                    if pending:
                        pending.pop(0)()

                def flush_units():
                    while pending:
                        pending.pop(0)()

                def enqueue_oproj(c, gsz=4):
                    tl = slice(c * 512, (c + 1) * 512)
                    state = {}
                    # GPSIMD cannot read PSUM; mix DVE/Act (Act carries exps)
                    copy_engines = [
                        ((lambda o, i: nc.vector.tensor_scalar_mul(o, i,
                                                                   ODESC))
                         if ch == "v" else
                         (lambda o, i: nc.scalar.mul(o, i, ODESC)))
                        for ch in CPY]

                    def unit(fo):
                        def f():
                            g = fo // gsz
                            if g not in state:
                                state[g] = obp.tile([128, gsz, 512], bf16,
                                                    tag=f"ob{gsz}",
                                                    name="ob_t")
                            ob_t = state[g]
                            fs = slice(fo * 128, (fo + 1) * 128)
                            pool = msc_rot[ppi[0] % len(msc_rot)]
                            ppi[0] += 1
                            pp = pool.tile(
                                [128, 512], f32,
                                tag=("pp" if pool is msc else "s"),
                                name="pp")
                            # 3 terms x 2 head k-pairs, aol term last
                            first = True
                            for wi, src, last in ((0, aoh, False),
                                                  (1, aoh, False),
                                                  (0, aol, True)):
                                for kp in range(2):
                                    ksl = slice(2 * kp, 2 * kp + 2)
                                    nc.tensor.matmul(
                                        pp,
                                        lhsT=wo_sb[wi][:, ksl, fs],
                                        rhs=src[:, ksl, tl],
                                        start=first,
                                        stop=(last and kp == 1),
                                        perf_mode=DR)
                                    first = False
                            copy_engines[fo % len(CPY)](
                                ob_t[:, fo % gsz, :], pp)
                            if fo % gsz == gsz - 1:
                                rows = slice(g * gsz * 128,
                                             (g + 1) * gsz * 128)
                                cg = slice(c * 512, (c + 1) * 512)
                                eng = (nc.gpsimd if (OBQ and g % 2 == 1)
                                       else nc.sync)
                                eng.dma_start(
                                    out=outT[rows, cg].rearrange(
                                        "(nf p) t -> p nf t", p=128),
                                    in_=ob_t)
                        return f

                    for fo in range(16):
                        pending.append(unit(fo))

                # Global step stream: score-matmul lookahead crosses chunk
                # boundaries so the next chunk's exps run during the previous
                # chunk's tail (norm chain / oproj flush) with no PE bubble.
                class Chunk:
                    def __init__(self, p, c):
                        self.p, self.c = p, c  # p = local head
                        self.jmax = 4 * (c + 1)
                        self.o_ps = None
                        self.sm_ps = None
                        self.racc = None
                        self.pts = {}
                        self.sq = []

                def tile_off(ck, j):
                    # Diagonal k-tile v=1..3: first 128v query cols are fully
                    # causal-masked -> compute only cols [128v:512]. Exact.
                    v = j - 4 * ck.c
                    return 128 * v if 1 <= v <= 3 else 0

                def emit_s(ck, j):
                    c = ck.c
                    off = tile_off(ck, j)
                    sp = sps.tile([128, 512], f32, tag="s", name="sp")
                    nc.tensor.matmul(
                        sp[:, off:],
                        lhsT=kt[ck.p][:, j * 128:(j + 1) * 128],
                        rhs=qt[ck.p][:, c * 512 + off:(c + 1) * 512],
                        start=True, stop=True)
                    ck.sq.append(sp)

                def emit_epv(ck, j):
                    p, c = ck.p, ck.c
                    off = tile_off(ck, j)
                    sp = ck.sq[j]
                    pt_t = ptp.tile([128, 512], bf16, tag="p", name="pt_t")
                    nc.scalar.activation(pt_t[:, off:], sp[:, off:],
                                         EXP, scale=SCALE)
                    if j >= 4 * c:
                        nc.vector.tensor_mul(pt_t[:, off:], pt_t[:, off:],
                                             mask_sb[:, j - 4 * c, off:])
                    if c == 3:
                        nc.vector.tensor_copy(
                            abar_raw[p][:, j:j + 1], sp[:, 511:512])
                    if ck.o_ps is None:
                        ck.o_ps = ops.tile([128, 512], f32, tag="o",
                                           name="o_ps")
                    nc.tensor.matmul(
                        ck.o_ps[:, off:],
                        lhsT=vres[:, j, p * 128:(p + 1) * 128],
                        rhs=pt_t[:, off:],
                        start=(j == 0), stop=(j == ck.jmax - 1))
                    if ck.sm_ps is None:
                        ck.sm_ps = smp.tile([2, 512], f32, tag="sm",
                                            name="sm_ps")
                    if j >= 4 * c:
                        # diagonal tile: individual (possibly trimmed) rowsum
                        nc.tensor.matmul(
                            ck.sm_ps[:, off:],
                            lhsT=(ones_b if c == 3 else ones_a),
                            rhs=pt_t[:, off:],
                            start=(j == 4 * c and c == 0),
                            stop=(j == ck.jmax - 1))
                        return
                    ck.pts[j] = pt_t
                    if j % 2 == 1:
                        # pair-sum in fp16 (0.05% step), tree-accumulate on
                        # DVE; a single 512-col PE matmul per chunk folds the
                        # running sum into sm_ps.
                        pr = prp.tile([128, 512], f16, tag="pr", name="pr")
                        nc.vector.tensor_add(pr, ck.pts[j - 1], ck.pts[j])
                        del ck.pts[j - 1], ck.pts[j]
                        if ck.racc is None:
                            ck.racc = pr
                        else:
                            nc.vector.tensor_add(ck.racc, ck.racc, pr)
                        if j == 4 * c - 1:
                            nc.tensor.matmul(
                                ck.sm_ps,
                                lhsT=ones_h,
                                rhs=ck.racc,
                                start=True, stop=False)

                def epilogue(ck):
                    p, c = ck.p, ck.c
                    cl = slice(c * 512, (c + 1) * 512)
                    rec = atsm.tile([1, 512], f32, tag="rec", name="rec")
                    nc.vector.reciprocal(rec, ck.sm_ps[0:1, :])
                    bc = atsm.tile([128, 512], f32, tag="bc", name="bc")
                    nc.gpsimd.partition_broadcast(bc, rec)
                    full = atsm.tile([128, 512], bf16, tag="full",
                                     name="full")
                    nc.vector.tensor_mul(full, ck.o_ps, bc)
                    # fp8 hi/lo split on Pool (Act does exps, DVE the rest)
                    nc.gpsimd.tensor_copy(aoh[:, p, cl], full)
                    nc.gpsimd.tensor_sub(aol[:, p, cl], full,
                                         aoh[:, p, cl])
                    nc.sync.dma_start(out=sumsd[p, 0:1, cl], in_=rec)
                    if c == 3:
                        tl_sb = atsm.tile([2, 512], f32, tag="smsb",
                                          name="tl_sb")
                        nc.vector.tensor_copy(tl_sb, ck.sm_ps)
                        nc.sync.dma_start(out=sumsd[p, 1:2, cl],
                                          in_=tl_sb[1:2, :])
                        ab_exp = atsm.tile([128, 16], f32, tag="abe",
                                           name="ab_exp")
                        nc.scalar.activation(
                            ab_exp, abar_raw[p], EXP, scale=SCALE)
                        nc.sync.dma_start(out=abard[p], in_=ab_exp)

                corder = [int(x) for x in env.get("BK_CORD", "1023")]
                chunks = [Chunk(hl, c)
                          for c in corder for hl in range(HL)]
                last_c = corder[-1]
                steps = [(ck, j) for ck in chunks for j in range(ck.jmax)]
                for k in range(LOOK):
                    emit_s(*steps[k])
                for i, (ck, j) in enumerate(steps):
                    if i + LOOK < len(steps):
                        emit_s(*steps[i + LOOK])
                    emit_epv(ck, j)
                    if ILV and j % 2 == 1:
                        for _ in range(ILVN):
                            emit_unit()
                    if j == ck.jmax - 1:
                        epilogue(ck)
                        if ck.p == HL - 1:
                            flush_units()
                            enqueue_oproj(ck.c,
                                          gsz=(2 if ck.c == last_c else 4))
                if TROT:
                    # scores done: reuse their banks for the tail flush
                    msc_rot.append(sps)
                flush_units()
            _sps_cm.__exit__(None, None, None)

    nc.compile()
    return nc


def _get_nc():
    if "nc" not in _NC_CACHE:
        _NC_CACHE["nc"] = build_nc()
    return _NC_CACHE["nc"]


def _split8(x):
    hi = x.astype(F8)
    lo = (x - hi.astype(np.float32)).astype(F8)
    return hi, lo


def _host_inputs(hidden_states, q_w, k_w, v_w, o_w):
    """Per-core input dicts. Core k: batch k//4, heads 4*(k%4)..4*(k%4)+3."""
    inv = 10000.0 ** (-np.arange(64, dtype=np.float64) / 64.0)
    t = np.arange(T, dtype=np.float64)
    fr = t[None, :] * inv[:, None]  # [64, T]
    # 1/WSCL descale of the x64-scaled Q/K baked into the rope tables
    cosT = (np.cos(np.concatenate([fr, fr], 0)) / WSCL).astype(np.float32)
    sinT = (np.sin(np.concatenate([fr, fr], 0)) / WSCL).astype(np.float32)
    sinT[:64] *= -1.0  # sign-baked for swap-halves rotate
    masks = np.zeros((128, 4, 512), dtype=np.float32)
    kk = np.arange(128)[:, None]
    tt = np.arange(512)[None, :]
    for v in range(4):
        masks[:, v, :] = (tt >= 128 * v + kk).astype(np.float32)
    masks = masks.reshape(128, 2048).astype(BF16)

    hs_b = []
    for b in range(B):
        hsT = np.ascontiguousarray(hidden_states[b].T)  # [HID, T]
        hs_b.append(_split8(hsT))
    w_g = []
    for g in range(HG):
        rs = slice(g * 512, (g + 1) * 512)
        w_g.append((
            _split8(WSCL * np.ascontiguousarray(q_w[rs, :].T)),
            _split8(WSCL * np.ascontiguousarray(k_w[rs, :].T)),
            _split8(VSCL * np.ascontiguousarray(v_w[rs, :].T)),
            _split8(WSCL * np.ascontiguousarray(o_w[:, rs].T)),
        ))

    in_maps = []
    for core in range(NCORES):
        b, g = core // HG, core % HG
        (wq_hi, wq_lo), (wk_hi, wk_lo), (wv_hi, wv_lo), (wo_hi, wo_lo) = \
            w_g[g]
        in_maps.append({
            "hsh": hs_b[b][0],
            "hsl": hs_b[b][1],
            "wqh": wq_hi, "wql": wq_lo,
            "wkh": wk_hi, "wkl": wk_lo,
            "wvh": wv_hi, "wvl": wv_lo,
            "woh": wo_hi, "wol": wo_lo,
            "cosT": cosT,
            "sinT": sinT,
            "masks": masks,
        })
    return in_maps


def _epilogue(out, results, hidden_states, v_w, o_w):
    """Add the CaM rank-1 correction per (b, h) on host."""
    for core in range(NCORES):
        r = results[core]
        b = core // HG
        for p in range(HL):
            h = (core % HG) * HL + p
            rec = np.asarray(r["sums"][p][0], np.float64)  # 1/rowsum
            rowsum = 1.0 / np.maximum(rec, 1e-30)
            tails = np.zeros(T, np.float64)
            tails[WS:] = np.asarray(r["sums"][p][1][WS:], np.float64)
            a_exp = np.asarray(r["abar"][p], np.float64).T.reshape(2048)
            a_bar = a_exp / max(float(rowsum[T - 1]), 1e-30)
            avg_w = max(float(np.mean(a_bar[WS:])), 1e-6)
            prob = float(np.clip(a_bar[EVICT] / avg_w, 0.0, 1.0))
            prob = float(np.nan_to_num(prob, nan=0.0, posinf=1.0, neginf=0.0))
            m = 1.0 if U_CONST[b, h] < prob else 0.0
            if m == 0.0:
                continue
            # exact v_e from fp32 inputs
            v_row = hidden_states[b, EVICT, :] @ v_w[h * D:(h + 1) * D, :].T
            v_e = v_row * (m / RB)  # [D]
            w_e = o_w[:, h * D:(h + 1) * D] @ v_e  # [HID]
            s_tail = (tails / np.maximum(rowsum, 1e-30)).astype(np.float32)
            out[b] += np.outer(s_tail, w_e).astype(np.float32)
    return out


def kernel(hidden_states, attention_mask, q_w, k_w, v_w, o_w):
    from concourse.bass_utils import run_bass_kernel_spmd

    nc = _get_nc()
    in_maps = _host_inputs(hidden_states, q_w, k_w, v_w, o_w)
    trace = bool(int(os.environ.get("BK_TRACE", "0")))
    res = run_bass_kernel_spmd(
        nc, in_maps, core_ids=list(range(NCORES)), trace=trace,
    )
    if trace and res.exec_time_ns is not None:
        print(f"HW exec time: {res.exec_time_ns} ns")
        _NC_CACHE["last_exec_ns"] = res.exec_time_ns
        _NC_CACHE["last_trace"] = res.instructions_and_trace
    results = res.results

    out = np.zeros((B, T, HID), dtype=np.float32)
    for core in range(NCORES):
        b = core // HG
        out[b] += np.asarray(results[core]["outT"], np.float32).T
    out = _epilogue(out, results, hidden_states, v_w, o_w)
    return out.astype(np.float32)
